# revision 1
# baseline (speedup 1.0000x reference)
"""Trainium2 Bass kernel for a custom attention block (qkv-proj + LN(q,k) +
RoPE + causal attention + out-proj), distributed over 8 NeuronCores.

Sharding: 2 cores per batch (B=4). Core role r=c%2 takes q-token blocks
{0,3} (r=0) or {1,2} (r=1) of 512 tokens; every core computes K/V for the
full 2048-token sequence of its batch (no collectives). The compiled
program is identical on all cores; all per-core differences are input
data (sliced x^T, cos/sin tables, causal masks).

Orientation: q^T / k^T are produced feature-on-partition ([hd, tokens]),
v token-on-partition. Attention computes s^T = (k^T)^T-slice @ q^T with
fp32r matmuls (full PE rate at moving-dim >= 256), exp(s - 8), mask
multiply (data-driven), PV as lhsT=v rhs=exp -> out^T, denominator via a
ones-column matmul, so no on-chip transposes are needed anywhere.

LN: mean subtraction is folded into host-pre-centered w_in rows; variance
comes from Square + ones-matmul partition reduction; rsqrt(var+eps) is
computed as Exp(-0.5*Ln(var+eps)) so all ACT functions live in one table
set (natural_log_exp_and_others).
"""

import math

import numpy as np

import concourse.bass as bass
import concourse.mybir as mybir
import concourse.tile as tile
from concourse import bacc
from concourse.bass import ds, ts

F32 = mybir.dt.float32
F32R = mybir.dt.float32r
AF = mybir.ActivationFunctionType
OP = mybir.AluOpType

P = 128
HD = 128

FULL_CFG = dict(
    D=2048,           # model dim (contraction dim for projections)
    S=2048,           # kv tokens per core (full sequence of its batch)
    NQTOK=1024,       # q tokens per core
    PT=256,           # projection s-tile width (moving dim)
    QT=512,           # attention q-tile width (moving dim)
    slots=(8, 16),    # kv 128-chunks visited per q-tile
    masked=(tuple(range(0, 8)), tuple(range(8, 16))),  # slots that get a mask
    EXP_BIAS=8.0,
    EPS=1e-5,
    MASK_F32=False,
)

SMALL_CFG = dict(
    D=512,
    S=1024,
    NQTOK=1024,
    PT=256,
    QT=512,
    slots=(8, 8),
    masked=(tuple(range(0, 8)), tuple(range(4, 8))),
    EXP_BIAS=8.0,
    EPS=1e-5,
    MASK_F32=False,
)


def _r(ap):
    """fp32 -> fp32r view for matmul operands."""
    return ap.bitcast(F32R)


def build_program(cfg):
    D = cfg["D"]
    S = cfg["S"]
    NQTOK = cfg["NQTOK"]
    PT = cfg["PT"]
    QT = cfg["QT"]
    slots = cfg["slots"]
    masked = cfg["masked"]
    EXP_BIAS = cfg["EXP_BIAS"]
    EPS = cfg["EPS"]

    NH = D // HD              # heads == e-chunks per q (and per k)
    DC = D // P               # contraction chunks
    NQ = NQTOK // QT          # q tiles
    S2 = S // 2               # kv half (x residency granularity)
    KC2 = S2 // P             # kv chunks per half
    VET = max(1, D // 512)    # v e-tiles of width 512
    VEW = min(512, D)         # v e-tile width
    VH = VEW // HD            # heads per v e-tile
    MAXM = max(len(m) for m in masked)
    QST_PER_TILE = QT // PT
    mdt = F32 if cfg.get("MASK_F32", True) else mybir.dt.bfloat16

    nc = bacc.Bacc("TRN2", target_bir_lowering=False, debug=False)

    # ---- I/O ----
    xTq = nc.dram_tensor("xTq", [D, NQTOK], F32, kind="ExternalInput").ap()
    xT = nc.dram_tensor("xT", [D, S], F32, kind="ExternalInput").ap()
    wqkT = nc.dram_tensor("wqkT", [2 * NH, P, DC, P], F32,
                          kind="ExternalInput").ap()
    wvT = nc.dram_tensor("wvT", [D, D], F32, kind="ExternalInput").ap()
    woT = nc.dram_tensor("woT", [D, D], F32, kind="ExternalInput").ap()
    cosq_i = nc.dram_tensor("cosq", [HD, NQTOK], F32, kind="ExternalInput").ap()
    sinq_i = nc.dram_tensor("sinqn", [HD, NQTOK], F32, kind="ExternalInput").ap()
    cosk_i = nc.dram_tensor("cosk", [HD, S], F32, kind="ExternalInput").ap()
    sink_i = nc.dram_tensor("sinkn", [HD, S], F32, kind="ExternalInput").ap()
    rotm_i = nc.dram_tensor("rotm", [P, P], F32, kind="ExternalInput").ap()
    onesc_i = nc.dram_tensor("onesc", [P, 1], F32, kind="ExternalInput").ap()
    onesr_i = nc.dram_tensor("onesr", [1, P], F32, kind="ExternalInput").ap()
    gq_i = nc.dram_tensor("gq", [P, NH], F32, kind="ExternalInput").ap()
    bq_i = nc.dram_tensor("bq", [P, NH], F32, kind="ExternalInput").ap()
    gk_i = nc.dram_tensor("gk", [P, NH], F32, kind="ExternalInput").ap()
    bk_i = nc.dram_tensor("bk", [P, NH], F32, kind="ExternalInput").ap()
    masks_i = nc.dram_tensor("masks", [NQ, P, MAXM, QT], mdt, kind="ExternalInput").ap()
    out_t = nc.dram_tensor("out", [D, NQTOK], F32, kind="ExternalOutput").ap()

    with tile.TileContext(nc) as tc:
        import contextlib

        ctx = contextlib.ExitStack()
        with ctx:
            sb = ctx.enter_context(tc.tile_pool(name="sb", bufs=1))
            psum = ctx.enter_context(tc.tile_pool(name="ps", bufs=1, space="PSUM"))
            dram = ctx.enter_context(tc.tile_pool(name="dram", bufs=1, space="DRAM"))

            # ---- DRAM scratch ----
            qts = dram.tile([P, NH, NQTOK], F32, tag="qts", name="qts")
            kts = dram.tile([P, NH, S], F32, tag="kts", name="kts")
            vs = dram.tile([NH, S, HD], F32, tag="vs", name="vs")
            ots = dram.tile([P, NH, NQTOK], F32, tag="ots", name="ots")

            # ---- constants / small inputs ----
            ones_col = sb.tile([P, 1], F32, tag="ones_col", name="ones_col")
            nc.sync.dma_start(_r(ones_col), _r(onesc_i))
            ones_row = sb.tile([1, P], F32, tag="ones_row", name="ones_row")
            nc.sync.dma_start(_r(ones_row), _r(onesr_i))
            eps1 = sb.tile([1, 1], F32, tag="eps1", name="eps1")
            nc.vector.memset(eps1, EPS)
            zero1 = sb.tile([1, 1], F32, tag="zero1", name="zero1")
            nc.vector.memset(zero1, 0.0)
            nege = sb.tile([P, 1], F32, tag="nege", name="nege")
            nc.vector.memset(nege, -EXP_BIAS)
            rotm = sb.tile([P, P], F32, tag="rotm", name="rotm")
            nc.sync.dma_start(_r(rotm), _r(rotm_i))
            gq = sb.tile([P, NH], F32, tag="gq", name="gq")
            nc.sync.dma_start(gq, gq_i)
            bq = sb.tile([P, NH], F32, tag="bq", name="bq")
            nc.sync.dma_start(bq, bq_i)
            gk = sb.tile([P, NH], F32, tag="gk", name="gk")
            nc.sync.dma_start(gk, gk_i)
            bk = sb.tile([P, NH], F32, tag="bk", name="bk")
            nc.sync.dma_start(bk, bk_i)

            def proj_ln_rope(x_sb, n_tok, st_global_off, wcol_off, n_st,
                             cos_sb, sin_sb, g_sb, b_sb, dst, tok0):
                """Project x_sb -> feature-partition [e, s] tiles, LN, rope,
                write to dst[:, :, tok0 + st*PT ...].

                x_sb: [P, DC, n_tok] sbuf; st covers n_st tiles of PT inside.
                wcol_off: column offset into wqkT (0 for q, D for k).
                cos_sb/sin_sb indexed at st_global_off + local offsets.
                """
                assert n_st % 2 == 0
                for grp in range(n_st // 2):
                    sts = [grp * 2, grp * 2 + 1]
                    gsl = ds(st_global_off + grp * 2 * PT, 2 * PT)
                    cos_t = sb.tile([HD, 2 * PT], F32, tag="cos", bufs=2,
                                    name="cos_t")
                    nc.sync.dma_start(cos_t, cos_sb[:, gsl])
                    sin_t = sb.tile([HD, 2 * PT], F32, tag="sin", bufs=2,
                                    name="sin_t")
                    nc.sync.dma_start(sin_t, sin_sb[:, gsl])
                    holds = {}
                    pstats = {}
                    for st in sts:
                        holds[st] = sb.tile([P, NH, PT], F32, tag="hold",
                                            bufs=3, name="hold")
                        pstats[st] = psum.tile([1, PT], F32, tag="stat",
                                               bufs=4, name="ps_stat")
                    for ec in range(NH):
                        w = sb.tile([P, DC, P], F32, tag="w", bufs=3, name="w")
                        nc.sync.dma_start(
                            _r(w), _r(wqkT[wcol_off // P + ec])
                        )
                        pss = {st: psum.tile([P, PT], F32, tag="mm", bufs=4,
                                             name="ps")
                               for st in sts}
                        for d in range(DC):
                            for st in sts:
                                nc.tensor.matmul(
                                    pss[st],
                                    lhsT=_r(w[:, d]),
                                    rhs=_r(x_sb[:, d, ds(st * PT, PT)]),
                                    start=(d == 0),
                                    stop=(d == DC - 1),
                                )
                        for st in sts:
                            nc.vector.tensor_copy(_r(holds[st][:, ec]), pss[st])
                            sq = sb.tile([P, PT], F32, tag="sq", bufs=2,
                                         name="sq")
                            nc.scalar.square(_r(sq), pss[st])
                            nc.tensor.matmul(
                                pstats[st],
                                lhsT=_r(ones_col),
                                rhs=_r(sq),
                                start=(ec == 0),
                                stop=(ec == NH - 1),
                            )
                    for st in sts:
                        hold = holds[st]
                        csl = ds((st % 2) * PT, PT)
                        # rsig = exp(-0.5 * ln(sumsq/D + eps))
                        lnv = sb.tile([1, PT], F32, tag="stats_sb", bufs=4,
                                      name="lnv")
                        nc.scalar.activation(lnv, pstats[st], AF.Ln,
                                             scale=1.0 / D, bias=eps1)
                        rsig = sb.tile([1, PT], F32, tag="stats_sb", bufs=4,
                                       name="rsig")
                        nc.scalar.activation(_r(rsig), lnv, AF.Exp, bias=zero1,
                                             scale=-0.5)
                        ps_rep = psum.tile([P, PT], F32, tag="stat", bufs=4,
                                           name="ps_rep")
                        nc.tensor.matmul(ps_rep, lhsT=_r(ones_row),
                                         rhs=_r(rsig))
                        # pass 1: DVE LN apply for all chunks first, so the
                        # rotation matmuls never head-of-line block the
                        # in-order PE stream on a DVE dependency.
                        for ec in range(NH):
                            ch = hold[:, ec]
                            nc.vector.tensor_tensor(_r(ch), ch, ps_rep,
                                                    op=OP.mult)
                            nc.vector.tensor_scalar(
                                _r(ch), ch,
                                scalar1=g_sb[:, ds(ec, 1)],
                                scalar2=b_sb[:, ds(ec, 1)],
                                op0=OP.mult, op1=OP.add,
                            )
                        # pass 2: rotation matmuls stream back-to-back
                        for ec in range(NH):
                            ch = hold[:, ec]
                            ps_rot = psum.tile([P, PT], F32, tag="mm", bufs=4,
                                               name="ps_rot")
                            nc.tensor.matmul(ps_rot, lhsT=_r(rotm), rhs=_r(ch))
                            tmp = sb.tile([P, PT], F32, tag="tmp", bufs=3,
                                          name="rtmp")
                            nc.vector.tensor_tensor(
                                tmp, ps_rot, sin_t[:, csl], op=OP.mult
                            )
                            nc.vector.tensor_tensor(_r(ch), ch, cos_t[:, csl],
                                                    op=OP.mult)
                            nc.vector.tensor_tensor(_r(ch), ch, tmp, op=OP.add)
                        nc.sync.dma_start(dst[:, :, ds(tok0 + st * PT, PT)],
                                          hold)

            # ---- Phase A: q projection ----
            xq = sb.tile([P, DC, max(NQTOK, S2)], F32, tag="bigx", bufs=1,
                         name="xq")
            xq = xq[:, :, :NQTOK]
            for d in range(DC):
                nc.sync.dma_start(_r(xq[:, d]), _r(xTq[ds(d * P, P), :]))
            proj_ln_rope(xq, NQTOK, 0, 0, NQTOK // PT, cosq_i, sinq_i,
                         gq, bq, qts, 0)

            # ---- Phase B+C: k and v projections, per x-half ----
            for half in range(2):
                xk = sb.tile([P, DC, max(NQTOK, S2)], F32, tag="bigx", bufs=1,
                             name="xk")
                xk = xk[:, :, :S2]
                for d in range(DC):
                    nc.sync.dma_start(
                        _r(xk[:, d]), _r(xT[ds(d * P, P), ds(half * S2, S2)])
                    )
                proj_ln_rope(xk, S2, half * S2, D, S2 // PT, cosk_i, sink_i,
                             gk, bk, kts, half * S2)
                # v: natural orientation, x as stationary
                n_grp = (KC2 + 3) // 4
                for grp in range(n_grp):
                    scs = [sc for sc in range(grp * 4, min((grp + 1) * 4, KC2))]
                    for et in range(VET):
                        psv = {}
                        for sc in scs:
                            psv[sc] = psum.tile([P, VEW], F32, tag="mm",
                                                bufs=4, name="psv")
                        for d in range(DC):
                            wv = sb.tile([P, VEW], F32, tag="w", bufs=3,
                                         name="wv")
                            nc.sync.dma_start(
                                _r(wv), _r(wvT[ds(d * P, P), ds(et * VEW, VEW)])
                            )
                            for sc in scs:
                                nc.tensor.matmul(
                                    psv[sc],
                                    lhsT=_r(xk[:, d, ds(sc * P, P)]),
                                    rhs=_r(wv),
                                    start=(d == 0),
                                    stop=(d == DC - 1),
                                )
                        for sc in scs:
                            vsb = sb.tile([P, VEW], F32, tag="vsb", bufs=2,
                                          name="vsb")
                            nc.vector.tensor_copy(vsb, psv[sc])
                            gsc = half * KC2 + sc
                            for hh in range(VH):
                                nc.sync.dma_start(
                                    vs[et * VH + hh, ds(gsc * P, P), :],
                                    vsb[:, ds(hh * HD, HD)],
                                )

            # ---- Phase D: attention + normalization ----
            for t in range(NQ):
                qsl_off = t * QT
                mt = sb.tile([P, MAXM, QT], mdt, tag="masks", bufs=1,
                             name="mt")
                nc.sync.dma_start(mt, masks_i[t])
                mpos = {kc: i for i, kc in enumerate(masked[t])}
                n_slots = slots[t]
                n_half = (n_slots + KC2 - 1) // KC2  # halves needed
                for h in range(NH):
                    qsl = sb.tile([P, QT], F32, tag="qslab", bufs=2,
                                  name="qsl")
                    nc.sync.dma_start(_r(qsl), _r(qts[:, h, ds(qsl_off, QT)]))
                    ksl = {}
                    vsl = {}
                    for hf in range(n_half):
                        ksl[hf] = sb.tile([P, S2], F32, tag="kslab", bufs=2,
                                          name="ksl")
                        nc.sync.dma_start(_r(ksl[hf]), _r(kts[:, h, ds(hf * S2, S2)]))
                        vsl[hf] = sb.tile([P, KC2, HD], F32, tag="vslab",
                                          bufs=2, name="vsl")
                        nc.sync.dma_start(
                            _r(vsl[hf]),
                            _r(vs[h, ds(hf * S2, S2), :].rearrange(
                                "(kc p) hd -> p kc hd", p=P
                            )),
                        )
                    psout = psum.tile([P, QT], F32, tag="mm", bufs=4,
                                      name="psout")
                    psden = psum.tile([1, QT], F32, tag="stat", bufs=4,
                                      name="psden")
                    for slot in range(n_slots):
                        hf = slot // KC2
                        kc = slot % KC2
                        pss = psum.tile([P, QT], F32, tag="mm", bufs=4,
                                        name="pss")
                        nc.tensor.matmul(
                            pss,
                            lhsT=_r(ksl[hf][:, ds(kc * P, P)]),
                            rhs=_r(qsl),
                        )
                        et = sb.tile([P, QT], F32, tag="exp", bufs=3,
                                     name="et")
                        nc.scalar.activation(_r(et), pss, AF.Exp, bias=nege)
                        if slot in mpos:
                            nc.vector.tensor_tensor(
                                _r(et), et, mt[:, mpos[slot]], op=OP.mult
                            )
                        nc.tensor.matmul(
                            psout,
                            lhsT=_r(vsl[hf][:, kc]),
                            rhs=_r(et),
                            start=(slot == 0),
                            stop=(slot == n_slots - 1),
                        )
                        nc.tensor.matmul(
                            psden,
                            lhsT=_r(ones_col),
                            rhs=_r(et),
                            start=(slot == 0),
                            stop=(slot == n_slots - 1),
                        )
                    rec0 = sb.tile([1, QT], F32, tag="stats_sb", bufs=4,
                                   name="rec0")
                    with nc.allow_low_precision(
                        reason="denominator reciprocal, 18 bits is plenty"
                    ):
                        nc.vector.reciprocal_approx_fast(rec0, psden)
                    rec = sb.tile([1, QT], F32, tag="stats_sb", bufs=4,
                                  name="rec")
                    nc.scalar.activation(_r(rec), rec0, AF.Copy)
                    psr = psum.tile([P, QT], F32, tag="stat", bufs=4,
                                    name="psr")
                    nc.tensor.matmul(psr, lhsT=_r(ones_row), rhs=_r(rec))
                    rsb = sb.tile([P, QT], F32, tag="tmp", bufs=3, name="rsb")
                    nc.scalar.activation(rsb, psr, AF.Copy)
                    ot = sb.tile([P, QT], F32, tag="outT", bufs=2, name="ot")
                    nc.vector.tensor_tensor(ot, psout, rsb, op=OP.mult)
                    nc.sync.dma_start(ots[:, h, ds(qsl_off, QT)], ot)

                # ---- Phase E: out-projection for this q tile ----
                EG = 4
                for eg in range(NH // EG):
                    psf = [
                        psum.tile([P, QT], F32, tag="mm", bufs=4, name="psf")
                        for _ in range(EG)
                    ]
                    for h in range(NH):
                        orh = sb.tile([P, QT], F32, tag="orhs", bufs=2,
                                      name="orh")
                        nc.sync.dma_start(_r(orh), _r(ots[:, h, ds(qsl_off, QT)]))
                        wo = sb.tile([P, EG * P], F32, tag="w", bufs=3,
                                     name="wo")
                        nc.sync.dma_start(
                            _r(wo), _r(woT[ds(h * P, P), ds(eg * EG * P, EG * P)])
                        )
                        for x in range(EG):
                            nc.tensor.matmul(
                                psf[x],
                                lhsT=_r(wo[:, ds(x * P, P)]),
                                rhs=_r(orh),
                                start=(h == 0),
                                stop=(h == NH - 1),
                            )
                    for x in range(EG):
                        fsb = sb.tile([P, QT], F32, tag="tmp", bufs=3,
                                      name="fsb")
                        nc.vector.tensor_copy(fsb, psf[x])
                        nc.sync.dma_start(
                            out_t[ds((eg * EG + x) * P, P), ds(qsl_off, QT)],
                            fsb,
                        )

    nc.compile()
    return nc


# --------------------------------------------------------------------------
# Host-side prep and driver
# --------------------------------------------------------------------------

def _q_blocks(role, n_blocks):
    """q-block indices (each 512 tokens) for a core role."""
    if n_blocks == 4:
        return [0, 3] if role == 0 else [1, 2]
    # degenerate small configs: one core covers all blocks
    return list(range(n_blocks))


def make_host_data(x, w_in, w_out, q_gamma, q_beta, k_gamma, k_beta, cfg,
                   n_cores=None):
    """Build per-core in_maps (list of dicts) + assembly metadata."""
    D = cfg["D"]
    S = cfg["S"]
    NQTOK = cfg["NQTOK"]
    QT = cfg["QT"]
    slots = cfg["slots"]
    masked = cfg["masked"]
    NH = D // HD
    NQ = NQTOK // QT
    MAXM = max(len(m) for m in masked)
    if cfg.get("MASK_F32", True):
        mdt = np.float32
    else:
        import ml_dtypes
        mdt = ml_dtypes.bfloat16
    B = x.shape[0]
    n_blocks = S // 512
    if n_cores is None:
        n_cores = B * (2048 // NQTOK) if S == 2048 else B

    w64 = np.asarray(w_in, np.float64)
    wq = w64[0:D]
    wk = w64[D:2 * D]
    wv = w64[2 * D:3 * D]
    wq_c = wq - wq.mean(axis=0, keepdims=True)
    wk_c = wk - wk.mean(axis=0, keepdims=True)
    wqkT2 = np.concatenate([wq_c.T, wk_c.T], axis=1).astype(np.float32)
    # pre-tile to [2*NH, P, DC, P]: tile ec -> [p, dc, e] with contiguous rows
    NHl = D // P
    DCl = D // P
    wqkT = np.ascontiguousarray(
        wqkT2.reshape(DCl, P, 2 * NHl, P).transpose(2, 1, 0, 3)
    )
    wvT = np.ascontiguousarray(wv.T.astype(np.float32))
    woT = np.ascontiguousarray(np.asarray(w_out, np.float64).T.astype(np.float32))

    inv = 1.0 / (10000.0 ** (np.arange(0, HD, 2, dtype=np.float64) / HD))
    tpos = np.arange(S, dtype=np.float64)
    fr = np.outer(tpos, inv)
    emb = np.concatenate([fr, fr], axis=-1)  # [S, HD]
    cosT = np.cos(emb).T  # [HD, S]
    sinTn = np.sin(emb).T

    # signed rotate-half permutation, as matmul lhsT:
    # out[p] = sum_{p'} rotmT[p', p] * in[p'] = rot_half(in)[p]
    h2 = HD // 2
    rotmT = np.zeros((P, P), np.float32)
    for p in range(h2):
        rotmT[p + h2, p] = -1.0
    for p in range(h2, HD):
        rotmT[p - h2, p] = 1.0

    scale = 1.0 / math.sqrt(HD)
    gq_a = np.ascontiguousarray(
        (np.asarray(q_gamma, np.float64) * scale).reshape(NH, P).T
    ).astype(np.float32)
    bq_a = np.ascontiguousarray(
        (np.asarray(q_beta, np.float64) * scale).reshape(NH, P).T
    ).astype(np.float32)
    gk_a = np.ascontiguousarray(
        np.asarray(k_gamma, np.float32).reshape(NH, P).T
    )
    bk_a = np.ascontiguousarray(
        np.asarray(k_beta, np.float32).reshape(NH, P).T
    )

    in_maps = []
    meta = []
    cores_per_batch = max(1, n_cores // B)
    for c in range(n_cores):
        b = c // cores_per_batch
        r = c % cores_per_batch
        blocks = _q_blocks(r if cores_per_batch > 1 else 0, n_blocks)
        blocks = blocks[: NQTOK // 512]
        qtok = np.concatenate(
            [np.arange(bk * 512, (bk + 1) * 512) for bk in blocks]
        )
        xb = np.asarray(x[b], np.float32)  # [S, D]
        xT = np.ascontiguousarray(xb.T)    # [D, S]
        xTq = np.ascontiguousarray(xT[:, qtok])
        cosq = np.ascontiguousarray(cosT[:, qtok].astype(np.float32))
        sinq = np.ascontiguousarray(sinTn[:, qtok].astype(np.float32))
        cosk = np.ascontiguousarray(cosT[:, :S].astype(np.float32))
        sink = np.ascontiguousarray(sinTn[:, :S].astype(np.float32))

        masks = np.zeros([NQ, P, MAXM, QT], np.float32)
        for t in range(NQ):
            q_start = blocks[t * (QT // 512)] * 512 if QT == 512 else None
            assert QT == 512
            q_start = blocks[t] * 512
            qq = np.arange(QT)
            kk = np.arange(P)
            for mi, kc in enumerate(masked[t]):
                masks[t, :, mi, :] = (
                    (kc * P + kk[:, None]) <= (q_start + qq[None, :])
                ).astype(np.float32)
        masks = masks.astype(mdt)

        in_maps.append(dict(
            xTq=xTq, xT=xT, wqkT=wqkT, wvT=wvT, woT=woT,
            cosq=cosq, sinqn=sinq, cosk=cosk, sinkn=sink,
            gq=gq_a, bq=bq_a, gk=gk_a, bk=bk_a, masks=masks,
            rotm=rotmT,
            onesc=np.ones((P, 1), np.float32),
            onesr=np.ones((1, P), np.float32),
        ))
        meta.append(dict(b=b, qtok=qtok))
    return in_maps, meta


_PROGRAM_CACHE = {}


def _get_program(cfg_key, cfg):
    if cfg_key not in _PROGRAM_CACHE:
        _PROGRAM_CACHE[cfg_key] = build_program(cfg)
    return _PROGRAM_CACHE[cfg_key]


def run_full(x, w_in, w_out, q_gamma, q_beta, k_gamma, k_beta,
             trace=False):
    from concourse.bass_utils import run_bass_kernel_spmd

    cfg = FULL_CFG
    B = x.shape[0]
    n_cores = 2 * B
    in_maps, meta = make_host_data(
        x, w_in, w_out, q_gamma, q_beta, k_gamma, k_beta, cfg,
        n_cores=n_cores,
    )
    nc = _get_program("full", cfg)
    res = run_bass_kernel_spmd(
        nc, in_maps, core_ids=list(range(n_cores)), trace=trace,
    )
    S, D = cfg["S"], cfg["D"]
    out = np.empty((B, S, D), np.float32)
    for c in range(n_cores):
        o = res.results[c]["out"]  # [D, NQTOK]
        out[meta[c]["b"], meta[c]["qtok"], :] = o.T
    return out, res


def kernel(x, w_in, w_out, q_gamma, q_beta, k_gamma, k_beta, n_heads=16,
           **_ignored):
    x = np.asarray(x, np.float32)
    assert int(np.asarray(n_heads)) * HD == x.shape[-1]
    out, _ = run_full(
        np.asarray(x, np.float32),
        np.asarray(w_in, np.float32),
        np.asarray(w_out, np.float32),
        np.asarray(q_gamma, np.float32),
        np.asarray(q_beta, np.float32),
        np.asarray(k_gamma, np.float32),
        np.asarray(k_beta, np.float32),
    )
    return out



# revision 10
# speedup vs baseline: 1.0004x; 1.0004x over previous
"""Trainium2 Bass kernel for a custom attention block (qkv-proj + LN(q,k) +
RoPE + causal attention + out-proj), distributed over 8 NeuronCores.

Sharding: 2 cores per batch (B=4). Core role r=c%2 takes q-token blocks
{0,3} (r=0) or {1,2} (r=1) of 512 tokens; every core computes K/V for the
full 2048-token sequence of its batch (no collectives). The compiled
program is identical on all cores; all per-core differences are input
data (sliced x^T, cos/sin tables, causal masks).

v2 (this file): all matmul operands are bf16 (same PE rate as fp32r at
moving-dim >= 256, half the DMA/SBUF), q/k/v round-trip DRAM in bf16 and
are streamed per-head with double buffering, projection loops are
ec-outer so each weight tile loads once per token-half, the attention
slot loop is software-pipelined (QK of slot s+2 issues before PV of slot
s so the in-order PE queue never waits on the Scalar-engine exp), the
softmax denominator accumulates on the otherwise-idle GpSimd engine
instead of a per-slot PE matmul, and the attention output stays in SBUF
for the out-projection.

LN: mean subtraction is folded into host-pre-centered w_in rows; variance
comes from Square + ones-matmul partition reduction; rsqrt(var+eps) is
computed as Exp(-0.5*Ln(var+eps)) so all ACT functions live in one table
set (natural_log_exp_and_others).
"""

import math

import numpy as np

import concourse.bass as bass
import concourse.mybir as mybir
import concourse.tile as tile
from concourse import bacc
from concourse.bass import ds, ts

F32 = mybir.dt.float32
F32R = mybir.dt.float32r
BF = mybir.dt.bfloat16
AF = mybir.ActivationFunctionType
OP = mybir.AluOpType

P = 128
HD = 128

FULL_CFG = dict(
    D=2048,           # model dim (contraction dim for projections)
    S=2048,           # kv tokens per core (full sequence of its batch)
    NQTOK=1024,       # q tokens per core
    PT=256,           # projection s-tile width (moving dim)
    QT=512,           # attention q-tile width (moving dim)
    slots=(8, 16),    # kv 128-chunks visited per q-tile
    masked=(tuple(range(0, 8)), tuple(range(8, 16))),  # slots that get a mask
    EXP_BIAS=8.0,
    EPS=1e-5,
)


def _r(ap):
    """fp32 -> fp32r view for matmul operands."""
    return ap.bitcast(F32R)


def build_program(cfg):
    D = cfg["D"]
    S = cfg["S"]
    NQTOK = cfg["NQTOK"]
    PT = cfg["PT"]
    QT = cfg["QT"]
    slots = cfg["slots"]
    masked = cfg["masked"]
    EXP_BIAS = cfg["EXP_BIAS"]
    EPS = cfg["EPS"]

    NH = D // HD              # heads == e-chunks per q (and per k)
    DC = D // P               # contraction chunks
    NQ = NQTOK // QT          # q tiles
    S2 = S // 2               # kv half (x residency granularity)
    KC2 = S2 // P             # kv chunks per half
    MAXM = max(len(m) for m in masked)
    VEW = 512                 # v-proj e-tile width
    VET = D // VEW

    nc = bacc.Bacc("TRN2", target_bir_lowering=False, debug=False)

    # ---- I/O ----
    xTq = nc.dram_tensor("xTq", [D, NQTOK], BF, kind="ExternalInput").ap()
    xT = nc.dram_tensor("xT", [D, S], BF, kind="ExternalInput").ap()
    wqkT = nc.dram_tensor("wqkT", [2 * NH, P, DC, P], BF,
                          kind="ExternalInput").ap()
    wvT = nc.dram_tensor("wvT", [D, D], BF, kind="ExternalInput").ap()
    woT = nc.dram_tensor("woT", [D, D], BF, kind="ExternalInput").ap()
    cosq_i = nc.dram_tensor("cosq", [HD, NQTOK], BF, kind="ExternalInput").ap()
    sinq_i = nc.dram_tensor("sinqn", [HD, NQTOK], BF, kind="ExternalInput").ap()
    cosk_i = nc.dram_tensor("cosk", [HD, S], BF, kind="ExternalInput").ap()
    sink_i = nc.dram_tensor("sinkn", [HD, S], BF, kind="ExternalInput").ap()
    rotm_i = nc.dram_tensor("rotm", [P, P], BF, kind="ExternalInput").ap()
    onesc_i = nc.dram_tensor("onesc", [P, 1], F32, kind="ExternalInput").ap()
    onesr_i = nc.dram_tensor("onesr", [1, P], F32, kind="ExternalInput").ap()
    gq_i = nc.dram_tensor("gq", [P, NH], F32, kind="ExternalInput").ap()
    bq_i = nc.dram_tensor("bq", [P, NH], F32, kind="ExternalInput").ap()
    gk_i = nc.dram_tensor("gk", [P, NH], F32, kind="ExternalInput").ap()
    bk_i = nc.dram_tensor("bk", [P, NH], F32, kind="ExternalInput").ap()
    masks_i = nc.dram_tensor("masks", [NQ, P, MAXM, QT], BF,
                             kind="ExternalInput").ap()
    out_t = nc.dram_tensor("out", [D, NQTOK], F32, kind="ExternalOutput").ap()

    with tile.TileContext(nc) as tc:
        import contextlib

        ctx = contextlib.ExitStack()
        with ctx:
            sb = ctx.enter_context(tc.tile_pool(name="sb", bufs=1))
            psum = ctx.enter_context(tc.tile_pool(name="ps", bufs=1, space="PSUM"))
            dram = ctx.enter_context(tc.tile_pool(name="dram", bufs=1, space="DRAM"))

            # ---- DRAM scratch (bf16) ----
            qts = dram.tile([P, NH, NQTOK], BF, tag="qts", name="qts")
            kts = dram.tile([P, NH, S], BF, tag="kts", name="kts")
            vs = dram.tile([NH, S, HD], BF, tag="vs", name="vs")

            # ---- constants / small inputs ----
            ones_col = sb.tile([P, 1], F32, tag="ones_col", name="ones_col")
            nc.sync.dma_start(_r(ones_col), _r(onesc_i))
            ones_row = sb.tile([1, P], F32, tag="ones_row", name="ones_row")
            nc.sync.dma_start(_r(ones_row), _r(onesr_i))
            eps1 = sb.tile([1, 1], F32, tag="eps1", name="eps1")
            nc.vector.memset(eps1, EPS)
            zero1 = sb.tile([1, 1], F32, tag="zero1", name="zero1")
            nc.vector.memset(zero1, 0.0)
            nege = sb.tile([P, 1], F32, tag="nege", name="nege")
            nc.vector.memset(nege, -EXP_BIAS)
            rotm = sb.tile([P, P], BF, tag="rotm", name="rotm")
            nc.sync.dma_start(rotm, rotm_i)
            gq = sb.tile([P, NH], F32, tag="gq", name="gq")
            nc.sync.dma_start(gq, gq_i)
            bq = sb.tile([P, NH], F32, tag="bq", name="bq")
            nc.sync.dma_start(bq, bq_i)
            gk = sb.tile([P, NH], F32, tag="gk", name="gk")
            nc.sync.dma_start(gk, gk_i)
            bk = sb.tile([P, NH], F32, tag="bk", name="bk")
            nc.sync.dma_start(bk, bk_i)
            # rope tables resident in SBUF (bf16)
            cosk = sb.tile([HD, S], BF, tag="cosk", name="cosk")
            nc.sync.dma_start(cosk, cosk_i)
            sink = sb.tile([HD, S], BF, tag="sink", name="sink")
            nc.sync.dma_start(sink, sink_i)
            cosq = sb.tile([HD, NQTOK], BF, tag="cosq", name="cosq")
            nc.sync.dma_start(cosq, cosq_i)
            sinq = sb.tile([HD, NQTOK], BF, tag="sinq", name="sinq")
            nc.sync.dma_start(sinq, sinq_i)

            def proj_ln_rope(x_src, tok0_src, n_st, wcol_off, cos_sb, cos_off,
                             g_sb, b_sb, dst, tok0_dst):
                """Project n_st*PT tokens of x into feature-partition [e, s]
                tiles, LN (host-centered w, device rsig), rope, write bf16 to
                dst[:, :, tok0_dst + ...].

                x_src: DRAM [D, *] bf16; tokens tok0_src..+n_st*PT.
                wcol_off: head-row offset into wqkT (0 for q, NH for k).
                cos_sb/sin pair indexed at cos_off + local token offsets.
                ec-outer: each weight tile is DMA'd once per call.
                """
                cos_t, sin_t = cos_sb
                # x tiles for this call: [P, DC, PT] per st, all resident
                xts = []
                for st in range(n_st):
                    xt = sb.tile([P, DC, PT], BF, tag="xt", bufs=n_st,
                                 name="xt")
                    for d in range(DC):
                        nc.sync.dma_start(
                            xt[:, d],
                            x_src[ds(d * P, P), ds(tok0_src + st * PT, PT)],
                        )
                    xts.append(xt)
                holds = [
                    sb.tile([P, NH, PT], BF, tag="hold", bufs=n_st,
                            name="hold")
                    for _ in range(n_st)
                ]
                # two psum stat tiles (full zero-region each), 2 st slices per
                # tile; accumulation groups stay 1-per-zero-region
                assert n_st % 2 == 0
                pstats_t = [
                    psum.tile([1, 2 * PT], F32, tag="stat", bufs=2,
                              name="pstats")
                    for _ in range(n_st // 2)
                ]

                def pstat_sl(st):
                    return pstats_t[st // 2][:, ds((st % 2) * PT, PT)]
                for ec in range(NH):
                    w = sb.tile([P, DC, P], BF, tag="w", bufs=3, name="w")
                    nc.sync.dma_start(w, wqkT[wcol_off + ec])
                    pss = {st: psum.tile([P, PT], F32, tag="mm", bufs=4,
                                         name="psp")
                           for st in range(n_st)}
                    for d in range(DC):
                        for st in range(n_st):
                            nc.tensor.matmul(
                                pss[st],
                                lhsT=w[:, d],
                                rhs=xts[st][:, d],
                                start=(d == 0),
                                stop=(d == DC - 1),
                            )
                    sq_all = sb.tile([P, n_st * PT], F32, tag="sq", bufs=2,
                                     name="sq_all")
                    for st in range(n_st):
                        nc.scalar.copy(holds[st][:, ec], pss[st])
                        nc.scalar.square(_r(sq_all[:, ds(st * PT, PT)]),
                                         pss[st])
                    for half_st in range(n_st // 2):
                        nc.tensor.matmul(
                            pstats_t[half_st],
                            lhsT=_r(ones_col),
                            rhs=_r(sq_all[:, ds(half_st * 2 * PT, 2 * PT)]),
                            start=(ec == 0),
                            stop=(ec == NH - 1),
                        )
                for st in range(n_st):
                    hold = holds[st]
                    csl = ds(cos_off + st * PT, PT)
                    # rsig = exp(-0.5 * ln(sumsq/D + eps))
                    lnv = sb.tile([1, PT], F32, tag="stats_sb", bufs=4,
                                  name="lnv")
                    nc.scalar.activation(lnv, pstat_sl(st),
                                         AF.Ln, scale=1.0 / D, bias=eps1)
                    rsig = sb.tile([1, PT], F32, tag="stats_sb", bufs=4,
                                   name="rsig")
                    nc.scalar.activation(_r(rsig), lnv, AF.Exp, bias=zero1,
                                         scale=-0.5)
                    ps_rep = psum.tile([P, PT], F32, tag="mm", bufs=4,
                                       name="ps_rep")
                    nc.tensor.matmul(ps_rep, lhsT=_r(ones_row), rhs=_r(rsig))
                    rep = sb.tile([P, PT], BF, tag="rep", bufs=4,
                                  name="rep")
                    nc.scalar.copy(rep, ps_rep)
                    # pass 1: LN apply (DVE) for all chunks first, so the
                    # rotation matmuls never head-of-line block the in-order
                    # PE stream on a DVE dependency.
                    for ec in range(NH):
                        ch = hold[:, ec]
                        nc.vector.tensor_tensor(ch, ch, rep, op=OP.mult)
                        nc.vector.tensor_scalar(
                            ch, ch,
                            scalar1=g_sb[:, ds(ec, 1)],
                            scalar2=b_sb[:, ds(ec, 1)],
                            op0=OP.mult, op1=OP.add,
                        )
                    # pass 2: rotation matmuls stream back-to-back; the
                    # sin-multiply runs on GpSimd (otherwise idle here)
                    for ec in range(NH):
                        ch = hold[:, ec]
                        ps_rot = psum.tile([P, PT], F32, tag="mm", bufs=4,
                                           name="ps_rot")
                        nc.tensor.matmul(ps_rot, lhsT=rotm, rhs=ch)
                        tmp = sb.tile([P, PT], BF, tag="tmp", bufs=3,
                                      name="rtmp")
                        nc.vector.tensor_tensor(tmp, ps_rot, sin_t[:, csl],
                                                op=OP.mult)
                        nc.vector.tensor_tensor(ch, ch, cos_t[:, csl],
                                                op=OP.mult)
                        nc.vector.tensor_tensor(ch, ch, tmp, op=OP.add)
                    nc.sync.dma_start(
                        dst[:, :, ds(tok0_dst + st * PT, PT)], hold
                    )

            # ---- Phase A: q projection (1024 tokens, 4 st) ----
            proj_ln_rope(xTq, 0, NQTOK // PT, 0, (cosq, sinq), 0,
                         gq, bq, qts, 0)

            # ---- Phase B+C: k and v projections, per x-half ----
            for half in range(2):
                proj_ln_rope(xT, half * S2, S2 // PT, NH, (cosk, sink),
                             half * S2, gk, bk, kts, half * S2)
                # v: natural orientation, x chunks stationary, wv moving
                for scg in range(KC2 // 4):
                    scs = [scg * 4 + i for i in range(4)]
                    for et in range(VET):
                        psv = {sc: psum.tile([P, VEW], F32, tag="mm", bufs=4,
                                             name="psv")
                               for sc in scs}
                        for d in range(DC):
                            wv = sb.tile([P, VEW], BF, tag="wv", bufs=3,
                                         name="wv")
                            nc.sync.dma_start(
                                wv, wvT[ds(d * P, P), ds(et * VEW, VEW)]
                            )
                            xv = sb.tile([P, 4 * P], BF, tag="xv", bufs=3,
                                         name="xv")
                            nc.sync.dma_start(
                                xv,
                                xT[ds(d * P, P),
                                   ds(half * S2 + scg * 4 * P, 4 * P)],
                            )
                            for i, sc in enumerate(scs):
                                nc.tensor.matmul(
                                    psv[sc],
                                    lhsT=xv[:, ds(i * P, P)],
                                    rhs=wv,
                                    start=(d == 0),
                                    stop=(d == DC - 1),
                                )
                        for sc in scs:
                            vsb = sb.tile([P, VEW], BF, tag="vsb", bufs=4,
                                          name="vsb")
                            nc.scalar.copy(vsb, psv[sc])
                            gsc = half * KC2 + sc
                            for hh in range(VEW // HD):
                                nc.sync.dma_start(
                                    vs[et * (VEW // HD) + hh,
                                       ds(gsc * P, P), :],
                                    vsb[:, ds(hh * HD, HD)],
                                )

            # ---- Phase D+E: attention + out-projection per q tile ----
            for t in range(NQ):
                qsl_off = t * QT
                mt = sb.tile([P, MAXM, QT], BF, tag="masks", bufs=1,
                             name="mt")
                nc.sync.dma_start(mt, masks_i[t])
                mpos = {kc: i for i, kc in enumerate(masked[t])}
                n_slots = slots[t]
                n_half = (n_slots + KC2 - 1) // KC2  # kv halves needed
                ots = sb.tile([P, NH, QT], BF, tag="ots", bufs=2, name="ots")
                for h in range(NH):
                    qsl = sb.tile([P, QT], BF, tag="qslab", bufs=3,
                                  name="qsl")
                    nc.sync.dma_start(qsl, qts[:, h, ds(qsl_off, QT)])
                    ksl = sb.tile([P, n_half * S2], BF, tag="kslab", bufs=2,
                                  name="ksl")
                    nc.sync.dma_start(ksl, kts[:, h, ds(0, n_half * S2)])
                    vsl = sb.tile([P, n_half * KC2, HD], BF, tag="vslab",
                                  bufs=2, name="vsl")
                    nc.sync.dma_start(
                        vsl,
                        vs[h, ds(0, n_half * S2), :].rearrange(
                            "(kc p) hd -> p kc hd", p=P
                        ),
                    )
                    psout = psum.tile([P, QT], F32, tag="acc", bufs=2,
                                      name="psout")
                    den = sb.tile([P, QT], F32, tag="den", bufs=2, name="den")
                    # software-pipelined slot loop: QK runs PIPE slots ahead
                    # of exp/PV so the in-order PE stream never waits on the
                    # Scalar engine.
                    PIPE = 3
                    pssq = {}
                    ets = {}

                    def issue_qk(s):
                        pssq[s] = psum.tile([P, QT], F32, tag="mm", bufs=4,
                                            name="pssq")
                        nc.tensor.matmul(
                            pssq[s],
                            lhsT=ksl[:, ds(s * P, P)],
                            rhs=qsl,
                        )

                    def issue_exp(s):
                        et = sb.tile([P, QT], BF, tag="exp", bufs=PIPE + 1,
                                     name="et")
                        nc.scalar.activation(et, pssq[s], AF.Exp, bias=nege)
                        del pssq[s]
                        if s in mpos:
                            nc.vector.tensor_tensor(
                                et, et, mt[:, mpos[s]], op=OP.mult
                            )
                        ets[s] = et

                    for s in range(min(PIPE, n_slots)):
                        issue_qk(s)
                    issue_exp(0)
                    for s in range(n_slots):
                        if s + PIPE < n_slots:
                            issue_qk(s + PIPE)
                        if s + 1 < n_slots:
                            issue_exp(s + 1)
                        et = ets.pop(s)
                        # denominator accumulates on GpSimd (partition-wise);
                        # reduced over partitions once per head below
                        if s == 0:
                            nc.gpsimd.tensor_copy(_r(den), et)
                        else:
                            nc.gpsimd.tensor_tensor(_r(den), den, et,
                                                    op=OP.add)
                        nc.tensor.matmul(
                            psout,
                            lhsT=vsl[:, s],
                            rhs=et,
                            start=(s == 0),
                            stop=(s == n_slots - 1),
                        )
                    psden = psum.tile([1, QT], F32, tag="stat", bufs=2,
                                      name="psden")
                    nc.tensor.matmul(psden, lhsT=_r(ones_col), rhs=_r(den))
                    rec0 = sb.tile([1, QT], F32, tag="stats_sb", bufs=4,
                                   name="rec0")
                    with nc.allow_low_precision(
                        reason="denominator reciprocal, 18 bits is plenty"
                    ):
                        nc.vector.reciprocal_approx_fast(rec0, psden)
                    rec = sb.tile([1, QT], F32, tag="stats_sb", bufs=4,
                                  name="rec")
                    nc.scalar.activation(_r(rec), rec0, AF.Copy)
                    psr = psum.tile([P, QT], F32, tag="mm", bufs=4,
                                    name="psr")
                    nc.tensor.matmul(psr, lhsT=_r(ones_row), rhs=_r(rec))
                    rsb = sb.tile([P, QT], F32, tag="rsb", bufs=2, name="rsb")
                    nc.scalar.copy(rsb, psr)
                    nc.vector.tensor_tensor(ots[:, h], psout, rsb, op=OP.mult)

                # ---- Phase E: out-projection for this q tile (from SBUF) ----
                EG = 2
                for eg in range(NH // EG):
                    psf = [
                        psum.tile([P, QT], F32, tag="mm", bufs=4, name="psf")
                        for _ in range(EG)
                    ]
                    for h in range(NH):
                        wo = sb.tile([P, EG * P], BF, tag="wo", bufs=3,
                                     name="wo")
                        nc.sync.dma_start(
                            wo, woT[ds(h * P, P), ds(eg * EG * P, EG * P)]
                        )
                        for x in range(EG):
                            nc.tensor.matmul(
                                psf[x],
                                lhsT=wo[:, ds(x * P, P)],
                                rhs=ots[:, h],
                                start=(h == 0),
                                stop=(h == NH - 1),
                            )
                    for x in range(EG):
                        fsb = sb.tile([P, QT], F32, tag="fsb", bufs=3,
                                      name="fsb")
                        nc.vector.tensor_copy(fsb, psf[x])
                        nc.sync.dma_start(
                            out_t[ds((eg * EG + x) * P, P), ds(qsl_off, QT)],
                            fsb,
                        )

    nc.compile()
    return nc


# --------------------------------------------------------------------------
# Host-side prep and driver
# --------------------------------------------------------------------------

def _q_blocks(role, n_blocks):
    """q-block indices (each 512 tokens) for a core role."""
    if n_blocks == 4:
        return [0, 3] if role == 0 else [1, 2]
    return list(range(n_blocks))


def make_host_data(x, w_in, w_out, q_gamma, q_beta, k_gamma, k_beta, cfg,
                   n_cores=None):
    """Build per-core in_maps (list of dicts) + assembly metadata."""
    import ml_dtypes

    BF_NP = ml_dtypes.bfloat16

    D = cfg["D"]
    S = cfg["S"]
    NQTOK = cfg["NQTOK"]
    QT = cfg["QT"]
    slots = cfg["slots"]
    masked = cfg["masked"]
    NH = D // HD
    NQ = NQTOK // QT
    MAXM = max(len(m) for m in masked)
    B = x.shape[0]
    n_blocks = S // 512
    if n_cores is None:
        n_cores = B * (2048 // NQTOK) if S == 2048 else B

    w64 = np.asarray(w_in, np.float64)
    wq = w64[0:D]
    wk = w64[D:2 * D]
    wv = w64[2 * D:3 * D]
    wq_c = wq - wq.mean(axis=0, keepdims=True)
    wk_c = wk - wk.mean(axis=0, keepdims=True)
    wqkT2 = np.concatenate([wq_c.T, wk_c.T], axis=1)
    # pre-tile to [2*NH, P, DC, P]: tile ec -> [p, dc, e] with contiguous rows
    DCl = D // P
    wqkT = np.ascontiguousarray(
        wqkT2.reshape(DCl, P, 2 * (D // P), P).transpose(2, 1, 0, 3)
    ).astype(BF_NP)
    wvT = np.ascontiguousarray(wv.T).astype(BF_NP)
    woT = np.ascontiguousarray(np.asarray(w_out, np.float64).T).astype(BF_NP)

    inv = 1.0 / (10000.0 ** (np.arange(0, HD, 2, dtype=np.float64) / HD))
    tpos = np.arange(S, dtype=np.float64)
    fr = np.outer(tpos, inv)
    emb = np.concatenate([fr, fr], axis=-1)  # [S, HD]
    cosT = np.cos(emb).T  # [HD, S]
    sinTn = np.sin(emb).T

    # signed rotate-half permutation, as matmul lhsT:
    # out[p] = sum_{p'} rotmT[p', p] * in[p'] = rot_half(in)[p]
    h2 = HD // 2
    rotmT = np.zeros((P, P), np.float32)
    for p in range(h2):
        rotmT[p + h2, p] = -1.0
    for p in range(h2, HD):
        rotmT[p - h2, p] = 1.0
    rotmT = rotmT.astype(BF_NP)

    scale = 1.0 / math.sqrt(HD)
    gq_a = np.ascontiguousarray(
        (np.asarray(q_gamma, np.float64) * scale).reshape(NH, P).T
    ).astype(np.float32)
    bq_a = np.ascontiguousarray(
        (np.asarray(q_beta, np.float64) * scale).reshape(NH, P).T
    ).astype(np.float32)
    gk_a = np.ascontiguousarray(
        np.asarray(k_gamma, np.float32).reshape(NH, P).T
    )
    bk_a = np.ascontiguousarray(
        np.asarray(k_beta, np.float32).reshape(NH, P).T
    )

    in_maps = []
    meta = []
    cores_per_batch = max(1, n_cores // B)
    for c in range(n_cores):
        b = c // cores_per_batch
        r = c % cores_per_batch
        blocks = _q_blocks(r if cores_per_batch > 1 else 0, n_blocks)
        blocks = blocks[: NQTOK // 512]
        qtok = np.concatenate(
            [np.arange(bk_ * 512, (bk_ + 1) * 512) for bk_ in blocks]
        )
        xb = np.asarray(x[b], np.float32)  # [S, D]
        xT = np.ascontiguousarray(xb.T).astype(BF_NP)    # [D, S]
        xTq = np.ascontiguousarray(xT[:, qtok])
        cosq = np.ascontiguousarray(cosT[:, qtok]).astype(BF_NP)
        sinq = np.ascontiguousarray(sinTn[:, qtok]).astype(BF_NP)
        cosk = np.ascontiguousarray(cosT[:, :S]).astype(BF_NP)
        sink = np.ascontiguousarray(sinTn[:, :S]).astype(BF_NP)

        masks = np.zeros([NQ, P, MAXM, QT], np.float32)
        for t in range(NQ):
            assert QT == 512
            q_start = blocks[t] * 512
            qq = np.arange(QT)
            kk = np.arange(P)
            for mi, kc in enumerate(masked[t]):
                masks[t, :, mi, :] = (
                    (kc * P + kk[:, None]) <= (q_start + qq[None, :])
                ).astype(np.float32)
        masks = masks.astype(BF_NP)

        in_maps.append(dict(
            xTq=xTq, xT=xT, wqkT=wqkT, wvT=wvT, woT=woT,
            cosq=cosq, sinqn=sinq, cosk=cosk, sinkn=sink,
            gq=gq_a, bq=bq_a, gk=gk_a, bk=bk_a, masks=masks,
            rotm=rotmT,
            onesc=np.ones((P, 1), np.float32),
            onesr=np.ones((1, P), np.float32),
        ))
        meta.append(dict(b=b, qtok=qtok))
    return in_maps, meta


_PROGRAM_CACHE = {}


def _get_program(cfg_key, cfg):
    if cfg_key not in _PROGRAM_CACHE:
        _PROGRAM_CACHE[cfg_key] = build_program(cfg)
    return _PROGRAM_CACHE[cfg_key]


def run_full(x, w_in, w_out, q_gamma, q_beta, k_gamma, k_beta,
             trace=False):
    from concourse.bass_utils import run_bass_kernel_spmd

    cfg = FULL_CFG
    B = x.shape[0]
    n_cores = 2 * B
    in_maps, meta = make_host_data(
        x, w_in, w_out, q_gamma, q_beta, k_gamma, k_beta, cfg,
        n_cores=n_cores,
    )
    nc = _get_program("full", cfg)
    res = run_bass_kernel_spmd(
        nc, in_maps, core_ids=list(range(n_cores)), trace=trace,
    )
    S, D = cfg["S"], cfg["D"]
    out = np.empty((B, S, D), np.float32)
    for c in range(n_cores):
        o = res.results[c]["out"]  # [D, NQTOK]
        out[meta[c]["b"], meta[c]["qtok"], :] = o.T
    return out, res


def kernel(x, w_in, w_out, q_gamma, q_beta, k_gamma, k_beta, n_heads=16,
           **_ignored):
    x = np.asarray(x, np.float32)
    assert int(np.asarray(n_heads)) * HD == x.shape[-1]
    out, _ = run_full(
        np.asarray(x, np.float32),
        np.asarray(w_in, np.float32),
        np.asarray(w_out, np.float32),
        np.asarray(q_gamma, np.float32),
        np.asarray(q_beta, np.float32),
        np.asarray(k_gamma, np.float32),
        np.asarray(k_beta, np.float32),
    )
    return out


# revision 14
# speedup vs baseline: 1.1575x; 1.1571x over previous
"""Trainium2 Bass kernel for a custom attention block (qkv-proj + LN(q,k) +
RoPE + causal attention + out-proj), distributed over 8 NeuronCores.

Sharding: 2 cores per batch (B=4). Core role r=c%2 takes q-token blocks
{0,3} (r=0) or {1,2} (r=1) of 512 tokens; every core computes K/V for the
full 2048-token sequence of its batch (no collectives). The compiled
program is identical on all cores; all per-core differences are input
data (sliced x^T, cos/sin tables, causal masks).

v3: all matmul operands bf16 (same PE rate as fp32r at moving>=256, half
the DMA/SBUF); q/k/v round-trip DRAM in bf16, streamed per-head with
double buffering. Projection calls are split into a matmul part and a
finish part (LN+rope) and interleaved, so the in-order PE queue always
has the next phase's matmuls while the DVE works on the previous phase's
LN/rope. The attention slot loop is software-pipelined (QK of slot s+3
issues before PV of slot s). DMA issue is spread across engines (weights
on GpSimd, attention slabs on Vector, rest on Sync) because a single
engine's dma_start rate (~2.4/us) is a serial bottleneck. V-store DMAs
are batched 4 heads per descriptor-set.

LN: mean subtraction is folded into host-pre-centered w_in rows; variance
comes from Square + ones-matmul partition reduction; rsqrt(var+eps) is
computed as Exp(-0.5*Ln(var+eps)) so all ACT functions live in one table
set (natural_log_exp_and_others).
"""

import math

import numpy as np

import concourse.bass as bass
import concourse.mybir as mybir
import concourse.tile as tile
from concourse import bacc
from concourse.bass import ds, ts

F32 = mybir.dt.float32
F32R = mybir.dt.float32r
BF = mybir.dt.bfloat16
AF = mybir.ActivationFunctionType
OP = mybir.AluOpType

P = 128
HD = 128

FULL_CFG = dict(
    D=2048,           # model dim (contraction dim for projections)
    S=2048,           # kv tokens per core (full sequence of its batch)
    NQTOK=1024,       # q tokens per core
    PT=256,           # projection s-tile width (moving dim)
    QT=512,           # attention q-tile width (moving dim)
    slots=(8, 16),    # kv 128-chunks visited per q-tile
    masked=(tuple(range(0, 8)), tuple(range(8, 16))),  # slots that get a mask
    EXP_BIAS=8.0,
    EPS=1e-5,
)


def _r(ap):
    """fp32 -> fp32r view for matmul operands."""
    return ap.bitcast(F32R)


def build_program(cfg):
    D = cfg["D"]
    S = cfg["S"]
    NQTOK = cfg["NQTOK"]
    PT = cfg["PT"]
    QT = cfg["QT"]
    slots = cfg["slots"]
    masked = cfg["masked"]
    EXP_BIAS = cfg["EXP_BIAS"]
    EPS = cfg["EPS"]

    NH = D // HD              # heads == e-chunks per q (and per k)
    DC = D // P               # contraction chunks
    NQ = NQTOK // QT          # q tiles
    S2 = S // 2               # kv half (x residency granularity)
    KC2 = S2 // P             # kv chunks per half
    MAXM = max(len(m) for m in masked)
    VEW = 512                 # v-proj e-tile width
    VET = D // VEW

    nc = bacc.Bacc("TRN2", target_bir_lowering=False, debug=False)

    # ---- I/O ----
    xTq = nc.dram_tensor("xTq", [D, NQTOK], BF, kind="ExternalInput").ap()
    xT = nc.dram_tensor("xT", [D, S], BF, kind="ExternalInput").ap()
    wqkT = nc.dram_tensor("wqkT", [2 * NH, P, DC, P], BF,
                          kind="ExternalInput").ap()
    wvT = nc.dram_tensor("wvT", [D, D], BF, kind="ExternalInput").ap()
    woT = nc.dram_tensor("woT", [D, D], BF, kind="ExternalInput").ap()
    cosq_i = nc.dram_tensor("cosq", [HD, NQTOK], BF, kind="ExternalInput").ap()
    sinq_i = nc.dram_tensor("sinqn", [HD, NQTOK], BF, kind="ExternalInput").ap()
    cosk_i = nc.dram_tensor("cosk", [HD, S], BF, kind="ExternalInput").ap()
    sink_i = nc.dram_tensor("sinkn", [HD, S], BF, kind="ExternalInput").ap()
    rotm_i = nc.dram_tensor("rotm", [P, P], BF, kind="ExternalInput").ap()
    onesc_i = nc.dram_tensor("onesc", [P, 1], F32, kind="ExternalInput").ap()
    onesr_i = nc.dram_tensor("onesr", [1, P], F32, kind="ExternalInput").ap()
    gq_i = nc.dram_tensor("gq", [P, NH], F32, kind="ExternalInput").ap()
    bq_i = nc.dram_tensor("bq", [P, NH], F32, kind="ExternalInput").ap()
    gk_i = nc.dram_tensor("gk", [P, NH], F32, kind="ExternalInput").ap()
    bk_i = nc.dram_tensor("bk", [P, NH], F32, kind="ExternalInput").ap()
    masks_i = nc.dram_tensor("masks", [NQ, P, MAXM, QT], BF,
                             kind="ExternalInput").ap()
    out_t = nc.dram_tensor("out", [D, NQTOK], F32, kind="ExternalOutput").ap()

    with tile.TileContext(nc) as tc:
        import contextlib

        ctx = contextlib.ExitStack()
        with ctx:
            sb = ctx.enter_context(tc.tile_pool(name="sb", bufs=1))
            psum = ctx.enter_context(tc.tile_pool(name="ps", bufs=1, space="PSUM"))
            dram = ctx.enter_context(tc.tile_pool(name="dram", bufs=1, space="DRAM"))

            # ---- DRAM scratch (bf16) ----
            qts = dram.tile([P, NH, NQTOK], BF, tag="qts", name="qts")
            kts = dram.tile([P, NH, S], BF, tag="kts", name="kts")
            vs = dram.tile([NH, S, HD], BF, tag="vs", name="vs")

            # ---- constants / small inputs ----
            ones_col = sb.tile([P, 1], F32, tag="ones_col", name="ones_col")
            nc.sync.dma_start(_r(ones_col), _r(onesc_i))
            ones_row = sb.tile([1, P], F32, tag="ones_row", name="ones_row")
            nc.sync.dma_start(_r(ones_row), _r(onesr_i))
            eps1 = sb.tile([1, 1], F32, tag="eps1", name="eps1")
            nc.vector.memset(eps1, EPS)
            zero1 = sb.tile([1, 1], F32, tag="zero1", name="zero1")
            nc.vector.memset(zero1, 0.0)
            nege = sb.tile([P, 1], F32, tag="nege", name="nege")
            nc.vector.memset(nege, -EXP_BIAS)
            ones_cb = sb.tile([P, 1], BF, tag="ones_cb", name="ones_cb")
            nc.vector.memset(ones_cb, 1.0)
            rotm = sb.tile([P, P], BF, tag="rotm", name="rotm")
            nc.sync.dma_start(rotm, rotm_i)
            gq = sb.tile([P, NH], F32, tag="gq", name="gq")
            nc.sync.dma_start(gq, gq_i)
            bq = sb.tile([P, NH], F32, tag="bq", name="bq")
            nc.sync.dma_start(bq, bq_i)
            gk = sb.tile([P, NH], F32, tag="gk", name="gk")
            nc.sync.dma_start(gk, gk_i)
            bk = sb.tile([P, NH], F32, tag="bk", name="bk")
            nc.sync.dma_start(bk, bk_i)
            # rope tables resident in SBUF (bf16)
            cosk = sb.tile([HD, S], BF, tag="cosk", name="cosk")
            nc.sync.dma_start(cosk, cosk_i)
            sink = sb.tile([HD, S], BF, tag="sink", name="sink")
            nc.sync.dma_start(sink, sink_i)
            cosq = sb.tile([HD, NQTOK], BF, tag="cosq", name="cosq")
            nc.sync.dma_start(cosq, cosq_i)
            sinq = sb.tile([HD, NQTOK], BF, tag="sinq", name="sinq")
            nc.sync.dma_start(sinq, sinq_i)

            def proj_mm(x_src, tok0_src, n_st, wcol_off):
                """Matmul part of a projection over n_st*PT tokens: returns
                (xts, holds, pstats) with holds filled (pre-LN, bf16) and
                pstats accumulating sum-of-squares per st slice."""
                xts = []
                for st in range(n_st):
                    xt = sb.tile([P, DC, PT], BF, tag="xt", bufs=4,
                                 name="xt")
                    for d in range(DC):
                        nc.sync.dma_start(
                            xt[:, d],
                            x_src[ds(d * P, P), ds(tok0_src + st * PT, PT)],
                        )
                    xts.append(xt)
                holds = [
                    sb.tile([P, NH, PT], BF, tag="hold", bufs=2 * n_st,
                            name="hold")
                    for _ in range(n_st)
                ]
                assert n_st % 2 == 0
                pstats = [
                    psum.tile([1, 2 * PT], F32, tag="stat", bufs=2,
                              name="pstats")
                    for _ in range(n_st // 2)
                ]
                for ec in range(NH):
                    w = sb.tile([P, DC, P], BF, tag="w", bufs=3, name="w")
                    nc.gpsimd.dma_start(w, wqkT[wcol_off + ec])
                    pss = {st: psum.tile([P, PT], F32, tag="mm", bufs=4,
                                         name="psp")
                           for st in range(n_st)}
                    for d in range(DC):
                        for st in range(n_st):
                            nc.tensor.matmul(
                                pss[st],
                                lhsT=w[:, d],
                                rhs=xts[st][:, d],
                                start=(d == 0),
                                stop=(d == DC - 1),
                            )
                    sq_all = sb.tile([P, n_st * PT], F32, tag="sq", bufs=2,
                                     name="sq_all")
                    for st in range(n_st):
                        nc.scalar.copy(holds[st][:, ec], pss[st])
                        nc.scalar.square(_r(sq_all[:, ds(st * PT, PT)]),
                                         pss[st])
                    for hs in range(n_st // 2):
                        nc.tensor.matmul(
                            pstats[hs],
                            lhsT=_r(ones_col),
                            rhs=_r(sq_all[:, ds(hs * 2 * PT, 2 * PT)]),
                            start=(ec == 0),
                            stop=(ec == NH - 1),
                        )
                return xts, holds, pstats

            def proj_fin(holds, pstats, n_st, cos_pair, cos_off, g_sb, b_sb,
                         dst, tok0_dst):
                """LN apply + rope + store for a projection's holds."""
                cos_t, sin_t = cos_pair
                for st in range(n_st):
                    hold = holds[st]
                    csl = ds(cos_off + st * PT, PT)
                    pst = pstats[st // 2][:, ds((st % 2) * PT, PT)]
                    # rsig = exp(-0.5 * ln(sumsq/D + eps))
                    lnv = sb.tile([1, PT], F32, tag="stats_sb", bufs=4,
                                  name="lnv")
                    nc.scalar.activation(lnv, pst, AF.Ln, scale=1.0 / D,
                                         bias=eps1)
                    rsig = sb.tile([1, PT], F32, tag="stats_sb", bufs=4,
                                   name="rsig")
                    nc.scalar.activation(_r(rsig), lnv, AF.Exp, bias=zero1,
                                         scale=-0.5)
                    ps_rep = psum.tile([P, PT], F32, tag="mm", bufs=4,
                                       name="ps_rep")
                    nc.tensor.matmul(ps_rep, lhsT=_r(ones_row), rhs=_r(rsig))
                    rep = sb.tile([P, PT], BF, tag="rep", bufs=2, name="rep")
                    nc.scalar.copy(rep, ps_rep)
                    # pass 1: LN apply (DVE) for all chunks first, so the
                    # rotation matmuls never head-of-line block the in-order
                    # PE stream on a DVE dependency.
                    for ec in range(NH):
                        ch = hold[:, ec]
                        nc.vector.tensor_tensor(ch, ch, rep, op=OP.mult)
                        nc.vector.tensor_scalar(
                            ch, ch,
                            scalar1=g_sb[:, ds(ec, 1)],
                            scalar2=b_sb[:, ds(ec, 1)],
                            op0=OP.mult, op1=OP.add,
                        )
                    # pass 2: rotation matmuls stream back-to-back
                    for ec in range(NH):
                        ch = hold[:, ec]
                        ps_rot = psum.tile([P, PT], F32, tag="mm", bufs=4,
                                           name="ps_rot")
                        nc.tensor.matmul(ps_rot, lhsT=rotm, rhs=ch)
                        tmp = sb.tile([P, PT], BF, tag="tmp", bufs=2,
                                      name="rtmp")
                        nc.vector.tensor_tensor(tmp, ps_rot, sin_t[:, csl],
                                                op=OP.mult)
                        nc.vector.tensor_tensor(ch, ch, cos_t[:, csl],
                                                op=OP.mult)
                        nc.vector.tensor_tensor(ch, ch, tmp, op=OP.add)
                    nc.sync.dma_start(
                        dst[:, :, ds(tok0_dst + st * PT, PT)], hold
                    )

            def v_proj(half):
                """v-projection for one x half: x chunks stationary."""
                for scg in range(KC2 // 4):
                    scs = [scg * 4 + i for i in range(4)]
                    for et in range(VET):
                        psv = {sc: psum.tile([P, VEW], F32, tag="mm", bufs=4,
                                             name="psv")
                               for sc in scs}
                        for d in range(DC):
                            wv = sb.tile([P, VEW], BF, tag="wv", bufs=3,
                                         name="wv")
                            nc.gpsimd.dma_start(
                                wv, wvT[ds(d * P, P), ds(et * VEW, VEW)]
                            )
                            xv = sb.tile([P, 4 * P], BF, tag="xv", bufs=3,
                                         name="xv")
                            nc.gpsimd.dma_start(
                                xv,
                                xT[ds(d * P, P),
                                   ds(half * S2 + scg * 4 * P, 4 * P)],
                            )
                            for i, sc in enumerate(scs):
                                nc.tensor.matmul(
                                    psv[sc],
                                    lhsT=xv[:, ds(i * P, P)],
                                    rhs=wv,
                                    start=(d == 0),
                                    stop=(d == DC - 1),
                                )
                        for sc in scs:
                            vsb = sb.tile([P, VEW], BF, tag="vsb", bufs=2,
                                          name="vsb")
                            nc.scalar.copy(vsb, psv[sc])
                            gsc = half * KC2 + sc
                            # batched store: 4 heads in one DMA
                            dst = vs[ds(et * (VEW // HD), VEW // HD),
                                     ds(gsc * P, P), :]
                            nc.sync.dma_start(
                                dst.rearrange("h p hd -> p h hd"),
                                vsb.rearrange("p (h hd) -> p h hd", hd=HD),
                            )

            # ---- Projections, interleaved so PE always has matmuls while
            # the DVE finishes the previous call's LN/rope ----
            a_xts, a_holds, a_pstats = proj_mm(xTq, 0, NQTOK // PT, 0)
            b0_xts, b0_holds, b0_pstats = proj_mm(xT, 0, S2 // PT, NH)
            proj_fin(a_holds, a_pstats, NQTOK // PT, (cosq, sinq), 0,
                     gq, bq, qts, 0)
            v_proj(0)
            proj_fin(b0_holds, b0_pstats, S2 // PT, (cosk, sink), 0,
                     gk, bk, kts, 0)
            b1_xts, b1_holds, b1_pstats = proj_mm(xT, S2, S2 // PT, NH)
            v_proj(1)
            proj_fin(b1_holds, b1_pstats, S2 // PT, (cosk, sink), S2,
                     gk, bk, kts, S2)

            # ---- Phase D+E: attention + out-projection per q tile ----
            for t in range(NQ):
                qsl_off = t * QT
                mt = sb.tile([P, MAXM, QT], BF, tag="masks", bufs=1,
                             name="mt")
                nc.sync.dma_start(mt, masks_i[t])
                mpos = {kc: i for i, kc in enumerate(masked[t])}
                n_slots = slots[t]
                n_half = (n_slots + KC2 - 1) // KC2  # kv halves needed
                ots = sb.tile([P, NH, QT], BF, tag="ots", bufs=1, name="ots")
                for h in range(NH):
                    qsl = sb.tile([P, QT], BF, tag="qslab", bufs=2,
                                  name="qsl")
                    nc.gpsimd.dma_start(qsl, qts[:, h, ds(qsl_off, QT)])
                    ksl = sb.tile([P, n_half * S2], BF, tag="kslab", bufs=2,
                                  name="ksl")
                    nc.gpsimd.dma_start(ksl, kts[:, h, ds(0, n_half * S2)])
                    vsl = sb.tile([P, n_half * KC2, HD], BF, tag="vslab",
                                  bufs=2, name="vsl")
                    nc.gpsimd.dma_start(
                        vsl,
                        vs[h, ds(0, n_half * S2), :].rearrange(
                            "(kc p) hd -> p kc hd", p=P
                        ),
                    )
                    psout = psum.tile([P, QT], F32, tag="acc", bufs=2,
                                      name="psout")
                    psden = psum.tile([1, QT], F32, tag="stat", bufs=2,
                                      name="psden")
                    # software-pipelined slot loop: QK runs PIPE slots ahead
                    # of exp/PV so the in-order PE stream never waits on the
                    # Scalar engine.
                    PIPE = 3
                    pssq = {}
                    ets = {}

                    def issue_qk(s):
                        pssq[s] = psum.tile([P, QT], F32, tag="mm", bufs=4,
                                            name="pssq")
                        nc.tensor.matmul(
                            pssq[s],
                            lhsT=ksl[:, ds(s * P, P)],
                            rhs=qsl,
                        )

                    def issue_exp(s):
                        et = sb.tile([P, QT], BF, tag="exp", bufs=PIPE + 1,
                                     name="et")
                        nc.scalar.activation(et, pssq[s], AF.Exp, bias=nege)
                        del pssq[s]
                        if s in mpos:
                            nc.vector.tensor_tensor(
                                et, et, mt[:, mpos[s]], op=OP.mult
                            )
                        ets[s] = et

                    for s in range(min(PIPE, n_slots)):
                        issue_qk(s)
                    issue_exp(0)
                    for s in range(n_slots):
                        if s + PIPE < n_slots:
                            issue_qk(s + PIPE)
                        if s + 1 < n_slots:
                            issue_exp(s + 1)
                        et = ets.pop(s)
                        nc.tensor.matmul(
                            psout,
                            lhsT=vsl[:, s],
                            rhs=et,
                            start=(s == 0),
                            stop=(s == n_slots - 1),
                        )
                        nc.tensor.matmul(
                            psden,
                            lhsT=ones_cb,
                            rhs=et,
                            start=(s == 0),
                            stop=(s == n_slots - 1),
                        )
                    rec0 = sb.tile([1, QT], F32, tag="stats_sb", bufs=4,
                                   name="rec0")
                    with nc.allow_low_precision(
                        reason="denominator reciprocal, 18 bits is plenty"
                    ):
                        nc.vector.reciprocal_approx_fast(rec0, psden)
                    rec = sb.tile([1, QT], F32, tag="stats_sb", bufs=4,
                                  name="rec")
                    nc.scalar.activation(_r(rec), rec0, AF.Copy)
                    psr = psum.tile([P, QT], F32, tag="mm", bufs=4,
                                    name="psr")
                    nc.tensor.matmul(psr, lhsT=_r(ones_row), rhs=_r(rec))
                    rsb = sb.tile([P, QT], BF, tag="rsb", bufs=2, name="rsb")
                    nc.scalar.copy(rsb, psr)
                    nc.vector.tensor_tensor(ots[:, h], psout, rsb, op=OP.mult)

                # ---- Phase E: out-projection for this q tile (from SBUF),
                # 4 psf banks per wo load ----
                EG = 4
                for eg in range(NH // EG):
                    psf = [
                        psum.tile([P, QT], F32, tag="mm", bufs=4, name="psf")
                        for _ in range(EG)
                    ]
                    for h in range(NH):
                        wo = sb.tile([P, EG * P], BF, tag="wo", bufs=3,
                                     name="wo")
                        nc.gpsimd.dma_start(
                            wo, woT[ds(h * P, P), ds(eg * EG * P, EG * P)]
                        )
                        for x in range(EG):
                            nc.tensor.matmul(
                                psf[x],
                                lhsT=wo[:, ds(x * P, P)],
                                rhs=ots[:, h],
                                start=(h == 0),
                                stop=(h == NH - 1),
                            )
                    for x in range(EG):
                        fsb = sb.tile([P, QT], F32, tag="fsb", bufs=2,
                                      name="fsb")
                        nc.vector.tensor_copy(fsb, psf[x])
                        nc.sync.dma_start(
                            out_t[ds((eg * EG + x) * P, P), ds(qsl_off, QT)],
                            fsb,
                        )

    nc.compile()
    return nc


# --------------------------------------------------------------------------
# Host-side prep and driver
# --------------------------------------------------------------------------

def _q_blocks(role, n_blocks):
    """q-block indices (each 512 tokens) for a core role."""
    if n_blocks == 4:
        return [0, 3] if role == 0 else [1, 2]
    return list(range(n_blocks))


def make_host_data(x, w_in, w_out, q_gamma, q_beta, k_gamma, k_beta, cfg,
                   n_cores=None):
    """Build per-core in_maps (list of dicts) + assembly metadata."""
    import ml_dtypes

    BF_NP = ml_dtypes.bfloat16

    D = cfg["D"]
    S = cfg["S"]
    NQTOK = cfg["NQTOK"]
    QT = cfg["QT"]
    slots = cfg["slots"]
    masked = cfg["masked"]
    NH = D // HD
    NQ = NQTOK // QT
    MAXM = max(len(m) for m in masked)
    B = x.shape[0]
    n_blocks = S // 512
    if n_cores is None:
        n_cores = B * (2048 // NQTOK) if S == 2048 else B

    w64 = np.asarray(w_in, np.float64)
    wq = w64[0:D]
    wk = w64[D:2 * D]
    wv = w64[2 * D:3 * D]
    wq_c = wq - wq.mean(axis=0, keepdims=True)
    wk_c = wk - wk.mean(axis=0, keepdims=True)
    wqkT2 = np.concatenate([wq_c.T, wk_c.T], axis=1)
    # pre-tile to [2*NH, P, DC, P]: tile ec -> [p, dc, e] with contiguous rows
    wqkT = np.ascontiguousarray(
        wqkT2.reshape(D // P, P, 2 * (D // P), P).transpose(2, 1, 0, 3)
    ).astype(BF_NP)
    wvT = np.ascontiguousarray(wv.T).astype(BF_NP)
    woT = np.ascontiguousarray(np.asarray(w_out, np.float64).T).astype(BF_NP)

    inv = 1.0 / (10000.0 ** (np.arange(0, HD, 2, dtype=np.float64) / HD))
    tpos = np.arange(S, dtype=np.float64)
    fr = np.outer(tpos, inv)
    emb = np.concatenate([fr, fr], axis=-1)  # [S, HD]
    cosT = np.cos(emb).T  # [HD, S]
    sinTn = np.sin(emb).T

    # signed rotate-half permutation, as matmul lhsT:
    # out[p] = sum_{p'} rotmT[p', p] * in[p'] = rot_half(in)[p]
    h2 = HD // 2
    rotmT = np.zeros((P, P), np.float32)
    for p in range(h2):
        rotmT[p + h2, p] = -1.0
    for p in range(h2, HD):
        rotmT[p - h2, p] = 1.0
    rotmT = rotmT.astype(BF_NP)

    scale = 1.0 / math.sqrt(HD)
    gq_a = np.ascontiguousarray(
        (np.asarray(q_gamma, np.float64) * scale).reshape(NH, P).T
    ).astype(np.float32)
    bq_a = np.ascontiguousarray(
        (np.asarray(q_beta, np.float64) * scale).reshape(NH, P).T
    ).astype(np.float32)
    gk_a = np.ascontiguousarray(
        np.asarray(k_gamma, np.float32).reshape(NH, P).T
    )
    bk_a = np.ascontiguousarray(
        np.asarray(k_beta, np.float32).reshape(NH, P).T
    )

    in_maps = []
    meta = []
    cores_per_batch = max(1, n_cores // B)
    for c in range(n_cores):
        b = c // cores_per_batch
        r = c % cores_per_batch
        blocks = _q_blocks(r if cores_per_batch > 1 else 0, n_blocks)
        blocks = blocks[: NQTOK // 512]
        qtok = np.concatenate(
            [np.arange(bk_ * 512, (bk_ + 1) * 512) for bk_ in blocks]
        )
        xb = np.asarray(x[b], np.float32)  # [S, D]
        xT = np.ascontiguousarray(xb.T).astype(BF_NP)    # [D, S]
        xTq = np.ascontiguousarray(xT[:, qtok])
        cosq = np.ascontiguousarray(cosT[:, qtok]).astype(BF_NP)
        sinq = np.ascontiguousarray(sinTn[:, qtok]).astype(BF_NP)
        cosk = np.ascontiguousarray(cosT[:, :S]).astype(BF_NP)
        sink = np.ascontiguousarray(sinTn[:, :S]).astype(BF_NP)

        masks = np.zeros([NQ, P, MAXM, QT], np.float32)
        for t in range(NQ):
            assert QT == 512
            q_start = blocks[t] * 512
            qq = np.arange(QT)
            kk = np.arange(P)
            for mi, kc in enumerate(masked[t]):
                masks[t, :, mi, :] = (
                    (kc * P + kk[:, None]) <= (q_start + qq[None, :])
                ).astype(np.float32)
        masks = masks.astype(BF_NP)

        in_maps.append(dict(
            xTq=xTq, xT=xT, wqkT=wqkT, wvT=wvT, woT=woT,
            cosq=cosq, sinqn=sinq, cosk=cosk, sinkn=sink,
            gq=gq_a, bq=bq_a, gk=gk_a, bk=bk_a, masks=masks,
            rotm=rotmT,
            onesc=np.ones((P, 1), np.float32),
            onesr=np.ones((1, P), np.float32),
        ))
        meta.append(dict(b=b, qtok=qtok))
    return in_maps, meta


_PROGRAM_CACHE = {}


def _get_program(cfg_key, cfg):
    if cfg_key not in _PROGRAM_CACHE:
        _PROGRAM_CACHE[cfg_key] = build_program(cfg)
    return _PROGRAM_CACHE[cfg_key]


def run_full(x, w_in, w_out, q_gamma, q_beta, k_gamma, k_beta,
             trace=False):
    from concourse.bass_utils import run_bass_kernel_spmd

    cfg = FULL_CFG
    B = x.shape[0]
    n_cores = 2 * B
    in_maps, meta = make_host_data(
        x, w_in, w_out, q_gamma, q_beta, k_gamma, k_beta, cfg,
        n_cores=n_cores,
    )
    nc = _get_program("full", cfg)
    res = run_bass_kernel_spmd(
        nc, in_maps, core_ids=list(range(n_cores)), trace=trace,
    )
    S, D = cfg["S"], cfg["D"]
    out = np.empty((B, S, D), np.float32)
    for c in range(n_cores):
        o = res.results[c]["out"]  # [D, NQTOK]
        out[meta[c]["b"], meta[c]["qtok"], :] = o.T
    return out, res


def kernel(x, w_in, w_out, q_gamma, q_beta, k_gamma, k_beta, n_heads=16,
           **_ignored):
    x = np.asarray(x, np.float32)
    assert int(np.asarray(n_heads)) * HD == x.shape[-1]
    out, _ = run_full(
        np.asarray(x, np.float32),
        np.asarray(w_in, np.float32),
        np.asarray(w_out, np.float32),
        np.asarray(q_gamma, np.float32),
        np.asarray(q_beta, np.float32),
        np.asarray(k_gamma, np.float32),
        np.asarray(k_beta, np.float32),
    )
    return out


# revision 15
# speedup vs baseline: 1.3326x; 1.1513x over previous
"""Trainium2 Bass kernel for a custom attention block (qkv-proj + LN(q,k) +
RoPE + causal attention + out-proj), distributed over 8 NeuronCores.

Sharding: 2 cores per batch (B=4). Core role r=c%2 takes q-token blocks
{0,3} (r=0) or {1,2} (r=1) of 512 tokens; every core computes K/V for the
full 2048-token sequence of its batch (no collectives). The compiled
program is identical on all cores; all per-core differences are input
data (sliced x^T, cos/sin tables, causal masks).

v3: all matmul operands bf16 (same PE rate as fp32r at moving>=256, half
the DMA/SBUF); q/k/v round-trip DRAM in bf16, streamed per-head with
double buffering. Projection calls are split into a matmul part and a
finish part (LN+rope) and interleaved, so the in-order PE queue always
has the next phase's matmuls while the DVE works on the previous phase's
LN/rope. The attention slot loop is software-pipelined (QK of slot s+3
issues before PV of slot s). DMA issue is spread across engines (weights
on GpSimd, attention slabs on Vector, rest on Sync) because a single
engine's dma_start rate (~2.4/us) is a serial bottleneck. V-store DMAs
are batched 4 heads per descriptor-set.

LN: mean subtraction is folded into host-pre-centered w_in rows; variance
comes from Square + ones-matmul partition reduction; rsqrt(var+eps) is
computed as Exp(-0.5*Ln(var+eps)) so all ACT functions live in one table
set (natural_log_exp_and_others).
"""

import math

import numpy as np

import concourse.bass as bass
import concourse.mybir as mybir
import concourse.tile as tile
from concourse import bacc
from concourse.bass import ds, ts

F32 = mybir.dt.float32
F32R = mybir.dt.float32r
BF = mybir.dt.bfloat16
AF = mybir.ActivationFunctionType
OP = mybir.AluOpType

P = 128
HD = 128

FULL_CFG = dict(
    D=2048,           # model dim (contraction dim for projections)
    S=2048,           # kv tokens per core (full sequence of its batch)
    NQTOK=1024,       # q tokens per core
    PT=256,           # projection s-tile width (moving dim)
    QT=512,           # attention q-tile width (moving dim)
    slots=(8, 16),    # kv 128-chunks visited per q-tile
    masked=(tuple(range(0, 8)), tuple(range(8, 16))),  # slots that get a mask
    EXP_BIAS=8.0,
    EPS=1e-5,
)


def _r(ap):
    """fp32 -> fp32r view for matmul operands."""
    return ap.bitcast(F32R)


def build_program(cfg):
    D = cfg["D"]
    S = cfg["S"]
    NQTOK = cfg["NQTOK"]
    PT = cfg["PT"]
    QT = cfg["QT"]
    slots = cfg["slots"]
    masked = cfg["masked"]
    EXP_BIAS = cfg["EXP_BIAS"]
    EPS = cfg["EPS"]

    NH = D // HD              # heads == e-chunks per q (and per k)
    DC = D // P               # contraction chunks
    NQ = NQTOK // QT          # q tiles
    S2 = S // 2               # kv half (x residency granularity)
    KC2 = S2 // P             # kv chunks per half
    MAXM = max(len(m) for m in masked)
    VEW = 512                 # v-proj e-tile width
    VET = D // VEW

    nc = bacc.Bacc("TRN2", target_bir_lowering=False, debug=False)

    # ---- I/O ----
    xTq = nc.dram_tensor("xTq", [D, NQTOK], BF, kind="ExternalInput").ap()
    xT = nc.dram_tensor("xT", [D, S], BF, kind="ExternalInput").ap()
    wqkT = nc.dram_tensor("wqkT", [2 * NH, P, DC, P], BF,
                          kind="ExternalInput").ap()
    wvT = nc.dram_tensor("wvT", [D, D], BF, kind="ExternalInput").ap()
    woT = nc.dram_tensor("woT", [D, D], BF, kind="ExternalInput").ap()
    cosq_i = nc.dram_tensor("cosq", [HD, NQTOK], BF, kind="ExternalInput").ap()
    sinq_i = nc.dram_tensor("sinqn", [HD, NQTOK], BF, kind="ExternalInput").ap()
    cosk_i = nc.dram_tensor("cosk", [HD, S], BF, kind="ExternalInput").ap()
    sink_i = nc.dram_tensor("sinkn", [HD, S], BF, kind="ExternalInput").ap()
    rotm_i = nc.dram_tensor("rotm", [P, P], BF, kind="ExternalInput").ap()
    onesc_i = nc.dram_tensor("onesc", [P, 1], F32, kind="ExternalInput").ap()
    onesr_i = nc.dram_tensor("onesr", [1, P], F32, kind="ExternalInput").ap()
    gq_i = nc.dram_tensor("gq", [P, NH], F32, kind="ExternalInput").ap()
    bq_i = nc.dram_tensor("bq", [P, NH], F32, kind="ExternalInput").ap()
    gk_i = nc.dram_tensor("gk", [P, NH], F32, kind="ExternalInput").ap()
    bk_i = nc.dram_tensor("bk", [P, NH], F32, kind="ExternalInput").ap()
    masks_i = nc.dram_tensor("masks", [NQ, P, MAXM, QT], BF,
                             kind="ExternalInput").ap()
    out_t = nc.dram_tensor("out", [D, NQTOK], F32, kind="ExternalOutput").ap()

    with tile.TileContext(nc) as tc:
        import contextlib

        ctx = contextlib.ExitStack()
        with ctx:
            sb = ctx.enter_context(tc.tile_pool(name="sb", bufs=1))
            psum = ctx.enter_context(tc.tile_pool(name="ps", bufs=1, space="PSUM"))
            dram = ctx.enter_context(tc.tile_pool(name="dram", bufs=1, space="DRAM"))

            # ---- DRAM scratch (bf16) ----
            qts = dram.tile([P, NH, NQTOK], BF, tag="qts", name="qts")
            kts = dram.tile([P, NH, S], BF, tag="kts", name="kts")
            vs = dram.tile([NH, P, S // P, HD], BF, tag="vs", name="vs")

            # ---- constants / small inputs ----
            ones_col = sb.tile([P, 1], F32, tag="ones_col", name="ones_col")
            nc.sync.dma_start(_r(ones_col), _r(onesc_i))
            ones_row = sb.tile([1, P], F32, tag="ones_row", name="ones_row")
            nc.sync.dma_start(_r(ones_row), _r(onesr_i))
            eps1 = sb.tile([1, 1], F32, tag="eps1", name="eps1")
            nc.vector.memset(eps1, EPS)
            zero1 = sb.tile([1, 1], F32, tag="zero1", name="zero1")
            nc.vector.memset(zero1, 0.0)
            nege = sb.tile([P, 1], F32, tag="nege", name="nege")
            nc.vector.memset(nege, -EXP_BIAS)
            ones_cb = sb.tile([P, 1], BF, tag="ones_cb", name="ones_cb")
            nc.vector.memset(ones_cb, 1.0)
            rotm = sb.tile([P, P], BF, tag="rotm", name="rotm")
            nc.sync.dma_start(rotm, rotm_i)
            gq = sb.tile([P, NH], F32, tag="gq", name="gq")
            nc.sync.dma_start(gq, gq_i)
            bq = sb.tile([P, NH], F32, tag="bq", name="bq")
            nc.sync.dma_start(bq, bq_i)
            gk = sb.tile([P, NH], F32, tag="gk", name="gk")
            nc.sync.dma_start(gk, gk_i)
            bk = sb.tile([P, NH], F32, tag="bk", name="bk")
            nc.sync.dma_start(bk, bk_i)
            # rope tables resident in SBUF (bf16)
            cosk = sb.tile([HD, S], BF, tag="cosk", name="cosk")
            nc.sync.dma_start(cosk, cosk_i)
            sink = sb.tile([HD, S], BF, tag="sink", name="sink")
            nc.sync.dma_start(sink, sink_i)
            cosq = sb.tile([HD, NQTOK], BF, tag="cosq", name="cosq")
            nc.sync.dma_start(cosq, cosq_i)
            sinq = sb.tile([HD, NQTOK], BF, tag="sinq", name="sinq")
            nc.sync.dma_start(sinq, sinq_i)

            def proj_mm(x_src, tok0_src, n_st, wcol_off):
                """Matmul part of a projection over n_st*PT tokens: returns
                (xts, holds, pstats) with holds filled (pre-LN, bf16) and
                pstats accumulating sum-of-squares per st slice."""
                xts = []
                for st in range(n_st):
                    xt = sb.tile([P, DC, PT], BF, tag="xt", bufs=4,
                                 name="xt")
                    for d in range(DC):
                        nc.sync.dma_start(
                            xt[:, d],
                            x_src[ds(d * P, P), ds(tok0_src + st * PT, PT)],
                        )
                    xts.append(xt)
                holds = [
                    sb.tile([P, NH, PT], BF, tag="hold", bufs=2 * n_st,
                            name="hold")
                    for _ in range(n_st)
                ]
                assert n_st % 2 == 0
                pstats = [
                    psum.tile([1, 2 * PT], F32, tag="stat", bufs=2,
                              name="pstats")
                    for _ in range(n_st // 2)
                ]
                for ec in range(NH):
                    w = sb.tile([P, DC, P], BF, tag="w", bufs=3, name="w")
                    nc.gpsimd.dma_start(w, wqkT[wcol_off + ec])
                    pss = {st: psum.tile([P, PT], F32, tag="mm", bufs=4,
                                         name="psp")
                           for st in range(n_st)}
                    for d in range(DC):
                        for st in range(n_st):
                            nc.tensor.matmul(
                                pss[st],
                                lhsT=w[:, d],
                                rhs=xts[st][:, d],
                                start=(d == 0),
                                stop=(d == DC - 1),
                            )
                    sq_all = sb.tile([P, n_st * PT], F32, tag="sq", bufs=2,
                                     name="sq_all")
                    for st in range(n_st):
                        nc.scalar.copy(holds[st][:, ec], pss[st])
                        nc.scalar.square(_r(sq_all[:, ds(st * PT, PT)]),
                                         pss[st])
                    for hs in range(n_st // 2):
                        nc.tensor.matmul(
                            pstats[hs],
                            lhsT=_r(ones_col),
                            rhs=_r(sq_all[:, ds(hs * 2 * PT, 2 * PT)]),
                            start=(ec == 0),
                            stop=(ec == NH - 1),
                        )
                return xts, holds, pstats

            def proj_fin(holds, pstats, n_st, cos_pair, cos_off, g_sb, b_sb,
                         dst, tok0_dst):
                """LN apply + rope + store for a projection's holds."""
                cos_t, sin_t = cos_pair
                for st in range(n_st):
                    hold = holds[st]
                    csl = ds(cos_off + st * PT, PT)
                    pst = pstats[st // 2][:, ds((st % 2) * PT, PT)]
                    # rsig = exp(-0.5 * ln(sumsq/D + eps))
                    lnv = sb.tile([1, PT], F32, tag="stats_sb", bufs=4,
                                  name="lnv")
                    nc.scalar.activation(lnv, pst, AF.Ln, scale=1.0 / D,
                                         bias=eps1)
                    rsig = sb.tile([1, PT], F32, tag="stats_sb", bufs=4,
                                   name="rsig")
                    nc.scalar.activation(_r(rsig), lnv, AF.Exp, bias=zero1,
                                         scale=-0.5)
                    ps_rep = psum.tile([P, PT], F32, tag="mm", bufs=4,
                                       name="ps_rep")
                    nc.tensor.matmul(ps_rep, lhsT=_r(ones_row), rhs=_r(rsig))
                    rep = sb.tile([P, PT], BF, tag="rep", bufs=2, name="rep")
                    nc.scalar.copy(rep, ps_rep)
                    # pass 1: LN apply (DVE) for all chunks first, so the
                    # rotation matmuls never head-of-line block the in-order
                    # PE stream on a DVE dependency.
                    for ec in range(NH):
                        ch = hold[:, ec]
                        nc.vector.tensor_tensor(ch, ch, rep, op=OP.mult)
                        nc.vector.tensor_scalar(
                            ch, ch,
                            scalar1=g_sb[:, ds(ec, 1)],
                            scalar2=b_sb[:, ds(ec, 1)],
                            op0=OP.mult, op1=OP.add,
                        )
                    # pass 2: rotation matmuls stream back-to-back
                    for ec in range(NH):
                        ch = hold[:, ec]
                        ps_rot = psum.tile([P, PT], F32, tag="mm", bufs=4,
                                           name="ps_rot")
                        nc.tensor.matmul(ps_rot, lhsT=rotm, rhs=ch)
                        tmp = sb.tile([P, PT], BF, tag="tmp", bufs=2,
                                      name="rtmp")
                        nc.vector.tensor_tensor(tmp, ps_rot, sin_t[:, csl],
                                                op=OP.mult)
                        nc.vector.tensor_tensor(ch, ch, cos_t[:, csl],
                                                op=OP.mult)
                        nc.vector.tensor_tensor(ch, ch, tmp, op=OP.add)
                    nc.sync.dma_start(
                        dst[:, :, ds(tok0_dst + st * PT, PT)], hold
                    )

            def v_proj(half, xts):
                """v-projection for one x half: x chunks (from the k-proj
                xt tiles) stationary, wv moving."""
                for scg in range(KC2 // 4):
                    scs = [scg * 4 + i for i in range(4)]
                    for et in range(VET):
                        psv = {sc: psum.tile([P, VEW], F32, tag="mm", bufs=4,
                                             name="psv")
                               for sc in scs}
                        for d in range(DC):
                            wv = sb.tile([P, VEW], BF, tag="wv", bufs=3,
                                         name="wv")
                            eng = nc.scalar if d % 2 else nc.gpsimd
                            eng.dma_start(
                                wv, wvT[ds(d * P, P), ds(et * VEW, VEW)]
                            )
                            for i, sc in enumerate(scs):
                                xtile = xts[sc // 2]
                                nc.tensor.matmul(
                                    psv[sc],
                                    lhsT=xtile[:, d, ds((sc % 2) * P, P)],
                                    rhs=wv,
                                    start=(d == 0),
                                    stop=(d == DC - 1),
                                )
                        for sc in scs:
                            vsb = sb.tile([P, VEW], BF, tag="vsb", bufs=2,
                                          name="vsb")
                            nc.scalar.copy(vsb, psv[sc])
                            gsc = half * KC2 + sc
                            # batched store: 4 heads in one DMA; layout
                            # [NH, P, KC, HD] keeps the attention-side read
                            # contiguous per partition
                            dst = vs[ds(et * (VEW // HD), VEW // HD),
                                     :, gsc, :]
                            nc.sync.dma_start(
                                dst.rearrange("h p hd -> p h hd"),
                                vsb.rearrange("p (h hd) -> p h hd", hd=HD),
                            )

            # ---- Projections, interleaved so PE always has matmuls while
            # the DVE finishes the previous call's LN/rope ----
            a_xts, a_holds, a_pstats = proj_mm(xTq, 0, NQTOK // PT, 0)
            b0_xts, b0_holds, b0_pstats = proj_mm(xT, 0, S2 // PT, NH)
            v_proj(0, b0_xts)
            proj_fin(a_holds, a_pstats, NQTOK // PT, (cosq, sinq), 0,
                     gq, bq, qts, 0)
            b1_xts, b1_holds, b1_pstats = proj_mm(xT, S2, S2 // PT, NH)
            proj_fin(b0_holds, b0_pstats, S2 // PT, (cosk, sink), 0,
                     gk, bk, kts, 0)
            v_proj(1, b1_xts)
            proj_fin(b1_holds, b1_pstats, S2 // PT, (cosk, sink), S2,
                     gk, bk, kts, S2)

            # ---- Phase D+E: attention + out-projection per q tile ----
            for t in range(NQ):
                qsl_off = t * QT
                mt = sb.tile([P, MAXM, QT], BF, tag="masks", bufs=1,
                             name="mt")
                nc.sync.dma_start(mt, masks_i[t])
                mpos = {kc: i for i, kc in enumerate(masked[t])}
                n_slots = slots[t]
                n_half = (n_slots + KC2 - 1) // KC2  # kv halves needed
                ots = sb.tile([P, NH, QT], BF, tag="ots", bufs=1, name="ots")
                for h in range(NH):
                    qsl = sb.tile([P, QT], BF, tag="qslab", bufs=2,
                                  name="qsl")
                    nc.gpsimd.dma_start(qsl, qts[:, h, ds(qsl_off, QT)])
                    ksl = sb.tile([P, n_half * S2], BF, tag="kslab", bufs=2,
                                  name="ksl")
                    nc.gpsimd.dma_start(ksl, kts[:, h, ds(0, n_half * S2)])
                    vsl = sb.tile([P, n_half * KC2, HD], BF, tag="vslab",
                                  bufs=2, name="vsl")
                    nc.gpsimd.dma_start(
                        vsl, vs[h, :, ds(0, n_half * KC2), :]
                    )
                    psout = psum.tile([P, QT], F32, tag="acc", bufs=2,
                                      name="psout")
                    psden = psum.tile([1, QT], F32, tag="stat", bufs=2,
                                      name="psden")
                    # software-pipelined slot loop: QK runs PIPE slots ahead
                    # of exp/PV so the in-order PE stream never waits on the
                    # Scalar engine.
                    PIPE = 4
                    pssq = {}
                    ets = {}

                    def issue_qk(s):
                        pssq[s] = psum.tile([P, QT], F32, tag="mm", bufs=4,
                                            name="pssq")
                        nc.tensor.matmul(
                            pssq[s],
                            lhsT=ksl[:, ds(s * P, P)],
                            rhs=qsl,
                        )

                    def issue_exp(s):
                        et = sb.tile([P, QT], BF, tag="exp", bufs=PIPE + 1,
                                     name="et")
                        nc.scalar.activation(et, pssq[s], AF.Exp, bias=nege)
                        del pssq[s]
                        if s in mpos:
                            nc.vector.tensor_tensor(
                                et, et, mt[:, mpos[s]], op=OP.mult
                            )
                        ets[s] = et

                    for s in range(min(PIPE, n_slots)):
                        issue_qk(s)
                    issue_exp(0)
                    for s in range(n_slots):
                        if s + PIPE < n_slots:
                            issue_qk(s + PIPE)
                        if s + 1 < n_slots:
                            issue_exp(s + 1)
                        et = ets.pop(s)
                        nc.tensor.matmul(
                            psout,
                            lhsT=vsl[:, s],
                            rhs=et,
                            start=(s == 0),
                            stop=(s == n_slots - 1),
                        )
                        nc.tensor.matmul(
                            psden,
                            lhsT=ones_cb,
                            rhs=et,
                            start=(s == 0),
                            stop=(s == n_slots - 1),
                        )
                    rec0 = sb.tile([1, QT], F32, tag="stats_sb", bufs=4,
                                   name="rec0")
                    with nc.allow_low_precision(
                        reason="denominator reciprocal, 18 bits is plenty"
                    ):
                        nc.vector.reciprocal_approx_fast(rec0, psden)
                    rec = sb.tile([1, QT], F32, tag="stats_sb", bufs=4,
                                  name="rec")
                    nc.scalar.activation(_r(rec), rec0, AF.Copy)
                    psr = psum.tile([P, QT], F32, tag="acc", bufs=2,
                                    name="psr")
                    nc.tensor.matmul(psr, lhsT=_r(ones_row), rhs=_r(rec))
                    rsb = sb.tile([P, QT], BF, tag="rsb", bufs=2, name="rsb")
                    nc.scalar.copy(rsb, psr)
                    nc.vector.tensor_tensor(ots[:, h], psout, rsb, op=OP.mult)

                # ---- Phase E: out-projection for this q tile (from SBUF),
                # 4 psf banks per wo load ----
                EG = 4
                for eg in range(NH // EG):
                    psf = [
                        psum.tile([P, QT], F32, tag="mm", bufs=4, name="psf")
                        for _ in range(EG)
                    ]
                    for h in range(NH):
                        wo = sb.tile([P, EG * P], BF, tag="wo", bufs=3,
                                     name="wo")
                        nc.gpsimd.dma_start(
                            wo, woT[ds(h * P, P), ds(eg * EG * P, EG * P)]
                        )
                        for x in range(EG):
                            nc.tensor.matmul(
                                psf[x],
                                lhsT=wo[:, ds(x * P, P)],
                                rhs=ots[:, h],
                                start=(h == 0),
                                stop=(h == NH - 1),
                            )
                    for x in range(EG):
                        fsb = sb.tile([P, QT], F32, tag="fsb", bufs=2,
                                      name="fsb")
                        nc.vector.tensor_copy(fsb, psf[x])
                        nc.sync.dma_start(
                            out_t[ds((eg * EG + x) * P, P), ds(qsl_off, QT)],
                            fsb,
                        )

    nc.compile()
    return nc


# --------------------------------------------------------------------------
# Host-side prep and driver
# --------------------------------------------------------------------------

def _q_blocks(role, n_blocks):
    """q-block indices (each 512 tokens) for a core role."""
    if n_blocks == 4:
        return [0, 3] if role == 0 else [1, 2]
    return list(range(n_blocks))


def make_host_data(x, w_in, w_out, q_gamma, q_beta, k_gamma, k_beta, cfg,
                   n_cores=None):
    """Build per-core in_maps (list of dicts) + assembly metadata."""
    import ml_dtypes

    BF_NP = ml_dtypes.bfloat16

    D = cfg["D"]
    S = cfg["S"]
    NQTOK = cfg["NQTOK"]
    QT = cfg["QT"]
    slots = cfg["slots"]
    masked = cfg["masked"]
    NH = D // HD
    NQ = NQTOK // QT
    MAXM = max(len(m) for m in masked)
    B = x.shape[0]
    n_blocks = S // 512
    if n_cores is None:
        n_cores = B * (2048 // NQTOK) if S == 2048 else B

    w64 = np.asarray(w_in, np.float64)
    wq = w64[0:D]
    wk = w64[D:2 * D]
    wv = w64[2 * D:3 * D]
    wq_c = wq - wq.mean(axis=0, keepdims=True)
    wk_c = wk - wk.mean(axis=0, keepdims=True)
    wqkT2 = np.concatenate([wq_c.T, wk_c.T], axis=1)
    # pre-tile to [2*NH, P, DC, P]: tile ec -> [p, dc, e] with contiguous rows
    wqkT = np.ascontiguousarray(
        wqkT2.reshape(D // P, P, 2 * (D // P), P).transpose(2, 1, 0, 3)
    ).astype(BF_NP)
    wvT = np.ascontiguousarray(wv.T).astype(BF_NP)
    woT = np.ascontiguousarray(np.asarray(w_out, np.float64).T).astype(BF_NP)

    inv = 1.0 / (10000.0 ** (np.arange(0, HD, 2, dtype=np.float64) / HD))
    tpos = np.arange(S, dtype=np.float64)
    fr = np.outer(tpos, inv)
    emb = np.concatenate([fr, fr], axis=-1)  # [S, HD]
    cosT = np.cos(emb).T  # [HD, S]
    sinTn = np.sin(emb).T

    # signed rotate-half permutation, as matmul lhsT:
    # out[p] = sum_{p'} rotmT[p', p] * in[p'] = rot_half(in)[p]
    h2 = HD // 2
    rotmT = np.zeros((P, P), np.float32)
    for p in range(h2):
        rotmT[p + h2, p] = -1.0
    for p in range(h2, HD):
        rotmT[p - h2, p] = 1.0
    rotmT = rotmT.astype(BF_NP)

    scale = 1.0 / math.sqrt(HD)
    gq_a = np.ascontiguousarray(
        (np.asarray(q_gamma, np.float64) * scale).reshape(NH, P).T
    ).astype(np.float32)
    bq_a = np.ascontiguousarray(
        (np.asarray(q_beta, np.float64) * scale).reshape(NH, P).T
    ).astype(np.float32)
    gk_a = np.ascontiguousarray(
        np.asarray(k_gamma, np.float32).reshape(NH, P).T
    )
    bk_a = np.ascontiguousarray(
        np.asarray(k_beta, np.float32).reshape(NH, P).T
    )

    in_maps = []
    meta = []
    cores_per_batch = max(1, n_cores // B)
    for c in range(n_cores):
        b = c // cores_per_batch
        r = c % cores_per_batch
        blocks = _q_blocks(r if cores_per_batch > 1 else 0, n_blocks)
        blocks = blocks[: NQTOK // 512]
        qtok = np.concatenate(
            [np.arange(bk_ * 512, (bk_ + 1) * 512) for bk_ in blocks]
        )
        xb = np.asarray(x[b], np.float32)  # [S, D]
        xT = np.ascontiguousarray(xb.T).astype(BF_NP)    # [D, S]
        xTq = np.ascontiguousarray(xT[:, qtok])
        cosq = np.ascontiguousarray(cosT[:, qtok]).astype(BF_NP)
        sinq = np.ascontiguousarray(sinTn[:, qtok]).astype(BF_NP)
        cosk = np.ascontiguousarray(cosT[:, :S]).astype(BF_NP)
        sink = np.ascontiguousarray(sinTn[:, :S]).astype(BF_NP)

        masks = np.zeros([NQ, P, MAXM, QT], np.float32)
        for t in range(NQ):
            assert QT == 512
            q_start = blocks[t] * 512
            qq = np.arange(QT)
            kk = np.arange(P)
            for mi, kc in enumerate(masked[t]):
                masks[t, :, mi, :] = (
                    (kc * P + kk[:, None]) <= (q_start + qq[None, :])
                ).astype(np.float32)
        masks = masks.astype(BF_NP)

        in_maps.append(dict(
            xTq=xTq, xT=xT, wqkT=wqkT, wvT=wvT, woT=woT,
            cosq=cosq, sinqn=sinq, cosk=cosk, sinkn=sink,
            gq=gq_a, bq=bq_a, gk=gk_a, bk=bk_a, masks=masks,
            rotm=rotmT,
            onesc=np.ones((P, 1), np.float32),
            onesr=np.ones((1, P), np.float32),
        ))
        meta.append(dict(b=b, qtok=qtok))
    return in_maps, meta


_PROGRAM_CACHE = {}


def _get_program(cfg_key, cfg):
    if cfg_key not in _PROGRAM_CACHE:
        _PROGRAM_CACHE[cfg_key] = build_program(cfg)
    return _PROGRAM_CACHE[cfg_key]


def run_full(x, w_in, w_out, q_gamma, q_beta, k_gamma, k_beta,
             trace=False):
    from concourse.bass_utils import run_bass_kernel_spmd

    cfg = FULL_CFG
    B = x.shape[0]
    n_cores = 2 * B
    in_maps, meta = make_host_data(
        x, w_in, w_out, q_gamma, q_beta, k_gamma, k_beta, cfg,
        n_cores=n_cores,
    )
    nc = _get_program("full", cfg)
    res = run_bass_kernel_spmd(
        nc, in_maps, core_ids=list(range(n_cores)), trace=trace,
    )
    S, D = cfg["S"], cfg["D"]
    out = np.empty((B, S, D), np.float32)
    for c in range(n_cores):
        o = res.results[c]["out"]  # [D, NQTOK]
        out[meta[c]["b"], meta[c]["qtok"], :] = o.T
    return out, res


def kernel(x, w_in, w_out, q_gamma, q_beta, k_gamma, k_beta, n_heads=16,
           **_ignored):
    x = np.asarray(x, np.float32)
    assert int(np.asarray(n_heads)) * HD == x.shape[-1]
    out, _ = run_full(
        np.asarray(x, np.float32),
        np.asarray(w_in, np.float32),
        np.asarray(w_out, np.float32),
        np.asarray(q_gamma, np.float32),
        np.asarray(q_beta, np.float32),
        np.asarray(k_gamma, np.float32),
        np.asarray(k_beta, np.float32),
    )
    return out


# revision 17
# speedup vs baseline: 1.4247x; 1.0690x over previous
"""Trainium2 Bass kernel for a custom attention block (qkv-proj + LN(q,k) +
RoPE + causal attention + out-proj), distributed over 8 NeuronCores.

Sharding: 2 cores per batch (B=4). Core role r=c%2 takes q-token blocks
{0,3} (r=0) or {1,2} (r=1) of 512 tokens; every core computes K/V for the
full 2048-token sequence of its batch (no collectives). The compiled
program is identical on all cores; all per-core differences are input
data (sliced x^T, cos/sin tables, causal masks).

v3: all matmul operands bf16 (same PE rate as fp32r at moving>=256, half
the DMA/SBUF); q/k/v round-trip DRAM in bf16, streamed per-head with
double buffering. Projection calls are split into a matmul part and a
finish part (LN+rope) and interleaved, so the in-order PE queue always
has the next phase's matmuls while the DVE works on the previous phase's
LN/rope. The attention slot loop is software-pipelined (QK of slot s+3
issues before PV of slot s). DMA issue is spread across engines (weights
on GpSimd, attention slabs on Vector, rest on Sync) because a single
engine's dma_start rate (~2.4/us) is a serial bottleneck. V-store DMAs
are batched 4 heads per descriptor-set.

LN: mean subtraction is folded into host-pre-centered w_in rows; variance
comes from Square + ones-matmul partition reduction; rsqrt(var+eps) is
computed as Exp(-0.5*Ln(var+eps)) so all ACT functions live in one table
set (natural_log_exp_and_others).
"""

import math

import numpy as np

import concourse.bass as bass
import concourse.mybir as mybir
import concourse.tile as tile
from concourse import bacc
from concourse.bass import ds, ts

F32 = mybir.dt.float32
F32R = mybir.dt.float32r
BF = mybir.dt.bfloat16
AF = mybir.ActivationFunctionType
OP = mybir.AluOpType

P = 128
HD = 128

FULL_CFG = dict(
    D=2048,           # model dim (contraction dim for projections)
    S=2048,           # kv tokens per core (full sequence of its batch)
    NQTOK=1024,       # q tokens per core
    PT=256,           # projection s-tile width (moving dim)
    QT=512,           # attention q-tile width (moving dim)
    slots=(8, 16),    # kv 128-chunks visited per q-tile
    masked=(tuple(range(0, 8)), tuple(range(8, 16))),  # slots that get a mask
    EXP_BIAS=8.0,
    EPS=1e-5,
)


def _r(ap):
    """fp32 -> fp32r view for matmul operands."""
    return ap.bitcast(F32R)


def _bc_mid(ap2d, n):
    """[P, T] AP -> [P, n, T] with stride-0 broadcast middle axis."""
    from concourse.bass_types import AP
    a = ap2d.ap
    assert len(a) == 2
    return AP(ap2d.tensor, ap2d.offset, [a[0], [0, n], a[1]])


def _bc_last(ap2d, n):
    """[P, H] AP -> [P, H, n] with stride-0 broadcast last axis."""
    from concourse.bass_types import AP
    a = ap2d.ap
    assert len(a) == 2
    return AP(ap2d.tensor, ap2d.offset, [a[0], a[1], [0, n]])


def build_program(cfg):
    D = cfg["D"]
    S = cfg["S"]
    NQTOK = cfg["NQTOK"]
    PT = cfg["PT"]
    QT = cfg["QT"]
    slots = cfg["slots"]
    masked = cfg["masked"]
    EXP_BIAS = cfg["EXP_BIAS"]
    EPS = cfg["EPS"]

    NH = D // HD              # heads == e-chunks per q (and per k)
    DC = D // P               # contraction chunks
    NQ = NQTOK // QT          # q tiles
    S2 = S // 2               # kv half (x residency granularity)
    KC2 = S2 // P             # kv chunks per half
    MAXM = max(len(m) for m in masked)
    VEW = 512                 # v-proj e-tile width
    VET = D // VEW

    nc = bacc.Bacc("TRN2", target_bir_lowering=False, debug=False)

    # ---- I/O ----
    xTq = nc.dram_tensor("xTq", [D, NQTOK], BF, kind="ExternalInput").ap()
    xT = nc.dram_tensor("xT", [D, S], BF, kind="ExternalInput").ap()
    wqkT = nc.dram_tensor("wqkT", [2 * NH, P, DC, P], BF,
                          kind="ExternalInput").ap()
    wvT = nc.dram_tensor("wvT", [D, D], BF, kind="ExternalInput").ap()
    woT = nc.dram_tensor("woT", [D, D], BF, kind="ExternalInput").ap()
    cosq_i = nc.dram_tensor("cosq", [HD, NQTOK], BF, kind="ExternalInput").ap()
    sinq_i = nc.dram_tensor("sinqn", [HD, NQTOK], BF, kind="ExternalInput").ap()
    cosk_i = nc.dram_tensor("cosk", [HD, S], BF, kind="ExternalInput").ap()
    sink_i = nc.dram_tensor("sinkn", [HD, S], BF, kind="ExternalInput").ap()
    rotm_i = nc.dram_tensor("rotm", [P, P], BF, kind="ExternalInput").ap()
    onesc_i = nc.dram_tensor("onesc", [P, 1], F32, kind="ExternalInput").ap()
    onesr_i = nc.dram_tensor("onesr", [1, P], F32, kind="ExternalInput").ap()
    gq_i = nc.dram_tensor("gq", [P, NH], BF, kind="ExternalInput").ap()
    bq_i = nc.dram_tensor("bq", [P, NH], BF, kind="ExternalInput").ap()
    gk_i = nc.dram_tensor("gk", [P, NH], BF, kind="ExternalInput").ap()
    bk_i = nc.dram_tensor("bk", [P, NH], BF, kind="ExternalInput").ap()
    masks_i = nc.dram_tensor("masks", [NQ, P, MAXM, QT], BF,
                             kind="ExternalInput").ap()
    out_t = nc.dram_tensor("out", [D, NQTOK], F32, kind="ExternalOutput").ap()

    with tile.TileContext(nc) as tc:
        import contextlib

        ctx = contextlib.ExitStack()
        with ctx:
            sb = ctx.enter_context(tc.tile_pool(name="sb", bufs=1))
            psum = ctx.enter_context(tc.tile_pool(name="ps", bufs=1, space="PSUM"))
            dram = ctx.enter_context(tc.tile_pool(name="dram", bufs=1, space="DRAM"))

            # ---- DRAM scratch (bf16) ----
            qts = dram.tile([P, NH, NQTOK], BF, tag="qts", name="qts")
            kts = dram.tile([P, NH, S], BF, tag="kts", name="kts")
            vs = dram.tile([NH, P, S // P, HD], BF, tag="vs", name="vs")

            # ---- constants / small inputs ----
            ones_col = sb.tile([P, 1], F32, tag="ones_col", name="ones_col")
            nc.sync.dma_start(_r(ones_col), _r(onesc_i))
            ones_row = sb.tile([1, P], F32, tag="ones_row", name="ones_row")
            nc.sync.dma_start(_r(ones_row), _r(onesr_i))
            eps1 = sb.tile([1, 1], F32, tag="eps1", name="eps1")
            nc.vector.memset(eps1, EPS)
            zero1 = sb.tile([1, 1], F32, tag="zero1", name="zero1")
            nc.vector.memset(zero1, 0.0)
            nege = sb.tile([P, 1], F32, tag="nege", name="nege")
            nc.vector.memset(nege, -EXP_BIAS)
            ones_cb = sb.tile([P, 1], BF, tag="ones_cb", name="ones_cb")
            nc.vector.memset(ones_cb, 1.0)
            rotm = sb.tile([P, P], BF, tag="rotm", name="rotm")
            nc.sync.dma_start(rotm, rotm_i)
            gq = sb.tile([P, NH], BF, tag="gq", name="gq")
            nc.sync.dma_start(gq, gq_i)
            bq = sb.tile([P, NH], BF, tag="bq", name="bq")
            nc.sync.dma_start(bq, bq_i)
            gk = sb.tile([P, NH], BF, tag="gk", name="gk")
            nc.sync.dma_start(gk, gk_i)
            bk = sb.tile([P, NH], BF, tag="bk", name="bk")
            nc.sync.dma_start(bk, bk_i)
            # rope tables resident in SBUF (bf16)
            cosk = sb.tile([HD, S], BF, tag="cosk", name="cosk")
            nc.sync.dma_start(cosk, cosk_i)
            sink = sb.tile([HD, S], BF, tag="sink", name="sink")
            nc.sync.dma_start(sink, sink_i)
            cosq = sb.tile([HD, NQTOK], BF, tag="cosq", name="cosq")
            nc.sync.dma_start(cosq, cosq_i)
            sinq = sb.tile([HD, NQTOK], BF, tag="sinq", name="sinq")
            nc.sync.dma_start(sinq, sinq_i)

            def proj_mm(x_src, tok0_src, n_st, wcol_off):
                """Matmul part of a projection over n_st*PT tokens: returns
                (xts, holds, pstats) with holds filled (pre-LN, bf16) and
                pstats accumulating sum-of-squares per st slice."""
                xts = []
                for st in range(n_st):
                    xt = sb.tile([P, DC, PT], BF, tag="xt", bufs=4,
                                 name="xt")
                    for d in range(DC):
                        nc.sync.dma_start(
                            xt[:, d],
                            x_src[ds(d * P, P), ds(tok0_src + st * PT, PT)],
                        )
                    xts.append(xt)
                holds = [
                    sb.tile([P, NH, PT], BF, tag="hold", bufs=2 * n_st,
                            name="hold")
                    for _ in range(n_st)
                ]
                assert n_st % 2 == 0
                pstats = [
                    psum.tile([1, 2 * PT], F32, tag="stat", bufs=2,
                              name="pstats")
                    for _ in range(n_st // 2)
                ]
                for ec in range(NH):
                    w = sb.tile([P, DC, P], BF, tag="w", bufs=3, name="w")
                    nc.gpsimd.dma_start(w, wqkT[wcol_off + ec])
                    pss = {st: psum.tile([P, PT], F32, tag="mm", bufs=4,
                                         name="psp")
                           for st in range(n_st)}
                    for d in range(DC):
                        for st in range(n_st):
                            nc.tensor.matmul(
                                pss[st],
                                lhsT=w[:, d],
                                rhs=xts[st][:, d],
                                start=(d == 0),
                                stop=(d == DC - 1),
                            )
                    sq_all = sb.tile([P, n_st * PT], F32, tag="sq", bufs=1,
                                     name="sq_all")
                    for st in range(n_st):
                        nc.scalar.copy(holds[st][:, ec], pss[st])
                        nc.scalar.square(_r(sq_all[:, ds(st * PT, PT)]),
                                         pss[st])
                    for hs in range(n_st // 2):
                        nc.tensor.matmul(
                            pstats[hs],
                            lhsT=_r(ones_col),
                            rhs=_r(sq_all[:, ds(hs * 2 * PT, 2 * PT)]),
                            start=(ec == 0),
                            stop=(ec == NH - 1),
                        )
                return xts, holds, pstats

            def proj_fin(holds, pstats, n_st, cos_pair, cos_off, g_sb, b_sb,
                         dst, tok0_dst):
                """LN apply + rope + store for a projection's holds."""
                cos_t, sin_t = cos_pair
                for st in range(n_st):
                    hold = holds[st]
                    csl = ds(cos_off + st * PT, PT)
                    pst = pstats[st // 2][:, ds((st % 2) * PT, PT)]
                    # rsig = exp(-0.5 * ln(sumsq/D + eps))
                    lnv = sb.tile([1, PT], F32, tag="stats_sb", bufs=4,
                                  name="lnv")
                    nc.scalar.activation(lnv, pst, AF.Ln, scale=1.0 / D,
                                         bias=eps1)
                    rsig = sb.tile([1, PT], F32, tag="stats_sb", bufs=4,
                                   name="rsig")
                    nc.scalar.activation(_r(rsig), lnv, AF.Exp, bias=zero1,
                                         scale=-0.5)
                    ps_rep = psum.tile([P, PT], F32, tag="mm", bufs=4,
                                       name="ps_rep")
                    nc.tensor.matmul(ps_rep, lhsT=_r(ones_row), rhs=_r(rsig))
                    rep = sb.tile([P, PT], BF, tag="rep", bufs=2, name="rep")
                    nc.scalar.copy(rep, ps_rep)
                    # pass 1: LN apply as 3 whole-slab DVE ops (stride-0
                    # broadcast of rsig / gamma / beta across chunks)
                    nc.vector.tensor_tensor(hold, hold, _bc_mid(rep, NH),
                                            op=OP.mult)
                    nc.vector.tensor_tensor(hold, hold, _bc_last(g_sb, PT),
                                            op=OP.mult)
                    nc.vector.tensor_tensor(hold, hold, _bc_last(b_sb, PT),
                                            op=OP.add)
                    # pass 2: rotation matmuls stream back-to-back; per-chunk
                    # sin-mult (reads PSUM), then 2 whole-slab ops
                    tmp_all = sb.tile([P, NH, PT], BF, tag="tmpa", bufs=1,
                                      name="tmp_all")
                    for ec in range(NH):
                        ch = hold[:, ec]
                        ps_rot = psum.tile([P, PT], F32, tag="mm", bufs=4,
                                           name="ps_rot")
                        nc.tensor.matmul(ps_rot, lhsT=rotm, rhs=ch)
                        nc.vector.tensor_tensor(tmp_all[:, ec], ps_rot,
                                                sin_t[:, csl], op=OP.mult)
                    nc.vector.tensor_tensor(hold, hold,
                                            _bc_mid(cos_t[:, csl], NH),
                                            op=OP.mult)
                    nc.vector.tensor_tensor(hold, hold, tmp_all, op=OP.add)
                    nc.sync.dma_start(
                        dst[:, :, ds(tok0_dst + st * PT, PT)], hold
                    )

            def v_proj(half, xts):
                """v-projection for one x half: x chunks (from the k-proj
                xt tiles) stationary, wv moving."""
                for scg in range(KC2 // 4):
                    scs = [scg * 4 + i for i in range(4)]
                    for et in range(VET):
                        psv = {sc: psum.tile([P, VEW], F32, tag="mm", bufs=4,
                                             name="psv")
                               for sc in scs}
                        for d in range(DC):
                            wv = sb.tile([P, VEW], BF, tag="wv", bufs=6,
                                         name="wv")
                            eng = nc.scalar if d % 2 else nc.gpsimd
                            eng.dma_start(
                                wv, wvT[ds(d * P, P), ds(et * VEW, VEW)]
                            )
                            for i, sc in enumerate(scs):
                                xtile = xts[sc // 2]
                                nc.tensor.matmul(
                                    psv[sc],
                                    lhsT=xtile[:, d, ds((sc % 2) * P, P)],
                                    rhs=wv,
                                    start=(d == 0),
                                    stop=(d == DC - 1),
                                )
                        for sc in scs:
                            vsb = sb.tile([P, VEW], BF, tag="vsb", bufs=2,
                                          name="vsb")
                            nc.scalar.copy(vsb, psv[sc])
                            gsc = half * KC2 + sc
                            # batched store: 4 heads in one DMA; layout
                            # [NH, P, KC, HD] keeps the attention-side read
                            # contiguous per partition
                            dst = vs[ds(et * (VEW // HD), VEW // HD),
                                     :, gsc, :]
                            nc.sync.dma_start(
                                dst.rearrange("h p hd -> p h hd"),
                                vsb.rearrange("p (h hd) -> p h hd", hd=HD),
                            )

            # ---- Projections, interleaved so PE always has matmuls while
            # the DVE finishes the previous call's LN/rope ----
            a_xts, a_holds, a_pstats = proj_mm(xTq, 0, NQTOK // PT, 0)
            b0_xts, b0_holds, b0_pstats = proj_mm(xT, 0, S2 // PT, NH)
            v_proj(0, b0_xts)
            proj_fin(a_holds, a_pstats, NQTOK // PT, (cosq, sinq), 0,
                     gq, bq, qts, 0)
            b1_xts, b1_holds, b1_pstats = proj_mm(xT, S2, S2 // PT, NH)
            proj_fin(b0_holds, b0_pstats, S2 // PT, (cosk, sink), 0,
                     gk, bk, kts, 0)
            v_proj(1, b1_xts)
            proj_fin(b1_holds, b1_pstats, S2 // PT, (cosk, sink), S2,
                     gk, bk, kts, S2)

            # ---- Phase D+E: attention + out-projection per q tile ----
            for t in range(NQ):
                qsl_off = t * QT
                mt = sb.tile([P, MAXM, QT], BF, tag="masks", bufs=1,
                             name="mt")
                nc.sync.dma_start(mt, masks_i[t])
                mpos = {kc: i for i, kc in enumerate(masked[t])}
                n_slots = slots[t]
                n_half = (n_slots + KC2 - 1) // KC2  # kv halves needed
                ots = sb.tile([P, NH, QT], BF, tag="ots", bufs=1, name="ots")
                for h in range(NH):
                    qsl = sb.tile([P, QT], BF, tag="qslab", bufs=2,
                                  name="qsl")
                    nc.gpsimd.dma_start(qsl, qts[:, h, ds(qsl_off, QT)])
                    ksl = sb.tile([P, n_half * S2], BF, tag="kslab", bufs=2,
                                  name="ksl")
                    nc.gpsimd.dma_start(ksl, kts[:, h, ds(0, n_half * S2)])
                    vsl = sb.tile([P, n_half * KC2, HD], BF, tag="vslab",
                                  bufs=2, name="vsl")
                    nc.gpsimd.dma_start(
                        vsl, vs[h, :, ds(0, n_half * KC2), :]
                    )
                    psout = psum.tile([P, QT], F32, tag="acc", bufs=2,
                                      name="psout")
                    psden = psum.tile([1, QT], F32, tag="stat", bufs=2,
                                      name="psden")
                    # software-pipelined slot loop: QK runs PIPE slots ahead
                    # of exp/PV so the in-order PE stream never waits on the
                    # Scalar engine.
                    PIPE = 4
                    pssq = {}
                    ets = {}

                    def issue_qk(s):
                        pssq[s] = psum.tile([P, QT], F32, tag="mm", bufs=4,
                                            name="pssq")
                        nc.tensor.matmul(
                            pssq[s],
                            lhsT=ksl[:, ds(s * P, P)],
                            rhs=qsl,
                        )

                    def issue_exp(s):
                        et = sb.tile([P, QT], BF, tag="exp", bufs=PIPE + 2,
                                     name="et")
                        nc.scalar.activation(et, pssq[s], AF.Exp, bias=nege)
                        del pssq[s]
                        if s in mpos:
                            nc.vector.tensor_tensor(
                                et, et, mt[:, mpos[s]], op=OP.mult
                            )
                        ets[s] = et

                    for s in range(min(PIPE, n_slots)):
                        issue_qk(s)
                    issue_exp(0)
                    for s in range(n_slots):
                        if s + PIPE < n_slots:
                            issue_qk(s + PIPE)
                        if s + 1 < n_slots:
                            issue_exp(s + 1)
                        et = ets.pop(s)
                        nc.tensor.matmul(
                            psout,
                            lhsT=vsl[:, s],
                            rhs=et,
                            start=(s == 0),
                            stop=(s == n_slots - 1),
                        )
                        nc.tensor.matmul(
                            psden,
                            lhsT=ones_cb,
                            rhs=et,
                            start=(s == 0),
                            stop=(s == n_slots - 1),
                        )
                    rec0 = sb.tile([1, QT], F32, tag="stats_sb", bufs=4,
                                   name="rec0")
                    with nc.allow_low_precision(
                        reason="denominator reciprocal, 18 bits is plenty"
                    ):
                        nc.vector.reciprocal_approx_fast(rec0, psden)
                    rec = sb.tile([1, QT], F32, tag="stats_sb", bufs=4,
                                  name="rec")
                    nc.vector.tensor_copy(_r(rec), rec0)
                    psr = psum.tile([P, QT], F32, tag="acc", bufs=2,
                                    name="psr")
                    nc.tensor.matmul(psr, lhsT=_r(ones_row), rhs=_r(rec))
                    rsb = sb.tile([P, QT], BF, tag="rsb", bufs=2, name="rsb")
                    nc.vector.tensor_copy(rsb, psr)
                    nc.vector.tensor_tensor(ots[:, h], psout, rsb, op=OP.mult)

                # ---- Phase E: out-projection for this q tile (from SBUF),
                # 4 psf banks per wo load ----
                EG = 4
                for eg in range(NH // EG):
                    psf = [
                        psum.tile([P, QT], F32, tag="mm", bufs=4, name="psf")
                        for _ in range(EG)
                    ]
                    for h in range(NH):
                        wo = sb.tile([P, EG * P], BF, tag="wo", bufs=3,
                                     name="wo")
                        nc.gpsimd.dma_start(
                            wo, woT[ds(h * P, P), ds(eg * EG * P, EG * P)]
                        )
                        for x in range(EG):
                            nc.tensor.matmul(
                                psf[x],
                                lhsT=wo[:, ds(x * P, P)],
                                rhs=ots[:, h],
                                start=(h == 0),
                                stop=(h == NH - 1),
                            )
                    for x in range(EG):
                        fsb = sb.tile([P, QT], F32, tag="fsb", bufs=2,
                                      name="fsb")
                        nc.vector.tensor_copy(fsb, psf[x])
                        nc.sync.dma_start(
                            out_t[ds((eg * EG + x) * P, P), ds(qsl_off, QT)],
                            fsb,
                        )

    nc.compile()
    return nc


# --------------------------------------------------------------------------
# Host-side prep and driver
# --------------------------------------------------------------------------

def _q_blocks(role, n_blocks):
    """q-block indices (each 512 tokens) for a core role."""
    if n_blocks == 4:
        return [0, 3] if role == 0 else [1, 2]
    return list(range(n_blocks))


def make_host_data(x, w_in, w_out, q_gamma, q_beta, k_gamma, k_beta, cfg,
                   n_cores=None):
    """Build per-core in_maps (list of dicts) + assembly metadata."""
    import ml_dtypes

    BF_NP = ml_dtypes.bfloat16

    D = cfg["D"]
    S = cfg["S"]
    NQTOK = cfg["NQTOK"]
    QT = cfg["QT"]
    slots = cfg["slots"]
    masked = cfg["masked"]
    NH = D // HD
    NQ = NQTOK // QT
    MAXM = max(len(m) for m in masked)
    B = x.shape[0]
    n_blocks = S // 512
    if n_cores is None:
        n_cores = B * (2048 // NQTOK) if S == 2048 else B

    w64 = np.asarray(w_in, np.float64)
    wq = w64[0:D]
    wk = w64[D:2 * D]
    wv = w64[2 * D:3 * D]
    wq_c = wq - wq.mean(axis=0, keepdims=True)
    wk_c = wk - wk.mean(axis=0, keepdims=True)
    wqkT2 = np.concatenate([wq_c.T, wk_c.T], axis=1)
    # pre-tile to [2*NH, P, DC, P]: tile ec -> [p, dc, e] with contiguous rows
    wqkT = np.ascontiguousarray(
        wqkT2.reshape(D // P, P, 2 * (D // P), P).transpose(2, 1, 0, 3)
    ).astype(BF_NP)
    wvT = np.ascontiguousarray(wv.T).astype(BF_NP)
    woT = np.ascontiguousarray(np.asarray(w_out, np.float64).T).astype(BF_NP)

    inv = 1.0 / (10000.0 ** (np.arange(0, HD, 2, dtype=np.float64) / HD))
    tpos = np.arange(S, dtype=np.float64)
    fr = np.outer(tpos, inv)
    emb = np.concatenate([fr, fr], axis=-1)  # [S, HD]
    cosT = np.cos(emb).T  # [HD, S]
    sinTn = np.sin(emb).T

    # signed rotate-half permutation, as matmul lhsT:
    # out[p] = sum_{p'} rotmT[p', p] * in[p'] = rot_half(in)[p]
    h2 = HD // 2
    rotmT = np.zeros((P, P), np.float32)
    for p in range(h2):
        rotmT[p + h2, p] = -1.0
    for p in range(h2, HD):
        rotmT[p - h2, p] = 1.0
    rotmT = rotmT.astype(BF_NP)

    scale = 1.0 / math.sqrt(HD)
    gq_a = np.ascontiguousarray(
        (np.asarray(q_gamma, np.float64) * scale).reshape(NH, P).T
    ).astype(BF_NP)
    bq_a = np.ascontiguousarray(
        (np.asarray(q_beta, np.float64) * scale).reshape(NH, P).T
    ).astype(BF_NP)
    gk_a = np.ascontiguousarray(
        np.asarray(k_gamma, np.float64).reshape(NH, P).T
    ).astype(BF_NP)
    bk_a = np.ascontiguousarray(
        np.asarray(k_beta, np.float64).reshape(NH, P).T
    ).astype(BF_NP)

    in_maps = []
    meta = []
    cores_per_batch = max(1, n_cores // B)
    for c in range(n_cores):
        b = c // cores_per_batch
        r = c % cores_per_batch
        blocks = _q_blocks(r if cores_per_batch > 1 else 0, n_blocks)
        blocks = blocks[: NQTOK // 512]
        qtok = np.concatenate(
            [np.arange(bk_ * 512, (bk_ + 1) * 512) for bk_ in blocks]
        )
        xb = np.asarray(x[b], np.float32)  # [S, D]
        xT = np.ascontiguousarray(xb.T).astype(BF_NP)    # [D, S]
        xTq = np.ascontiguousarray(xT[:, qtok])
        cosq = np.ascontiguousarray(cosT[:, qtok]).astype(BF_NP)
        sinq = np.ascontiguousarray(sinTn[:, qtok]).astype(BF_NP)
        cosk = np.ascontiguousarray(cosT[:, :S]).astype(BF_NP)
        sink = np.ascontiguousarray(sinTn[:, :S]).astype(BF_NP)

        masks = np.zeros([NQ, P, MAXM, QT], np.float32)
        for t in range(NQ):
            assert QT == 512
            q_start = blocks[t] * 512
            qq = np.arange(QT)
            kk = np.arange(P)
            for mi, kc in enumerate(masked[t]):
                masks[t, :, mi, :] = (
                    (kc * P + kk[:, None]) <= (q_start + qq[None, :])
                ).astype(np.float32)
        masks = masks.astype(BF_NP)

        in_maps.append(dict(
            xTq=xTq, xT=xT, wqkT=wqkT, wvT=wvT, woT=woT,
            cosq=cosq, sinqn=sinq, cosk=cosk, sinkn=sink,
            gq=gq_a, bq=bq_a, gk=gk_a, bk=bk_a, masks=masks,
            rotm=rotmT,
            onesc=np.ones((P, 1), np.float32),
            onesr=np.ones((1, P), np.float32),
        ))
        meta.append(dict(b=b, qtok=qtok))
    return in_maps, meta


_PROGRAM_CACHE = {}


def _get_program(cfg_key, cfg):
    if cfg_key not in _PROGRAM_CACHE:
        _PROGRAM_CACHE[cfg_key] = build_program(cfg)
    return _PROGRAM_CACHE[cfg_key]


def run_full(x, w_in, w_out, q_gamma, q_beta, k_gamma, k_beta,
             trace=False):
    from concourse.bass_utils import run_bass_kernel_spmd

    cfg = FULL_CFG
    B = x.shape[0]
    n_cores = 2 * B
    in_maps, meta = make_host_data(
        x, w_in, w_out, q_gamma, q_beta, k_gamma, k_beta, cfg,
        n_cores=n_cores,
    )
    nc = _get_program("full", cfg)
    res = run_bass_kernel_spmd(
        nc, in_maps, core_ids=list(range(n_cores)), trace=trace,
    )
    S, D = cfg["S"], cfg["D"]
    out = np.empty((B, S, D), np.float32)
    for c in range(n_cores):
        o = res.results[c]["out"]  # [D, NQTOK]
        out[meta[c]["b"], meta[c]["qtok"], :] = o.T
    return out, res


def kernel(x, w_in, w_out, q_gamma, q_beta, k_gamma, k_beta, n_heads=16,
           **_ignored):
    x = np.asarray(x, np.float32)
    assert int(np.asarray(n_heads)) * HD == x.shape[-1]
    out, _ = run_full(
        np.asarray(x, np.float32),
        np.asarray(w_in, np.float32),
        np.asarray(w_out, np.float32),
        np.asarray(q_gamma, np.float32),
        np.asarray(q_beta, np.float32),
        np.asarray(k_gamma, np.float32),
        np.asarray(k_beta, np.float32),
    )
    return out


# revision 21
# speedup vs baseline: 1.4614x; 1.0258x over previous
"""Trainium2 Bass kernel for a custom attention block (qkv-proj + LN(q,k) +
RoPE + causal attention + out-proj), distributed over 8 NeuronCores.

Sharding: 2 cores per batch (B=4). Core role r=c%2 takes q-token blocks
{0,3} (r=0) or {1,2} (r=1) of 512 tokens; every core computes K/V for the
full 2048-token sequence of its batch (no collectives). The compiled
program is identical on all cores; all per-core differences are input
data (sliced x^T, cos/sin tables, causal masks).

v3: all matmul operands bf16 (same PE rate as fp32r at moving>=256, half
the DMA/SBUF); q/k/v round-trip DRAM in bf16, streamed per-head with
double buffering. Projection calls are split into a matmul part and a
finish part (LN+rope) and interleaved, so the in-order PE queue always
has the next phase's matmuls while the DVE works on the previous phase's
LN/rope. The attention slot loop is software-pipelined (QK of slot s+3
issues before PV of slot s). DMA issue is spread across engines (weights
on GpSimd, attention slabs on Vector, rest on Sync) because a single
engine's dma_start rate (~2.4/us) is a serial bottleneck. V-store DMAs
are batched 4 heads per descriptor-set.

LN: mean subtraction is folded into host-pre-centered w_in rows; variance
comes from Square + ones-matmul partition reduction; rsqrt(var+eps) is
computed as Exp(-0.5*Ln(var+eps)) so all ACT functions live in one table
set (natural_log_exp_and_others).
"""

import math

import numpy as np

import concourse.bass as bass
import concourse.mybir as mybir
import concourse.tile as tile
from concourse import bacc
from concourse.bass import ds, ts

F32 = mybir.dt.float32
F32R = mybir.dt.float32r
BF = mybir.dt.bfloat16
AF = mybir.ActivationFunctionType
OP = mybir.AluOpType

P = 128
HD = 128

FULL_CFG = dict(
    D=2048,           # model dim (contraction dim for projections)
    S=2048,           # kv tokens per core (full sequence of its batch)
    NQTOK=1024,       # q tokens per core
    PT=256,           # projection s-tile width (moving dim)
    QT=512,           # attention q-tile width (moving dim)
    slots=(8, 16),    # kv 128-chunks visited per q-tile
    masked=(tuple(range(0, 8)), tuple(range(8, 16))),  # slots that get a mask
    EXP_BIAS=8.0,
    EPS=1e-5,
)


def _r(ap):
    """fp32 -> fp32r view for matmul operands."""
    return ap.bitcast(F32R)


def _bc_mid(ap2d, n):
    """[P, T] AP -> [P, n, T] with stride-0 broadcast middle axis."""
    from concourse.bass_types import AP
    a = ap2d.ap
    assert len(a) == 2
    return AP(ap2d.tensor, ap2d.offset, [a[0], [0, n], a[1]])


def _bc_last(ap2d, n):
    """[P, H] AP -> [P, H, n] with stride-0 broadcast last axis."""
    from concourse.bass_types import AP
    a = ap2d.ap
    assert len(a) == 2
    return AP(ap2d.tensor, ap2d.offset, [a[0], a[1], [0, n]])


def build_program(cfg):
    D = cfg["D"]
    S = cfg["S"]
    NQTOK = cfg["NQTOK"]
    PT = cfg["PT"]
    QT = cfg["QT"]
    slots = cfg["slots"]
    masked = cfg["masked"]
    EXP_BIAS = cfg["EXP_BIAS"]
    EPS = cfg["EPS"]

    NH = D // HD              # heads == e-chunks per q (and per k)
    DC = D // P               # contraction chunks
    NQ = NQTOK // QT          # q tiles
    S2 = S // 2               # kv half (x residency granularity)
    KC2 = S2 // P             # kv chunks per half
    MAXM = max(len(m) for m in masked)
    VEW = 512                 # v-proj e-tile width
    VET = D // VEW

    nc = bacc.Bacc("TRN2", target_bir_lowering=False, debug=False)

    # ---- I/O ----
    xTq = nc.dram_tensor("xTq", [D, NQTOK], BF, kind="ExternalInput").ap()
    xT = nc.dram_tensor("xT", [D, S], BF, kind="ExternalInput").ap()
    wqkT = nc.dram_tensor("wqkT", [2 * NH, P, DC, P], BF,
                          kind="ExternalInput").ap()
    wvT = nc.dram_tensor("wvT", [D, D], BF, kind="ExternalInput").ap()
    woT = nc.dram_tensor("woT", [D, D], BF, kind="ExternalInput").ap()
    cosq_i = nc.dram_tensor("cosq", [HD, NQTOK], BF, kind="ExternalInput").ap()
    sinq_i = nc.dram_tensor("sinqn", [HD, NQTOK], BF, kind="ExternalInput").ap()
    cosk_i = nc.dram_tensor("cosk", [HD, S], BF, kind="ExternalInput").ap()
    sink_i = nc.dram_tensor("sinkn", [HD, S], BF, kind="ExternalInput").ap()
    rotm_i = nc.dram_tensor("rotm", [P, P], BF, kind="ExternalInput").ap()
    onesc_i = nc.dram_tensor("onesc", [P, 1], F32, kind="ExternalInput").ap()
    onesr_i = nc.dram_tensor("onesr", [1, P], F32, kind="ExternalInput").ap()
    gq_i = nc.dram_tensor("gq", [P, NH], BF, kind="ExternalInput").ap()
    bq_i = nc.dram_tensor("bq", [P, NH], BF, kind="ExternalInput").ap()
    gk_i = nc.dram_tensor("gk", [P, NH], BF, kind="ExternalInput").ap()
    bk_i = nc.dram_tensor("bk", [P, NH], BF, kind="ExternalInput").ap()
    masks_i = nc.dram_tensor("masks", [NQ, P, MAXM, QT], BF,
                             kind="ExternalInput").ap()
    out_t = nc.dram_tensor("out", [D, NQTOK], F32, kind="ExternalOutput").ap()

    with tile.TileContext(nc) as tc:
        import contextlib

        ctx = contextlib.ExitStack()
        with ctx:
            sb = ctx.enter_context(tc.tile_pool(name="sb", bufs=1))
            psum = ctx.enter_context(tc.tile_pool(name="ps", bufs=1, space="PSUM"))
            dram = ctx.enter_context(tc.tile_pool(name="dram", bufs=1, space="DRAM"))

            # ---- DRAM scratch (bf16) ----
            qts = dram.tile([P, NH, NQTOK], BF, tag="qts", name="qts")
            kts = dram.tile([P, NH, S], BF, tag="kts", name="kts")
            vs = dram.tile([NH, P, S // P, HD], BF, tag="vs", name="vs")

            # ---- constants / small inputs ----
            ones_col = sb.tile([P, 1], F32, tag="ones_col", name="ones_col")
            nc.sync.dma_start(_r(ones_col), _r(onesc_i))
            ones_row = sb.tile([1, P], F32, tag="ones_row", name="ones_row")
            nc.sync.dma_start(_r(ones_row), _r(onesr_i))
            eps1 = sb.tile([1, 1], F32, tag="eps1", name="eps1")
            nc.vector.memset(eps1, EPS)
            zero1 = sb.tile([1, 1], F32, tag="zero1", name="zero1")
            nc.vector.memset(zero1, 0.0)
            nege = sb.tile([P, 1], F32, tag="nege", name="nege")
            nc.vector.memset(nege, -EXP_BIAS)
            ones_cb = sb.tile([P, 1], BF, tag="ones_cb", name="ones_cb")
            nc.vector.memset(ones_cb, 1.0)
            rotm = sb.tile([P, P], BF, tag="rotm", name="rotm")
            nc.sync.dma_start(rotm, rotm_i)
            gq = sb.tile([P, NH], BF, tag="gq", name="gq")
            nc.sync.dma_start(gq, gq_i)
            bq = sb.tile([P, NH], BF, tag="bq", name="bq")
            nc.sync.dma_start(bq, bq_i)
            gk = sb.tile([P, NH], BF, tag="gk", name="gk")
            nc.sync.dma_start(gk, gk_i)
            bk = sb.tile([P, NH], BF, tag="bk", name="bk")
            nc.sync.dma_start(bk, bk_i)
            # rope tables resident in SBUF (bf16)
            cosk = sb.tile([HD, S], BF, tag="cosk", name="cosk")
            nc.sync.dma_start(cosk, cosk_i)
            sink = sb.tile([HD, S], BF, tag="sink", name="sink")
            nc.sync.dma_start(sink, sink_i)
            cosq = sb.tile([HD, NQTOK], BF, tag="cosq", name="cosq")
            nc.sync.dma_start(cosq, cosq_i)
            sinq = sb.tile([HD, NQTOK], BF, tag="sinq", name="sinq")
            nc.sync.dma_start(sinq, sinq_i)

            def proj_mm(x_src, tok0_src, n_st, wcol_off):
                """Matmul part of a projection over n_st*PT tokens: returns
                (xts, holds, pstats) with holds filled (pre-LN, bf16) and
                pstats accumulating sum-of-squares per st slice."""
                xts = []
                for st in range(n_st):
                    xt = sb.tile([P, DC, PT], BF, tag="xt", bufs=4,
                                 name="xt")
                    for d in range(DC):
                        nc.sync.dma_start(
                            xt[:, d],
                            x_src[ds(d * P, P), ds(tok0_src + st * PT, PT)],
                        )
                    xts.append(xt)
                holds = [
                    sb.tile([P, NH, PT], BF, tag="hold", bufs=2 * n_st,
                            name="hold")
                    for _ in range(n_st)
                ]
                assert n_st % 2 == 0
                pstats = [
                    psum.tile([1, 2 * PT], F32, tag="stat", bufs=2,
                              name="pstats")
                    for _ in range(n_st // 2)
                ]
                for ec in range(NH):
                    w = sb.tile([P, DC, P], BF, tag="w", bufs=3, name="w")
                    nc.gpsimd.dma_start(w, wqkT[wcol_off + ec])
                    pss = {st: psum.tile([P, PT], F32, tag="mm", bufs=4,
                                         name="psp")
                           for st in range(n_st)}
                    for d in range(DC):
                        for st in range(n_st):
                            nc.tensor.matmul(
                                pss[st],
                                lhsT=w[:, d],
                                rhs=xts[st][:, d],
                                start=(d == 0),
                                stop=(d == DC - 1),
                            )
                    sq_all = sb.tile([P, n_st * PT], F32, tag="sq", bufs=1,
                                     name="sq_all")
                    for st in range(n_st):
                        nc.scalar.copy(holds[st][:, ec], pss[st])
                        nc.scalar.square(_r(sq_all[:, ds(st * PT, PT)]),
                                         pss[st])
                    for hs in range(n_st // 2):
                        nc.tensor.matmul(
                            pstats[hs],
                            lhsT=_r(ones_col),
                            rhs=_r(sq_all[:, ds(hs * 2 * PT, 2 * PT)]),
                            start=(ec == 0),
                            stop=(ec == NH - 1),
                        )
                # rsig + its broadcast, emitted HERE so the PE ops sit right
                # after the stats matmuls (not behind a later phase)
                reps = []
                for st in range(n_st):
                    pst = pstats[st // 2][:, ds((st % 2) * PT, PT)]
                    lnv = sb.tile([1, PT], F32, tag="stats_sb", bufs=4,
                                  name="lnv")
                    nc.scalar.activation(lnv, pst, AF.Ln, scale=1.0 / D,
                                         bias=eps1)
                    rsig = sb.tile([1, PT], F32, tag="stats_sb", bufs=4,
                                   name="rsig")
                    nc.scalar.activation(_r(rsig), lnv, AF.Exp, bias=zero1,
                                         scale=-0.5)
                    ps_rep = psum.tile([P, PT], F32, tag="mm", bufs=4,
                                       name="ps_rep")
                    nc.tensor.matmul(ps_rep, lhsT=_r(ones_row), rhs=_r(rsig))
                    rep = sb.tile([P, PT], BF, tag="rep", bufs=2 * n_st,
                                  name="rep")
                    nc.scalar.copy(rep, ps_rep)
                    reps.append(rep)
                return xts, holds, pstats, reps

            def proj_fin(holds, reps, n_st, cos_pair, cos_off, g_sb, b_sb,
                         dst, tok0_dst):
                """LN apply + rope + store for a projection's holds.
                Pure DVE/Scalar except the rotation matmuls."""
                cos_t, sin_t = cos_pair
                for st in range(n_st):
                    hold = holds[st]
                    csl = ds(cos_off + st * PT, PT)
                    # LN apply as 3 whole-slab DVE ops (stride-0 broadcast
                    # of rsig / gamma / beta across chunks)
                    nc.vector.tensor_tensor(hold, hold, _bc_mid(reps[st], NH),
                                            op=OP.mult)
                    nc.vector.tensor_tensor(hold, hold, _bc_last(g_sb, PT),
                                            op=OP.mult)
                    nc.vector.tensor_tensor(hold, hold, _bc_last(b_sb, PT),
                                            op=OP.add)
                    # rotation matmuls stream back-to-back; Scalar drains the
                    # psums to bf16, then 3 whole-slab DVE ops finish rope
                    rot_all = sb.tile([P, NH, PT], BF, tag="rota", bufs=1,
                                      name="rot_all")
                    for ec in range(NH):
                        ps_rot = psum.tile([P, PT], F32, tag="mm", bufs=4,
                                           name="ps_rot")
                        nc.tensor.matmul(ps_rot, lhsT=rotm, rhs=hold[:, ec])
                        nc.scalar.copy(rot_all[:, ec], ps_rot)
                    nc.vector.tensor_tensor(rot_all, rot_all,
                                            _bc_mid(sin_t[:, csl], NH),
                                            op=OP.mult)
                    nc.vector.tensor_tensor(hold, hold,
                                            _bc_mid(cos_t[:, csl], NH),
                                            op=OP.mult)
                    nc.vector.tensor_tensor(hold, hold, rot_all, op=OP.add)
                    nc.sync.dma_start(
                        dst[:, :, ds(tok0_dst + st * PT, PT)], hold
                    )

            def v_proj(half, xts):
                """v-projection for one x half: x chunks (from the k-proj
                xt tiles) stationary, wv moving."""
                for scg in range(KC2 // 4):
                    scs = [scg * 4 + i for i in range(4)]
                    for et in range(VET):
                        psv = {sc: psum.tile([P, VEW], F32, tag="mm", bufs=4,
                                             name="psv")
                               for sc in scs}
                        for d in range(DC):
                            wv = sb.tile([P, VEW], BF, tag="wv", bufs=5,
                                         name="wv")
                            eng = nc.scalar if d % 2 else nc.gpsimd
                            eng.dma_start(
                                wv, wvT[ds(d * P, P), ds(et * VEW, VEW)]
                            )
                            for i, sc in enumerate(scs):
                                xtile = xts[sc // 2]
                                nc.tensor.matmul(
                                    psv[sc],
                                    lhsT=xtile[:, d, ds((sc % 2) * P, P)],
                                    rhs=wv,
                                    start=(d == 0),
                                    stop=(d == DC - 1),
                                )
                        for sc in scs:
                            vsb = sb.tile([P, VEW], BF, tag="vsb", bufs=2,
                                          name="vsb")
                            nc.scalar.copy(vsb, psv[sc])
                            gsc = half * KC2 + sc
                            # batched store: 4 heads in one DMA; layout
                            # [NH, P, KC, HD] keeps the attention-side read
                            # contiguous per partition
                            dst = vs[ds(et * (VEW // HD), VEW // HD),
                                     :, gsc, :]
                            nc.sync.dma_start(
                                dst.rearrange("h p hd -> p h hd"),
                                vsb.rearrange("p (h hd) -> p h hd", hd=HD),
                            )

            # ---- Projections, interleaved so PE always has matmuls while
            # the DVE finishes the previous call's LN/rope ----
            a_xts, a_holds, a_pst, a_reps = proj_mm(xTq, 0, NQTOK // PT, 0)
            b0_xts, b0_holds, b0_pst, b0_reps = proj_mm(xT, 0, S2 // PT, NH)
            v_proj(0, b0_xts)
            proj_fin(a_holds, a_reps, NQTOK // PT, (cosq, sinq), 0,
                     gq, bq, qts, 0)
            b1_xts, b1_holds, b1_pst, b1_reps = proj_mm(xT, S2, S2 // PT, NH)
            proj_fin(b0_holds, b0_reps, S2 // PT, (cosk, sink), 0,
                     gk, bk, kts, 0)
            v_proj(1, b1_xts)
            proj_fin(b1_holds, b1_reps, S2 // PT, (cosk, sink), S2,
                     gk, bk, kts, S2)

            # ---- Phase D+E: attention + out-projection per q tile ----
            for t in range(NQ):
                qsl_off = t * QT
                mt = sb.tile([P, MAXM, QT], BF, tag="masks", bufs=1,
                             name="mt")
                nc.sync.dma_start(mt, masks_i[t])
                mpos = {kc: i for i, kc in enumerate(masked[t])}
                n_slots = slots[t]
                n_half = (n_slots + KC2 - 1) // KC2  # kv halves needed
                ots = sb.tile([P, NH, QT], BF, tag="ots", bufs=1, name="ots")
                for h in range(NH):
                    qsl = sb.tile([P, QT], BF, tag="qslab", bufs=2,
                                  name="qsl")
                    nc.gpsimd.dma_start(qsl, qts[:, h, ds(qsl_off, QT)])
                    ksl = sb.tile([P, n_half * S2], BF, tag="kslab", bufs=2,
                                  name="ksl")
                    nc.gpsimd.dma_start(ksl, kts[:, h, ds(0, n_half * S2)])
                    vsl = sb.tile([P, n_half * KC2, HD], BF, tag="vslab",
                                  bufs=2, name="vsl")
                    nc.gpsimd.dma_start(
                        vsl, vs[h, :, ds(0, n_half * KC2), :]
                    )
                    psout = psum.tile([P, QT], F32, tag="acc", bufs=2,
                                      name="psout")
                    psden = psum.tile([1, QT], F32, tag="stat", bufs=2,
                                      name="psden")
                    # software-pipelined slot loop: QK runs PIPE slots ahead
                    # of exp/PV so the in-order PE stream never waits on the
                    # Scalar engine.
                    PIPE = 4
                    pssq = {}
                    ets = {}

                    def issue_qk(s):
                        pssq[s] = psum.tile([P, QT], F32, tag="mm", bufs=4,
                                            name="pssq")
                        nc.tensor.matmul(
                            pssq[s],
                            lhsT=ksl[:, ds(s * P, P)],
                            rhs=qsl,
                        )

                    def issue_exp(s):
                        et = sb.tile([P, QT], BF, tag="exp", bufs=PIPE + 1,
                                     name="et")
                        nc.scalar.activation(et, pssq[s], AF.Exp, bias=nege)
                        del pssq[s]
                        if s in mpos:
                            nc.vector.tensor_tensor(
                                et, et, mt[:, mpos[s]], op=OP.mult
                            )
                        ets[s] = et

                    for s in range(min(PIPE, n_slots)):
                        issue_qk(s)
                    issue_exp(0)
                    for s in range(n_slots):
                        if s + PIPE < n_slots:
                            issue_qk(s + PIPE)
                        if s + 1 < n_slots:
                            issue_exp(s + 1)
                        et = ets.pop(s)
                        nc.tensor.matmul(
                            psout,
                            lhsT=vsl[:, s],
                            rhs=et,
                            start=(s == 0),
                            stop=(s == n_slots - 1),
                        )
                        nc.tensor.matmul(
                            psden,
                            lhsT=ones_cb,
                            rhs=et,
                            start=(s == 0),
                            stop=(s == n_slots - 1),
                        )
                    rec0 = sb.tile([1, QT], F32, tag="stats_sb", bufs=4,
                                   name="rec0")
                    with nc.allow_low_precision(
                        reason="denominator reciprocal, 18 bits is plenty"
                    ):
                        nc.vector.reciprocal_approx_fast(rec0, psden)
                    rec = sb.tile([1, QT], F32, tag="stats_sb", bufs=4,
                                  name="rec")
                    nc.vector.tensor_copy(_r(rec), rec0)
                    psr = psum.tile([P, QT], F32, tag="acc", bufs=2,
                                    name="psr")
                    nc.tensor.matmul(psr, lhsT=_r(ones_row), rhs=_r(rec))
                    rsb = sb.tile([P, QT], BF, tag="rsb", bufs=1, name="rsb")
                    nc.vector.tensor_copy(rsb, psr)
                    nc.vector.tensor_tensor(ots[:, h], psout, rsb, op=OP.mult)

                # ---- Phase E: out-projection for this q tile (from SBUF),
                # 4 psf banks per wo load ----
                EG = 4
                for eg in range(NH // EG):
                    psf = [
                        psum.tile([P, QT], F32, tag="mm", bufs=4, name="psf")
                        for _ in range(EG)
                    ]
                    for h in range(NH):
                        wo = sb.tile([P, EG * P], BF, tag="wo", bufs=3,
                                     name="wo")
                        eng = nc.scalar if h % 2 else nc.gpsimd
                        eng.dma_start(
                            wo, woT[ds(h * P, P), ds(eg * EG * P, EG * P)]
                        )
                        for x in range(EG):
                            nc.tensor.matmul(
                                psf[x],
                                lhsT=wo[:, ds(x * P, P)],
                                rhs=ots[:, h],
                                start=(h == 0),
                                stop=(h == NH - 1),
                            )
                    for x in range(EG):
                        fsb = sb.tile([P, QT], F32, tag="fsb", bufs=2,
                                      name="fsb")
                        nc.vector.tensor_copy(fsb, psf[x])
                        nc.sync.dma_start(
                            out_t[ds((eg * EG + x) * P, P), ds(qsl_off, QT)],
                            fsb,
                        )

    nc.compile()
    return nc


# --------------------------------------------------------------------------
# Host-side prep and driver
# --------------------------------------------------------------------------

def _q_blocks(role, n_blocks):
    """q-block indices (each 512 tokens) for a core role."""
    if n_blocks == 4:
        return [0, 3] if role == 0 else [1, 2]
    return list(range(n_blocks))


def make_host_data(x, w_in, w_out, q_gamma, q_beta, k_gamma, k_beta, cfg,
                   n_cores=None):
    """Build per-core in_maps (list of dicts) + assembly metadata."""
    import ml_dtypes

    BF_NP = ml_dtypes.bfloat16

    D = cfg["D"]
    S = cfg["S"]
    NQTOK = cfg["NQTOK"]
    QT = cfg["QT"]
    slots = cfg["slots"]
    masked = cfg["masked"]
    NH = D // HD
    NQ = NQTOK // QT
    MAXM = max(len(m) for m in masked)
    B = x.shape[0]
    n_blocks = S // 512
    if n_cores is None:
        n_cores = B * (2048 // NQTOK) if S == 2048 else B

    w64 = np.asarray(w_in, np.float64)
    wq = w64[0:D]
    wk = w64[D:2 * D]
    wv = w64[2 * D:3 * D]
    wq_c = wq - wq.mean(axis=0, keepdims=True)
    wk_c = wk - wk.mean(axis=0, keepdims=True)
    wqkT2 = np.concatenate([wq_c.T, wk_c.T], axis=1)
    # pre-tile to [2*NH, P, DC, P]: tile ec -> [p, dc, e] with contiguous rows
    wqkT = np.ascontiguousarray(
        wqkT2.reshape(D // P, P, 2 * (D // P), P).transpose(2, 1, 0, 3)
    ).astype(BF_NP)
    wvT = np.ascontiguousarray(wv.T).astype(BF_NP)
    woT = np.ascontiguousarray(np.asarray(w_out, np.float64).T).astype(BF_NP)

    inv = 1.0 / (10000.0 ** (np.arange(0, HD, 2, dtype=np.float64) / HD))
    tpos = np.arange(S, dtype=np.float64)
    fr = np.outer(tpos, inv)
    emb = np.concatenate([fr, fr], axis=-1)  # [S, HD]
    cosT = np.cos(emb).T  # [HD, S]
    sinTn = np.sin(emb).T

    # signed rotate-half permutation, as matmul lhsT:
    # out[p] = sum_{p'} rotmT[p', p] * in[p'] = rot_half(in)[p]
    h2 = HD // 2
    rotmT = np.zeros((P, P), np.float32)
    for p in range(h2):
        rotmT[p + h2, p] = -1.0
    for p in range(h2, HD):
        rotmT[p - h2, p] = 1.0
    rotmT = rotmT.astype(BF_NP)

    scale = 1.0 / math.sqrt(HD)
    gq_a = np.ascontiguousarray(
        (np.asarray(q_gamma, np.float64) * scale).reshape(NH, P).T
    ).astype(BF_NP)
    bq_a = np.ascontiguousarray(
        (np.asarray(q_beta, np.float64) * scale).reshape(NH, P).T
    ).astype(BF_NP)
    gk_a = np.ascontiguousarray(
        np.asarray(k_gamma, np.float64).reshape(NH, P).T
    ).astype(BF_NP)
    bk_a = np.ascontiguousarray(
        np.asarray(k_beta, np.float64).reshape(NH, P).T
    ).astype(BF_NP)

    in_maps = []
    meta = []
    cores_per_batch = max(1, n_cores // B)
    for c in range(n_cores):
        b = c // cores_per_batch
        r = c % cores_per_batch
        blocks = _q_blocks(r if cores_per_batch > 1 else 0, n_blocks)
        blocks = blocks[: NQTOK // 512]
        qtok = np.concatenate(
            [np.arange(bk_ * 512, (bk_ + 1) * 512) for bk_ in blocks]
        )
        xb = np.asarray(x[b], np.float32)  # [S, D]
        xT = np.ascontiguousarray(xb.T).astype(BF_NP)    # [D, S]
        xTq = np.ascontiguousarray(xT[:, qtok])
        cosq = np.ascontiguousarray(cosT[:, qtok]).astype(BF_NP)
        sinq = np.ascontiguousarray(sinTn[:, qtok]).astype(BF_NP)
        cosk = np.ascontiguousarray(cosT[:, :S]).astype(BF_NP)
        sink = np.ascontiguousarray(sinTn[:, :S]).astype(BF_NP)

        masks = np.zeros([NQ, P, MAXM, QT], np.float32)
        for t in range(NQ):
            assert QT == 512
            q_start = blocks[t] * 512
            qq = np.arange(QT)
            kk = np.arange(P)
            for mi, kc in enumerate(masked[t]):
                masks[t, :, mi, :] = (
                    (kc * P + kk[:, None]) <= (q_start + qq[None, :])
                ).astype(np.float32)
        masks = masks.astype(BF_NP)

        in_maps.append(dict(
            xTq=xTq, xT=xT, wqkT=wqkT, wvT=wvT, woT=woT,
            cosq=cosq, sinqn=sinq, cosk=cosk, sinkn=sink,
            gq=gq_a, bq=bq_a, gk=gk_a, bk=bk_a, masks=masks,
            rotm=rotmT,
            onesc=np.ones((P, 1), np.float32),
            onesr=np.ones((1, P), np.float32),
        ))
        meta.append(dict(b=b, qtok=qtok))
    return in_maps, meta


_PROGRAM_CACHE = {}


def _get_program(cfg_key, cfg):
    if cfg_key not in _PROGRAM_CACHE:
        _PROGRAM_CACHE[cfg_key] = build_program(cfg)
    return _PROGRAM_CACHE[cfg_key]


def run_full(x, w_in, w_out, q_gamma, q_beta, k_gamma, k_beta,
             trace=False):
    from concourse.bass_utils import run_bass_kernel_spmd

    cfg = FULL_CFG
    B = x.shape[0]
    n_cores = 2 * B
    in_maps, meta = make_host_data(
        x, w_in, w_out, q_gamma, q_beta, k_gamma, k_beta, cfg,
        n_cores=n_cores,
    )
    nc = _get_program("full", cfg)
    res = run_bass_kernel_spmd(
        nc, in_maps, core_ids=list(range(n_cores)), trace=trace,
    )
    S, D = cfg["S"], cfg["D"]
    out = np.empty((B, S, D), np.float32)
    for c in range(n_cores):
        o = res.results[c]["out"]  # [D, NQTOK]
        out[meta[c]["b"], meta[c]["qtok"], :] = o.T
    return out, res


def kernel(x, w_in, w_out, q_gamma, q_beta, k_gamma, k_beta, n_heads=16,
           **_ignored):
    x = np.asarray(x, np.float32)
    assert int(np.asarray(n_heads)) * HD == x.shape[-1]
    out, _ = run_full(
        np.asarray(x, np.float32),
        np.asarray(w_in, np.float32),
        np.asarray(w_out, np.float32),
        np.asarray(q_gamma, np.float32),
        np.asarray(q_beta, np.float32),
        np.asarray(k_gamma, np.float32),
        np.asarray(k_beta, np.float32),
    )
    return out


# revision 22
# speedup vs baseline: 1.4740x; 1.0086x over previous
"""Trainium2 Bass kernel for a custom attention block (qkv-proj + LN(q,k) +
RoPE + causal attention + out-proj), distributed over 8 NeuronCores.

Sharding: 2 cores per batch (B=4). Core role r=c%2 takes q-token blocks
{0,3} (r=0) or {1,2} (r=1) of 512 tokens; every core computes K/V for the
full 2048-token sequence of its batch (no collectives). The compiled
program is identical on all cores; all per-core differences are input
data (sliced x^T, cos/sin tables, causal masks).

v3: all matmul operands bf16 (same PE rate as fp32r at moving>=256, half
the DMA/SBUF); q/k/v round-trip DRAM in bf16, streamed per-head with
double buffering. Projection calls are split into a matmul part and a
finish part (LN+rope) and interleaved, so the in-order PE queue always
has the next phase's matmuls while the DVE works on the previous phase's
LN/rope. The attention slot loop is software-pipelined (QK of slot s+3
issues before PV of slot s). DMA issue is spread across engines (weights
on GpSimd, attention slabs on Vector, rest on Sync) because a single
engine's dma_start rate (~2.4/us) is a serial bottleneck. V-store DMAs
are batched 4 heads per descriptor-set.

LN: mean subtraction is folded into host-pre-centered w_in rows; variance
comes from Square + ones-matmul partition reduction; rsqrt(var+eps) is
computed as Exp(-0.5*Ln(var+eps)) so all ACT functions live in one table
set (natural_log_exp_and_others).
"""

import math

import numpy as np

import concourse.bass as bass
import concourse.mybir as mybir
import concourse.tile as tile
from concourse import bacc
from concourse.bass import ds, ts

F32 = mybir.dt.float32
F32R = mybir.dt.float32r
BF = mybir.dt.bfloat16
AF = mybir.ActivationFunctionType
OP = mybir.AluOpType

P = 128
HD = 128

FULL_CFG = dict(
    D=2048,           # model dim (contraction dim for projections)
    S=2048,           # kv tokens per core (full sequence of its batch)
    NQTOK=1024,       # q tokens per core
    PT=256,           # projection s-tile width (moving dim)
    QT=512,           # attention q-tile width (moving dim)
    slots=(8, 16),    # kv 128-chunks visited per q-tile
    masked=(tuple(range(0, 8)), tuple(range(8, 16))),  # slots that get a mask
    EXP_BIAS=8.0,
    EPS=1e-5,
)


def _r(ap):
    """fp32 -> fp32r view for matmul operands."""
    return ap.bitcast(F32R)


def _bc_mid(ap2d, n):
    """[P, T] AP -> [P, n, T] with stride-0 broadcast middle axis."""
    from concourse.bass_types import AP
    a = ap2d.ap
    assert len(a) == 2
    return AP(ap2d.tensor, ap2d.offset, [a[0], [0, n], a[1]])


def _bc_last(ap2d, n):
    """[P, H] AP -> [P, H, n] with stride-0 broadcast last axis."""
    from concourse.bass_types import AP
    a = ap2d.ap
    assert len(a) == 2
    return AP(ap2d.tensor, ap2d.offset, [a[0], a[1], [0, n]])


def build_program(cfg, skip_gb=False):
    D = cfg["D"]
    S = cfg["S"]
    NQTOK = cfg["NQTOK"]
    PT = cfg["PT"]
    QT = cfg["QT"]
    slots = cfg["slots"]
    masked = cfg["masked"]
    EXP_BIAS = cfg["EXP_BIAS"]
    EPS = cfg["EPS"]

    NH = D // HD              # heads == e-chunks per q (and per k)
    DC = D // P               # contraction chunks
    NQ = NQTOK // QT          # q tiles
    S2 = S // 2               # kv half (x residency granularity)
    KC2 = S2 // P             # kv chunks per half
    MAXM = max(len(m) for m in masked)
    VEW = 512                 # v-proj e-tile width
    VET = D // VEW

    nc = bacc.Bacc("TRN2", target_bir_lowering=False, debug=False)

    # ---- I/O ----
    xTq = nc.dram_tensor("xTq", [D, NQTOK], BF, kind="ExternalInput").ap()
    xT = nc.dram_tensor("xT", [D, S], BF, kind="ExternalInput").ap()
    wqkT = nc.dram_tensor("wqkT", [2 * NH, P, DC, P], BF,
                          kind="ExternalInput").ap()
    wvT = nc.dram_tensor("wvT", [D, D], BF, kind="ExternalInput").ap()
    woT = nc.dram_tensor("woT", [D, D], BF, kind="ExternalInput").ap()
    cosq_i = nc.dram_tensor("cosq", [HD, NQTOK], BF, kind="ExternalInput").ap()
    sinq_i = nc.dram_tensor("sinqn", [HD, NQTOK], BF, kind="ExternalInput").ap()
    cosk_i = nc.dram_tensor("cosk", [HD, S], BF, kind="ExternalInput").ap()
    sink_i = nc.dram_tensor("sinkn", [HD, S], BF, kind="ExternalInput").ap()
    rotm_i = nc.dram_tensor("rotm", [P, P], BF, kind="ExternalInput").ap()
    onesc_i = nc.dram_tensor("onesc", [P, 1], F32, kind="ExternalInput").ap()
    onesr_i = nc.dram_tensor("onesr", [1, P], F32, kind="ExternalInput").ap()
    gq_i = nc.dram_tensor("gq", [P, NH], BF, kind="ExternalInput").ap()
    bq_i = nc.dram_tensor("bq", [P, NH], BF, kind="ExternalInput").ap()
    gk_i = nc.dram_tensor("gk", [P, NH], BF, kind="ExternalInput").ap()
    bk_i = nc.dram_tensor("bk", [P, NH], BF, kind="ExternalInput").ap()
    masks_i = nc.dram_tensor("masks", [NQ, P, MAXM, QT], BF,
                             kind="ExternalInput").ap()
    out_t = nc.dram_tensor("out", [D, NQTOK], F32, kind="ExternalOutput").ap()

    with tile.TileContext(nc) as tc:
        import contextlib

        ctx = contextlib.ExitStack()
        with ctx:
            sb = ctx.enter_context(tc.tile_pool(name="sb", bufs=1))
            psum = ctx.enter_context(tc.tile_pool(name="ps", bufs=1, space="PSUM"))
            dram = ctx.enter_context(tc.tile_pool(name="dram", bufs=1, space="DRAM"))

            # ---- DRAM scratch (bf16) ----
            qts = dram.tile([P, NH, NQTOK], BF, tag="qts", name="qts")
            kts = dram.tile([P, NH, S], BF, tag="kts", name="kts")
            vs = dram.tile([NH, P, S // P, HD], BF, tag="vs", name="vs")

            # ---- constants / small inputs ----
            ones_col = sb.tile([P, 1], F32, tag="ones_col", name="ones_col")
            nc.sync.dma_start(_r(ones_col), _r(onesc_i))
            ones_row = sb.tile([1, P], F32, tag="ones_row", name="ones_row")
            nc.sync.dma_start(_r(ones_row), _r(onesr_i))
            eps1 = sb.tile([1, 1], F32, tag="eps1", name="eps1")
            nc.vector.memset(eps1, EPS)
            zero1 = sb.tile([1, 1], F32, tag="zero1", name="zero1")
            nc.vector.memset(zero1, 0.0)
            nege = sb.tile([P, 1], F32, tag="nege", name="nege")
            nc.vector.memset(nege, -EXP_BIAS)
            ones_cb = sb.tile([P, 1], BF, tag="ones_cb", name="ones_cb")
            nc.vector.memset(ones_cb, 1.0)
            rotm = sb.tile([P, P], BF, tag="rotm", name="rotm")
            nc.sync.dma_start(rotm, rotm_i)
            gq = sb.tile([P, NH], BF, tag="gq", name="gq")
            nc.sync.dma_start(gq, gq_i)
            bq = sb.tile([P, NH], BF, tag="bq", name="bq")
            nc.sync.dma_start(bq, bq_i)
            gk = sb.tile([P, NH], BF, tag="gk", name="gk")
            nc.sync.dma_start(gk, gk_i)
            bk = sb.tile([P, NH], BF, tag="bk", name="bk")
            nc.sync.dma_start(bk, bk_i)
            # rope tables resident in SBUF (bf16)
            cosk = sb.tile([HD, S], BF, tag="cosk", name="cosk")
            nc.sync.dma_start(cosk, cosk_i)
            sink = sb.tile([HD, S], BF, tag="sink", name="sink")
            nc.sync.dma_start(sink, sink_i)
            cosq = sb.tile([HD, NQTOK], BF, tag="cosq", name="cosq")
            nc.sync.dma_start(cosq, cosq_i)
            sinq = sb.tile([HD, NQTOK], BF, tag="sinq", name="sinq")
            nc.sync.dma_start(sinq, sinq_i)

            def proj_mm(x_src, tok0_src, n_st, wcol_off, rep_scale=1.0):
                """Matmul part of a projection over n_st*PT tokens: returns
                (xts, holds, pstats) with holds filled (pre-LN, bf16) and
                pstats accumulating sum-of-squares per st slice."""
                xts = []
                for st in range(n_st):
                    xt = sb.tile([P, DC, PT], BF, tag="xt", bufs=4,
                                 name="xt")
                    for d in range(DC):
                        nc.sync.dma_start(
                            xt[:, d],
                            x_src[ds(d * P, P), ds(tok0_src + st * PT, PT)],
                        )
                    xts.append(xt)
                holds = [
                    sb.tile([P, NH, PT], BF, tag="hold", bufs=2 * n_st,
                            name="hold")
                    for _ in range(n_st)
                ]
                assert n_st % 2 == 0
                pstats = [
                    psum.tile([1, 2 * PT], F32, tag="stat", bufs=2,
                              name="pstats")
                    for _ in range(n_st // 2)
                ]
                for ec in range(NH):
                    w = sb.tile([P, DC, P], BF, tag="w", bufs=3, name="w")
                    nc.gpsimd.dma_start(w, wqkT[wcol_off + ec])
                    pss = {st: psum.tile([P, PT], F32, tag="mm", bufs=4,
                                         name="psp")
                           for st in range(n_st)}
                    for d in range(DC):
                        for st in range(n_st):
                            nc.tensor.matmul(
                                pss[st],
                                lhsT=w[:, d],
                                rhs=xts[st][:, d],
                                start=(d == 0),
                                stop=(d == DC - 1),
                            )
                    sq_all = sb.tile([P, n_st * PT], F32, tag="sq", bufs=1,
                                     name="sq_all")
                    for st in range(n_st):
                        nc.scalar.copy(holds[st][:, ec], pss[st])
                        nc.scalar.square(_r(sq_all[:, ds(st * PT, PT)]),
                                         pss[st])
                    for hs in range(n_st // 2):
                        nc.tensor.matmul(
                            pstats[hs],
                            lhsT=_r(ones_col),
                            rhs=_r(sq_all[:, ds(hs * 2 * PT, 2 * PT)]),
                            start=(ec == 0),
                            stop=(ec == NH - 1),
                        )
                # rsig + its broadcast, emitted HERE so the PE ops sit right
                # after the stats matmuls (not behind a later phase)
                reps = []
                for st in range(n_st):
                    pst = pstats[st // 2][:, ds((st % 2) * PT, PT)]
                    lnv = sb.tile([1, PT], F32, tag="stats_sb", bufs=4,
                                  name="lnv")
                    nc.scalar.activation(lnv, pst, AF.Ln, scale=1.0 / D,
                                         bias=eps1)
                    rsig = sb.tile([1, PT], F32, tag="stats_sb", bufs=4,
                                   name="rsig")
                    nc.scalar.activation(_r(rsig), lnv, AF.Exp, bias=zero1,
                                         scale=-0.5)
                    if rep_scale != 1.0:
                        nc.scalar.activation(_r(rsig), rsig, AF.Copy,
                                             scale=rep_scale)
                    ps_rep = psum.tile([P, PT], F32, tag="mm", bufs=4,
                                       name="ps_rep")
                    nc.tensor.matmul(ps_rep, lhsT=_r(ones_row), rhs=_r(rsig))
                    rep = sb.tile([P, PT], BF, tag="rep", bufs=2 * n_st,
                                  name="rep")
                    nc.scalar.copy(rep, ps_rep)
                    reps.append(rep)
                return xts, holds, pstats, reps

            def proj_fin(holds, reps, n_st, cos_pair, cos_off, g_sb, b_sb,
                         dst, tok0_dst):
                """LN apply + rope + store for a projection's holds.
                Pure DVE/Scalar except the rotation matmuls."""
                cos_t, sin_t = cos_pair
                for st in range(n_st):
                    hold = holds[st]
                    csl = ds(cos_off + st * PT, PT)
                    # LN apply as 3 whole-slab DVE ops (stride-0 broadcast
                    # of rsig / gamma / beta across chunks)
                    nc.vector.tensor_tensor(hold, hold, _bc_mid(reps[st], NH),
                                            op=OP.mult)
                    if not skip_gb:
                        nc.vector.tensor_tensor(hold, hold,
                                                _bc_last(g_sb, PT),
                                                op=OP.mult)
                        nc.vector.tensor_tensor(hold, hold,
                                                _bc_last(b_sb, PT),
                                                op=OP.add)
                    # rotation matmuls stream back-to-back; Scalar drains the
                    # psums to bf16, then 3 whole-slab DVE ops finish rope
                    rot_all = sb.tile([P, NH, PT], BF, tag="rota", bufs=1,
                                      name="rot_all")
                    for ec in range(NH):
                        ps_rot = psum.tile([P, PT], F32, tag="mm", bufs=4,
                                           name="ps_rot")
                        nc.tensor.matmul(ps_rot, lhsT=rotm, rhs=hold[:, ec])
                        nc.scalar.copy(rot_all[:, ec], ps_rot)
                    nc.vector.tensor_tensor(rot_all, rot_all,
                                            _bc_mid(sin_t[:, csl], NH),
                                            op=OP.mult)
                    nc.vector.tensor_tensor(hold, hold,
                                            _bc_mid(cos_t[:, csl], NH),
                                            op=OP.mult)
                    nc.vector.tensor_tensor(hold, hold, rot_all, op=OP.add)
                    nc.sync.dma_start(
                        dst[:, :, ds(tok0_dst + st * PT, PT)], hold
                    )

            def v_proj(half, xts):
                """v-projection for one x half: x chunks (from the k-proj
                xt tiles) stationary, wv moving."""
                for scg in range(KC2 // 4):
                    scs = [scg * 4 + i for i in range(4)]
                    for et in range(VET):
                        psv = {sc: psum.tile([P, VEW], F32, tag="mm", bufs=4,
                                             name="psv")
                               for sc in scs}
                        for d in range(DC):
                            wv = sb.tile([P, VEW], BF, tag="wv", bufs=5,
                                         name="wv")
                            eng = nc.scalar if d % 2 else nc.gpsimd
                            eng.dma_start(
                                wv, wvT[ds(d * P, P), ds(et * VEW, VEW)]
                            )
                            for i, sc in enumerate(scs):
                                xtile = xts[sc // 2]
                                nc.tensor.matmul(
                                    psv[sc],
                                    lhsT=xtile[:, d, ds((sc % 2) * P, P)],
                                    rhs=wv,
                                    start=(d == 0),
                                    stop=(d == DC - 1),
                                )
                        for sc in scs:
                            vsb = sb.tile([P, VEW], BF, tag="vsb", bufs=2,
                                          name="vsb")
                            nc.scalar.copy(vsb, psv[sc])
                            gsc = half * KC2 + sc
                            # batched store: 4 heads in one DMA; layout
                            # [NH, P, KC, HD] keeps the attention-side read
                            # contiguous per partition
                            dst = vs[ds(et * (VEW // HD), VEW // HD),
                                     :, gsc, :]
                            nc.sync.dma_start(
                                dst.rearrange("h p hd -> p h hd"),
                                vsb.rearrange("p (h hd) -> p h hd", hd=HD),
                            )

            # ---- Projections, interleaved so PE always has matmuls while
            # the DVE finishes the previous call's LN/rope ----
            qscale = 1.0 / math.sqrt(HD) if skip_gb else 1.0
            a_xts, a_holds, a_pst, a_reps = proj_mm(xTq, 0, NQTOK // PT, 0,
                                                    rep_scale=qscale)
            b0_xts, b0_holds, b0_pst, b0_reps = proj_mm(xT, 0, S2 // PT, NH)
            v_proj(0, b0_xts)
            proj_fin(a_holds, a_reps, NQTOK // PT, (cosq, sinq), 0,
                     gq, bq, qts, 0)
            proj_fin(b0_holds, b0_reps, S2 // PT, (cosk, sink), 0,
                     gk, bk, kts, 0)
            b1_xts, b1_holds, b1_pst, b1_reps = proj_mm(xT, S2, S2 // PT, NH)
            v_proj(1, b1_xts)
            proj_fin(b1_holds, b1_reps, S2 // PT, (cosk, sink), S2,
                     gk, bk, kts, S2)

            # ---- Phase D+E: attention + out-projection per q tile ----
            for t in range(NQ):
                qsl_off = t * QT
                mt = sb.tile([P, MAXM, QT], BF, tag="masks", bufs=1,
                             name="mt")
                nc.sync.dma_start(mt, masks_i[t])
                mpos = {kc: i for i, kc in enumerate(masked[t])}
                n_slots = slots[t]
                n_half = (n_slots + KC2 - 1) // KC2  # kv halves needed
                ots = sb.tile([P, NH, QT], BF, tag="ots", bufs=1, name="ots")
                for h in range(NH):
                    qsl = sb.tile([P, QT], BF, tag="qslab", bufs=2,
                                  name="qsl")
                    nc.gpsimd.dma_start(qsl, qts[:, h, ds(qsl_off, QT)])
                    ksl = sb.tile([P, n_half * S2], BF, tag="kslab", bufs=2,
                                  name="ksl")
                    nc.gpsimd.dma_start(ksl, kts[:, h, ds(0, n_half * S2)])
                    vsl = sb.tile([P, n_half * KC2, HD], BF, tag="vslab",
                                  bufs=2, name="vsl")
                    nc.gpsimd.dma_start(
                        vsl, vs[h, :, ds(0, n_half * KC2), :]
                    )
                    psout = psum.tile([P, QT], F32, tag="acc", bufs=2,
                                      name="psout")
                    psden = psum.tile([1, QT], F32, tag="stat", bufs=2,
                                      name="psden")
                    # software-pipelined slot loop: QK runs PIPE slots ahead
                    # of exp/PV so the in-order PE stream never waits on the
                    # Scalar engine.
                    PIPE = 4
                    pssq = {}
                    ets = {}

                    def issue_qk(s):
                        pssq[s] = psum.tile([P, QT], F32, tag="mm", bufs=4,
                                            name="pssq")
                        nc.tensor.matmul(
                            pssq[s],
                            lhsT=ksl[:, ds(s * P, P)],
                            rhs=qsl,
                        )

                    def issue_exp(s):
                        et = sb.tile([P, QT], BF, tag="exp", bufs=PIPE + 1,
                                     name="et")
                        nc.scalar.activation(et, pssq[s], AF.Exp, bias=nege)
                        del pssq[s]
                        if s in mpos:
                            nc.vector.tensor_tensor(
                                et, et, mt[:, mpos[s]], op=OP.mult
                            )
                        ets[s] = et

                    for s in range(min(PIPE, n_slots)):
                        issue_qk(s)
                    issue_exp(0)
                    for s in range(n_slots):
                        if s + PIPE < n_slots:
                            issue_qk(s + PIPE)
                        if s + 1 < n_slots:
                            issue_exp(s + 1)
                        et = ets.pop(s)
                        nc.tensor.matmul(
                            psout,
                            lhsT=vsl[:, s],
                            rhs=et,
                            start=(s == 0),
                            stop=(s == n_slots - 1),
                        )
                        nc.tensor.matmul(
                            psden,
                            lhsT=ones_cb,
                            rhs=et,
                            start=(s == 0),
                            stop=(s == n_slots - 1),
                        )
                    rec0 = sb.tile([1, QT], F32, tag="stats_sb", bufs=4,
                                   name="rec0")
                    with nc.allow_low_precision(
                        reason="denominator reciprocal, 18 bits is plenty"
                    ):
                        nc.vector.reciprocal_approx_fast(rec0, psden)
                    rec = sb.tile([1, QT], F32, tag="stats_sb", bufs=4,
                                  name="rec")
                    nc.vector.tensor_copy(_r(rec), rec0)
                    psr = psum.tile([P, QT], F32, tag="acc", bufs=2,
                                    name="psr")
                    nc.tensor.matmul(psr, lhsT=_r(ones_row), rhs=_r(rec))
                    rsb = sb.tile([P, QT], BF, tag="rsb", bufs=1, name="rsb")
                    nc.vector.tensor_copy(rsb, psr)
                    nc.vector.tensor_tensor(ots[:, h], psout, rsb, op=OP.mult)

                # ---- Phase E: out-projection for this q tile (from SBUF),
                # 4 psf banks per wo load ----
                EG = 4
                for eg in range(NH // EG):
                    psf = [
                        psum.tile([P, QT], F32, tag="mm", bufs=4, name="psf")
                        for _ in range(EG)
                    ]
                    for h in range(NH):
                        wo = sb.tile([P, EG * P], BF, tag="wo", bufs=3,
                                     name="wo")
                        eng = nc.scalar if h % 2 else nc.gpsimd
                        eng.dma_start(
                            wo, woT[ds(h * P, P), ds(eg * EG * P, EG * P)]
                        )
                        for x in range(EG):
                            nc.tensor.matmul(
                                psf[x],
                                lhsT=wo[:, ds(x * P, P)],
                                rhs=ots[:, h],
                                start=(h == 0),
                                stop=(h == NH - 1),
                            )
                    for x in range(EG):
                        fsb = sb.tile([P, QT], F32, tag="fsb", bufs=2,
                                      name="fsb")
                        nc.vector.tensor_copy(fsb, psf[x])
                        nc.sync.dma_start(
                            out_t[ds((eg * EG + x) * P, P), ds(qsl_off, QT)],
                            fsb,
                        )

    nc.compile()
    return nc


# --------------------------------------------------------------------------
# Host-side prep and driver
# --------------------------------------------------------------------------

def _q_blocks(role, n_blocks):
    """q-block indices (each 512 tokens) for a core role."""
    if n_blocks == 4:
        return [0, 3] if role == 0 else [1, 2]
    return list(range(n_blocks))


def make_host_data(x, w_in, w_out, q_gamma, q_beta, k_gamma, k_beta, cfg,
                   n_cores=None):
    """Build per-core in_maps (list of dicts) + assembly metadata."""
    import ml_dtypes

    BF_NP = ml_dtypes.bfloat16

    D = cfg["D"]
    S = cfg["S"]
    NQTOK = cfg["NQTOK"]
    QT = cfg["QT"]
    slots = cfg["slots"]
    masked = cfg["masked"]
    NH = D // HD
    NQ = NQTOK // QT
    MAXM = max(len(m) for m in masked)
    B = x.shape[0]
    n_blocks = S // 512
    if n_cores is None:
        n_cores = B * (2048 // NQTOK) if S == 2048 else B

    w64 = np.asarray(w_in, np.float64)
    wq = w64[0:D]
    wk = w64[D:2 * D]
    wv = w64[2 * D:3 * D]
    wq_c = wq - wq.mean(axis=0, keepdims=True)
    wk_c = wk - wk.mean(axis=0, keepdims=True)
    wqkT2 = np.concatenate([wq_c.T, wk_c.T], axis=1)
    # pre-tile to [2*NH, P, DC, P]: tile ec -> [p, dc, e] with contiguous rows
    wqkT = np.ascontiguousarray(
        wqkT2.reshape(D // P, P, 2 * (D // P), P).transpose(2, 1, 0, 3)
    ).astype(BF_NP)
    wvT = np.ascontiguousarray(wv.T).astype(BF_NP)
    woT = np.ascontiguousarray(np.asarray(w_out, np.float64).T).astype(BF_NP)

    inv = 1.0 / (10000.0 ** (np.arange(0, HD, 2, dtype=np.float64) / HD))
    tpos = np.arange(S, dtype=np.float64)
    fr = np.outer(tpos, inv)
    emb = np.concatenate([fr, fr], axis=-1)  # [S, HD]
    cosT = np.cos(emb).T  # [HD, S]
    sinTn = np.sin(emb).T

    # signed rotate-half permutation, as matmul lhsT:
    # out[p] = sum_{p'} rotmT[p', p] * in[p'] = rot_half(in)[p]
    h2 = HD // 2
    rotmT = np.zeros((P, P), np.float32)
    for p in range(h2):
        rotmT[p + h2, p] = -1.0
    for p in range(h2, HD):
        rotmT[p - h2, p] = 1.0
    rotmT = rotmT.astype(BF_NP)

    scale = 1.0 / math.sqrt(HD)
    gq_a = np.ascontiguousarray(
        (np.asarray(q_gamma, np.float64) * scale).reshape(NH, P).T
    ).astype(BF_NP)
    bq_a = np.ascontiguousarray(
        (np.asarray(q_beta, np.float64) * scale).reshape(NH, P).T
    ).astype(BF_NP)
    gk_a = np.ascontiguousarray(
        np.asarray(k_gamma, np.float64).reshape(NH, P).T
    ).astype(BF_NP)
    bk_a = np.ascontiguousarray(
        np.asarray(k_beta, np.float64).reshape(NH, P).T
    ).astype(BF_NP)

    in_maps = []
    meta = []
    cores_per_batch = max(1, n_cores // B)
    for c in range(n_cores):
        b = c // cores_per_batch
        r = c % cores_per_batch
        blocks = _q_blocks(r if cores_per_batch > 1 else 0, n_blocks)
        blocks = blocks[: NQTOK // 512]
        qtok = np.concatenate(
            [np.arange(bk_ * 512, (bk_ + 1) * 512) for bk_ in blocks]
        )
        xb = np.asarray(x[b], np.float32)  # [S, D]
        xT = np.ascontiguousarray(xb.T).astype(BF_NP)    # [D, S]
        xTq = np.ascontiguousarray(xT[:, qtok])
        cosq = np.ascontiguousarray(cosT[:, qtok]).astype(BF_NP)
        sinq = np.ascontiguousarray(sinTn[:, qtok]).astype(BF_NP)
        cosk = np.ascontiguousarray(cosT[:, :S]).astype(BF_NP)
        sink = np.ascontiguousarray(sinTn[:, :S]).astype(BF_NP)

        masks = np.zeros([NQ, P, MAXM, QT], np.float32)
        for t in range(NQ):
            assert QT == 512
            q_start = blocks[t] * 512
            qq = np.arange(QT)
            kk = np.arange(P)
            for mi, kc in enumerate(masked[t]):
                masks[t, :, mi, :] = (
                    (kc * P + kk[:, None]) <= (q_start + qq[None, :])
                ).astype(np.float32)
        masks = masks.astype(BF_NP)

        in_maps.append(dict(
            xTq=xTq, xT=xT, wqkT=wqkT, wvT=wvT, woT=woT,
            cosq=cosq, sinqn=sinq, cosk=cosk, sinkn=sink,
            gq=gq_a, bq=bq_a, gk=gk_a, bk=bk_a, masks=masks,
            rotm=rotmT,
            onesc=np.ones((P, 1), np.float32),
            onesr=np.ones((1, P), np.float32),
        ))
        meta.append(dict(b=b, qtok=qtok))
    return in_maps, meta


_PROGRAM_CACHE = {}


def _get_program(cfg_key, cfg, skip_gb=False):
    if cfg_key not in _PROGRAM_CACHE:
        _PROGRAM_CACHE[cfg_key] = build_program(cfg, skip_gb=skip_gb)
    return _PROGRAM_CACHE[cfg_key]


def run_full(x, w_in, w_out, q_gamma, q_beta, k_gamma, k_beta,
             trace=False):
    from concourse.bass_utils import run_bass_kernel_spmd

    cfg = FULL_CFG
    B = x.shape[0]
    n_cores = 2 * B
    in_maps, meta = make_host_data(
        x, w_in, w_out, q_gamma, q_beta, k_gamma, k_beta, cfg,
        n_cores=n_cores,
    )
    skip_gb = bool(
        np.all(np.asarray(q_gamma) == 1.0) and np.all(np.asarray(q_beta) == 0.0)
        and np.all(np.asarray(k_gamma) == 1.0)
        and np.all(np.asarray(k_beta) == 0.0)
    )
    nc = _get_program(("full", skip_gb), cfg, skip_gb)
    res = run_bass_kernel_spmd(
        nc, in_maps, core_ids=list(range(n_cores)), trace=trace,
    )
    S, D = cfg["S"], cfg["D"]
    out = np.empty((B, S, D), np.float32)
    for c in range(n_cores):
        o = res.results[c]["out"]  # [D, NQTOK]
        out[meta[c]["b"], meta[c]["qtok"], :] = o.T
    return out, res


def kernel(x, w_in, w_out, q_gamma, q_beta, k_gamma, k_beta, n_heads=16,
           **_ignored):
    x = np.asarray(x, np.float32)
    assert int(np.asarray(n_heads)) * HD == x.shape[-1]
    out, _ = run_full(
        np.asarray(x, np.float32),
        np.asarray(w_in, np.float32),
        np.asarray(w_out, np.float32),
        np.asarray(q_gamma, np.float32),
        np.asarray(q_beta, np.float32),
        np.asarray(k_gamma, np.float32),
        np.asarray(k_beta, np.float32),
    )
    return out


# revision 23
# speedup vs baseline: 1.4794x; 1.0036x over previous
"""Trainium2 Bass kernel for a custom attention block (qkv-proj + LN(q,k) +
RoPE + causal attention + out-proj), distributed over 8 NeuronCores.

Sharding: 2 cores per batch (B=4). Core role r=c%2 takes q-token blocks
{0,3} (r=0) or {1,2} (r=1) of 512 tokens; every core computes K/V for the
full 2048-token sequence of its batch (no collectives). The compiled
program is identical on all cores; all per-core differences are input
data (sliced x^T, cos/sin tables, causal masks).

v3: all matmul operands bf16 (same PE rate as fp32r at moving>=256, half
the DMA/SBUF); q/k/v round-trip DRAM in bf16, streamed per-head with
double buffering. Projection calls are split into a matmul part and a
finish part (LN+rope) and interleaved, so the in-order PE queue always
has the next phase's matmuls while the DVE works on the previous phase's
LN/rope. The attention slot loop is software-pipelined (QK of slot s+3
issues before PV of slot s). DMA issue is spread across engines (weights
on GpSimd, attention slabs on Vector, rest on Sync) because a single
engine's dma_start rate (~2.4/us) is a serial bottleneck. V-store DMAs
are batched 4 heads per descriptor-set.

LN: mean subtraction is folded into host-pre-centered w_in rows; variance
comes from Square + ones-matmul partition reduction; rsqrt(var+eps) is
computed as Exp(-0.5*Ln(var+eps)) so all ACT functions live in one table
set (natural_log_exp_and_others).
"""

import math

import numpy as np

import concourse.bass as bass
import concourse.mybir as mybir
import concourse.tile as tile
from concourse import bacc
from concourse.bass import ds, ts

F32 = mybir.dt.float32
F32R = mybir.dt.float32r
BF = mybir.dt.bfloat16
AF = mybir.ActivationFunctionType
OP = mybir.AluOpType

P = 128
HD = 128

FULL_CFG = dict(
    D=2048,           # model dim (contraction dim for projections)
    S=2048,           # kv tokens per core (full sequence of its batch)
    NQTOK=1024,       # q tokens per core
    PT=256,           # projection s-tile width (moving dim)
    QT=512,           # attention q-tile width (moving dim)
    slots=(8, 16),    # kv 128-chunks visited per q-tile
    masked=(tuple(range(0, 8)), tuple(range(8, 16))),  # slots that get a mask
    EXP_BIAS=8.0,
    EPS=1e-5,
)


def _r(ap):
    """fp32 -> fp32r view for matmul operands."""
    return ap.bitcast(F32R)


def _bc_mid(ap2d, n):
    """[P, T] AP -> [P, n, T] with stride-0 broadcast middle axis."""
    from concourse.bass_types import AP
    a = ap2d.ap
    assert len(a) == 2
    return AP(ap2d.tensor, ap2d.offset, [a[0], [0, n], a[1]])


def _bc_last(ap2d, n):
    """[P, H] AP -> [P, H, n] with stride-0 broadcast last axis."""
    from concourse.bass_types import AP
    a = ap2d.ap
    assert len(a) == 2
    return AP(ap2d.tensor, ap2d.offset, [a[0], a[1], [0, n]])


def build_program(cfg, skip_gb=False):
    D = cfg["D"]
    S = cfg["S"]
    NQTOK = cfg["NQTOK"]
    PT = cfg["PT"]
    QT = cfg["QT"]
    slots = cfg["slots"]
    masked = cfg["masked"]
    EXP_BIAS = cfg["EXP_BIAS"]
    EPS = cfg["EPS"]

    NH = D // HD              # heads == e-chunks per q (and per k)
    DC = D // P               # contraction chunks
    NQ = NQTOK // QT          # q tiles
    S2 = S // 2               # kv half (x residency granularity)
    KC2 = S2 // P             # kv chunks per half
    MAXM = max(len(m) for m in masked)
    VEW = 512                 # v-proj e-tile width
    VET = D // VEW

    nc = bacc.Bacc("TRN2", target_bir_lowering=False, debug=False)

    # ---- I/O ----
    xTq = nc.dram_tensor("xTq", [D, NQTOK], BF, kind="ExternalInput").ap()
    xT = nc.dram_tensor("xT", [D, S], BF, kind="ExternalInput").ap()
    wqkT = nc.dram_tensor("wqkT", [2 * NH, P, DC, P], BF,
                          kind="ExternalInput").ap()
    wvT = nc.dram_tensor("wvT", [D, D], BF, kind="ExternalInput").ap()
    woT = nc.dram_tensor("woT", [D, D], BF, kind="ExternalInput").ap()
    cosq_i = nc.dram_tensor("cosq", [HD, NQTOK], BF, kind="ExternalInput").ap()
    sinq_i = nc.dram_tensor("sinqn", [HD, NQTOK], BF, kind="ExternalInput").ap()
    cosk_i = nc.dram_tensor("cosk", [HD, S], BF, kind="ExternalInput").ap()
    sink_i = nc.dram_tensor("sinkn", [HD, S], BF, kind="ExternalInput").ap()
    rotm_i = nc.dram_tensor("rotm", [P, P], BF, kind="ExternalInput").ap()
    onesc_i = nc.dram_tensor("onesc", [P, 1], F32, kind="ExternalInput").ap()
    onesr_i = nc.dram_tensor("onesr", [1, P], F32, kind="ExternalInput").ap()
    gq_i = nc.dram_tensor("gq", [P, NH], BF, kind="ExternalInput").ap()
    bq_i = nc.dram_tensor("bq", [P, NH], BF, kind="ExternalInput").ap()
    gk_i = nc.dram_tensor("gk", [P, NH], BF, kind="ExternalInput").ap()
    bk_i = nc.dram_tensor("bk", [P, NH], BF, kind="ExternalInput").ap()
    masks_i = nc.dram_tensor("masks", [NQ, P, MAXM, QT], BF,
                             kind="ExternalInput").ap()
    out_t = nc.dram_tensor("out", [D, NQTOK], F32, kind="ExternalOutput").ap()

    with tile.TileContext(nc) as tc:
        import contextlib

        ctx = contextlib.ExitStack()
        with ctx:
            sb = ctx.enter_context(tc.tile_pool(name="sb", bufs=1))
            psum = ctx.enter_context(tc.tile_pool(name="ps", bufs=1, space="PSUM"))
            dram = ctx.enter_context(tc.tile_pool(name="dram", bufs=1, space="DRAM"))

            # ---- DRAM scratch (bf16) ----
            qts = dram.tile([P, NH, NQTOK], BF, tag="qts", name="qts")
            kts = dram.tile([P, NH, S], BF, tag="kts", name="kts")
            vs = dram.tile([NH, P, S // P, HD], BF, tag="vs", name="vs")

            # ---- constants / small inputs ----
            ones_col = sb.tile([P, 1], F32, tag="ones_col", name="ones_col")
            nc.sync.dma_start(_r(ones_col), _r(onesc_i))
            ones_row = sb.tile([1, P], F32, tag="ones_row", name="ones_row")
            nc.sync.dma_start(_r(ones_row), _r(onesr_i))
            eps1 = sb.tile([1, 1], F32, tag="eps1", name="eps1")
            nc.vector.memset(eps1, EPS)
            zero1 = sb.tile([1, 1], F32, tag="zero1", name="zero1")
            nc.vector.memset(zero1, 0.0)
            nege = sb.tile([P, 1], F32, tag="nege", name="nege")
            nc.vector.memset(nege, -EXP_BIAS)
            ones_cb = sb.tile([P, 1], BF, tag="ones_cb", name="ones_cb")
            nc.vector.memset(ones_cb, 1.0)
            rotm = sb.tile([P, P], BF, tag="rotm", name="rotm")
            nc.sync.dma_start(rotm, rotm_i)
            gq = sb.tile([P, NH], BF, tag="gq", name="gq")
            nc.sync.dma_start(gq, gq_i)
            bq = sb.tile([P, NH], BF, tag="bq", name="bq")
            nc.sync.dma_start(bq, bq_i)
            gk = sb.tile([P, NH], BF, tag="gk", name="gk")
            nc.sync.dma_start(gk, gk_i)
            bk = sb.tile([P, NH], BF, tag="bk", name="bk")
            nc.sync.dma_start(bk, bk_i)
            # rope tables resident in SBUF (bf16)
            cosk = sb.tile([HD, S], BF, tag="cosk", name="cosk")
            nc.scalar.dma_start(cosk, cosk_i)
            sink = sb.tile([HD, S], BF, tag="sink", name="sink")
            nc.scalar.dma_start(sink, sink_i)
            cosq = sb.tile([HD, NQTOK], BF, tag="cosq", name="cosq")
            nc.gpsimd.dma_start(cosq, cosq_i)
            sinq = sb.tile([HD, NQTOK], BF, tag="sinq", name="sinq")
            nc.gpsimd.dma_start(sinq, sinq_i)

            def proj_mm(x_src, tok0_src, n_st, wcol_off, rep_scale=1.0):
                """Matmul part of a projection over n_st*PT tokens: returns
                (xts, holds, pstats) with holds filled (pre-LN, bf16) and
                pstats accumulating sum-of-squares per st slice."""
                xts = []
                for st in range(n_st):
                    xt = sb.tile([P, DC, PT], BF, tag="xt", bufs=4,
                                 name="xt")
                    for d in range(DC):
                        nc.sync.dma_start(
                            xt[:, d],
                            x_src[ds(d * P, P), ds(tok0_src + st * PT, PT)],
                        )
                    xts.append(xt)
                holds = [
                    sb.tile([P, NH, PT], BF, tag="hold", bufs=2 * n_st,
                            name="hold")
                    for _ in range(n_st)
                ]
                assert n_st % 2 == 0
                pstats = [
                    psum.tile([1, 2 * PT], F32, tag="stat", bufs=2,
                              name="pstats")
                    for _ in range(n_st // 2)
                ]
                for ec in range(NH):
                    w = sb.tile([P, DC, P], BF, tag="w", bufs=3, name="w")
                    nc.gpsimd.dma_start(w, wqkT[wcol_off + ec])
                    pss = {st: psum.tile([P, PT], F32, tag="mm", bufs=4,
                                         name="psp")
                           for st in range(n_st)}
                    for d in range(DC):
                        for st in range(n_st):
                            nc.tensor.matmul(
                                pss[st],
                                lhsT=w[:, d],
                                rhs=xts[st][:, d],
                                start=(d == 0),
                                stop=(d == DC - 1),
                            )
                    sq_all = sb.tile([P, n_st * PT], F32, tag="sq", bufs=1,
                                     name="sq_all")
                    for st in range(n_st):
                        nc.scalar.copy(holds[st][:, ec], pss[st])
                        nc.scalar.square(_r(sq_all[:, ds(st * PT, PT)]),
                                         pss[st])
                    for hs in range(n_st // 2):
                        nc.tensor.matmul(
                            pstats[hs],
                            lhsT=_r(ones_col),
                            rhs=_r(sq_all[:, ds(hs * 2 * PT, 2 * PT)]),
                            start=(ec == 0),
                            stop=(ec == NH - 1),
                        )
                # rsig + its broadcast, emitted HERE so the PE ops sit right
                # after the stats matmuls (not behind a later phase)
                reps = []
                for st in range(n_st):
                    pst = pstats[st // 2][:, ds((st % 2) * PT, PT)]
                    lnv = sb.tile([1, PT], F32, tag="stats_sb", bufs=4,
                                  name="lnv")
                    nc.scalar.activation(lnv, pst, AF.Ln, scale=1.0 / D,
                                         bias=eps1)
                    rsig = sb.tile([1, PT], F32, tag="stats_sb", bufs=4,
                                   name="rsig")
                    nc.scalar.activation(_r(rsig), lnv, AF.Exp, bias=zero1,
                                         scale=-0.5)
                    if rep_scale != 1.0:
                        nc.scalar.activation(_r(rsig), rsig, AF.Copy,
                                             scale=rep_scale)
                    ps_rep = psum.tile([P, PT], F32, tag="mm", bufs=4,
                                       name="ps_rep")
                    nc.tensor.matmul(ps_rep, lhsT=_r(ones_row), rhs=_r(rsig))
                    rep = sb.tile([P, PT], BF, tag="rep", bufs=2 * n_st,
                                  name="rep")
                    nc.scalar.copy(rep, ps_rep)
                    reps.append(rep)
                return xts, holds, pstats, reps

            def proj_fin(holds, reps, n_st, cos_pair, cos_off, g_sb, b_sb,
                         dst, tok0_dst):
                """LN apply + rope + store for a projection's holds.
                Pure DVE/Scalar except the rotation matmuls."""
                cos_t, sin_t = cos_pair
                for st in range(n_st):
                    hold = holds[st]
                    csl = ds(cos_off + st * PT, PT)
                    # LN apply as 3 whole-slab DVE ops (stride-0 broadcast
                    # of rsig / gamma / beta across chunks)
                    nc.vector.tensor_tensor(hold, hold, _bc_mid(reps[st], NH),
                                            op=OP.mult)
                    if not skip_gb:
                        nc.vector.tensor_tensor(hold, hold,
                                                _bc_last(g_sb, PT),
                                                op=OP.mult)
                        nc.vector.tensor_tensor(hold, hold,
                                                _bc_last(b_sb, PT),
                                                op=OP.add)
                    # rotation matmuls stream back-to-back; Scalar drains the
                    # psums to bf16, then 3 whole-slab DVE ops finish rope
                    rot_all = sb.tile([P, NH, PT], BF, tag="rota", bufs=1,
                                      name="rot_all")
                    assert 2 * PT * 4 <= 2048  # one PSUM bank
                    for ec2 in range(NH // 2):
                        ps_rot = psum.tile([P, 2 * PT], F32, tag="mm", bufs=4,
                                           name="ps_rot")
                        nc.tensor.matmul(
                            ps_rot, lhsT=rotm,
                            rhs=hold[:, ds(2 * ec2, 2)].rearrange(
                                "p e t -> p (e t)"),
                        )
                        nc.scalar.copy(
                            rot_all[:, ds(2 * ec2, 2)].rearrange(
                                "p e t -> p (e t)"),
                            ps_rot,
                        )
                    nc.vector.tensor_tensor(rot_all, rot_all,
                                            _bc_mid(sin_t[:, csl], NH),
                                            op=OP.mult)
                    nc.vector.tensor_tensor(hold, hold,
                                            _bc_mid(cos_t[:, csl], NH),
                                            op=OP.mult)
                    nc.vector.tensor_tensor(hold, hold, rot_all, op=OP.add)
                    nc.sync.dma_start(
                        dst[:, :, ds(tok0_dst + st * PT, PT)], hold
                    )

            def v_proj(half, xts):
                """v-projection for one x half: x chunks (from the k-proj
                xt tiles) stationary, wv moving."""
                for scg in range(KC2 // 4):
                    scs = [scg * 4 + i for i in range(4)]
                    for et in range(VET):
                        psv = {sc: psum.tile([P, VEW], F32, tag="mm", bufs=4,
                                             name="psv")
                               for sc in scs}
                        for d in range(DC):
                            wv = sb.tile([P, VEW], BF, tag="wv", bufs=5,
                                         name="wv")
                            eng = nc.scalar if d % 2 else nc.gpsimd
                            eng.dma_start(
                                wv, wvT[ds(d * P, P), ds(et * VEW, VEW)]
                            )
                            for i, sc in enumerate(scs):
                                xtile = xts[sc // 2]
                                nc.tensor.matmul(
                                    psv[sc],
                                    lhsT=xtile[:, d, ds((sc % 2) * P, P)],
                                    rhs=wv,
                                    start=(d == 0),
                                    stop=(d == DC - 1),
                                )
                        for sc in scs:
                            vsb = sb.tile([P, VEW], BF, tag="vsb", bufs=2,
                                          name="vsb")
                            nc.scalar.copy(vsb, psv[sc])
                            gsc = half * KC2 + sc
                            # batched store: 4 heads in one DMA; layout
                            # [NH, P, KC, HD] keeps the attention-side read
                            # contiguous per partition
                            dst = vs[ds(et * (VEW // HD), VEW // HD),
                                     :, gsc, :]
                            nc.sync.dma_start(
                                dst.rearrange("h p hd -> p h hd"),
                                vsb.rearrange("p (h hd) -> p h hd", hd=HD),
                            )

            # ---- Projections, interleaved so PE always has matmuls while
            # the DVE finishes the previous call's LN/rope ----
            qscale = 1.0 / math.sqrt(HD) if skip_gb else 1.0
            a_xts, a_holds, a_pst, a_reps = proj_mm(xTq, 0, NQTOK // PT, 0,
                                                    rep_scale=qscale)
            b0_xts, b0_holds, b0_pst, b0_reps = proj_mm(xT, 0, S2 // PT, NH)
            v_proj(0, b0_xts)
            proj_fin(a_holds, a_reps, NQTOK // PT, (cosq, sinq), 0,
                     gq, bq, qts, 0)
            proj_fin(b0_holds, b0_reps, S2 // PT, (cosk, sink), 0,
                     gk, bk, kts, 0)
            b1_xts, b1_holds, b1_pst, b1_reps = proj_mm(xT, S2, S2 // PT, NH)
            v_proj(1, b1_xts)
            proj_fin(b1_holds, b1_reps, S2 // PT, (cosk, sink), S2,
                     gk, bk, kts, S2)

            # ---- Phase D+E: attention + out-projection per q tile ----
            for t in range(NQ):
                qsl_off = t * QT
                mt = sb.tile([P, MAXM, QT], BF, tag="masks", bufs=1,
                             name="mt")
                nc.sync.dma_start(mt, masks_i[t])
                mpos = {kc: i for i, kc in enumerate(masked[t])}
                n_slots = slots[t]
                n_half = (n_slots + KC2 - 1) // KC2  # kv halves needed
                ots = sb.tile([P, NH, QT], BF, tag="ots", bufs=1, name="ots")
                for h in range(NH):
                    qsl = sb.tile([P, QT], BF, tag="qslab", bufs=2,
                                  name="qsl")
                    nc.gpsimd.dma_start(qsl, qts[:, h, ds(qsl_off, QT)])
                    ksl = sb.tile([P, n_half * S2], BF, tag="kslab", bufs=2,
                                  name="ksl")
                    nc.gpsimd.dma_start(ksl, kts[:, h, ds(0, n_half * S2)])
                    vsl = sb.tile([P, n_half * KC2, HD], BF, tag="vslab",
                                  bufs=2, name="vsl")
                    nc.gpsimd.dma_start(
                        vsl, vs[h, :, ds(0, n_half * KC2), :]
                    )
                    psout = psum.tile([P, QT], F32, tag="acc", bufs=2,
                                      name="psout")
                    psden = psum.tile([1, QT], F32, tag="stat", bufs=2,
                                      name="psden")
                    # software-pipelined slot loop: QK runs PIPE slots ahead
                    # of exp/PV so the in-order PE stream never waits on the
                    # Scalar engine.
                    PIPE = 4
                    pssq = {}
                    ets = {}

                    def issue_qk(s):
                        pssq[s] = psum.tile([P, QT], F32, tag="mm", bufs=4,
                                            name="pssq")
                        nc.tensor.matmul(
                            pssq[s],
                            lhsT=ksl[:, ds(s * P, P)],
                            rhs=qsl,
                        )

                    def issue_exp(s):
                        et = sb.tile([P, QT], BF, tag="exp", bufs=PIPE + 1,
                                     name="et")
                        nc.scalar.activation(et, pssq[s], AF.Exp, bias=nege)
                        del pssq[s]
                        if s in mpos:
                            nc.vector.tensor_tensor(
                                et, et, mt[:, mpos[s]], op=OP.mult
                            )
                        ets[s] = et

                    for s in range(min(PIPE, n_slots)):
                        issue_qk(s)
                    issue_exp(0)
                    for s in range(n_slots):
                        if s + PIPE < n_slots:
                            issue_qk(s + PIPE)
                        if s + 1 < n_slots:
                            issue_exp(s + 1)
                        et = ets.pop(s)
                        nc.tensor.matmul(
                            psout,
                            lhsT=vsl[:, s],
                            rhs=et,
                            start=(s == 0),
                            stop=(s == n_slots - 1),
                        )
                        nc.tensor.matmul(
                            psden,
                            lhsT=ones_cb,
                            rhs=et,
                            start=(s == 0),
                            stop=(s == n_slots - 1),
                        )
                    rec0 = sb.tile([1, QT], F32, tag="stats_sb", bufs=4,
                                   name="rec0")
                    with nc.allow_low_precision(
                        reason="denominator reciprocal, 18 bits is plenty"
                    ):
                        nc.vector.reciprocal_approx_fast(rec0, psden)
                    rec = sb.tile([1, QT], F32, tag="stats_sb", bufs=4,
                                  name="rec")
                    nc.vector.tensor_copy(_r(rec), rec0)
                    psr = psum.tile([P, QT], F32, tag="acc", bufs=2,
                                    name="psr")
                    nc.tensor.matmul(psr, lhsT=_r(ones_row), rhs=_r(rec))
                    rsb = sb.tile([P, QT], BF, tag="rsb", bufs=1, name="rsb")
                    nc.vector.tensor_copy(rsb, psr)
                    nc.vector.tensor_tensor(ots[:, h], psout, rsb, op=OP.mult)

                # ---- Phase E: out-projection for this q tile (from SBUF),
                # 4 psf banks per wo load ----
                EG = 4
                for eg in range(NH // EG):
                    psf = [
                        psum.tile([P, QT], F32, tag="mm", bufs=4, name="psf")
                        for _ in range(EG)
                    ]
                    for h in range(NH):
                        wo = sb.tile([P, EG * P], BF, tag="wo", bufs=3,
                                     name="wo")
                        eng = nc.scalar if h % 2 else nc.gpsimd
                        eng.dma_start(
                            wo, woT[ds(h * P, P), ds(eg * EG * P, EG * P)]
                        )
                        for x in range(EG):
                            nc.tensor.matmul(
                                psf[x],
                                lhsT=wo[:, ds(x * P, P)],
                                rhs=ots[:, h],
                                start=(h == 0),
                                stop=(h == NH - 1),
                            )
                    for x in range(EG):
                        fsb = sb.tile([P, QT], F32, tag="fsb", bufs=2,
                                      name="fsb")
                        nc.vector.tensor_copy(fsb, psf[x])
                        nc.sync.dma_start(
                            out_t[ds((eg * EG + x) * P, P), ds(qsl_off, QT)],
                            fsb,
                        )

    nc.compile()
    return nc


# --------------------------------------------------------------------------
# Host-side prep and driver
# --------------------------------------------------------------------------

def _q_blocks(role, n_blocks):
    """q-block indices (each 512 tokens) for a core role."""
    if n_blocks == 4:
        return [0, 3] if role == 0 else [1, 2]
    return list(range(n_blocks))


def make_host_data(x, w_in, w_out, q_gamma, q_beta, k_gamma, k_beta, cfg,
                   n_cores=None):
    """Build per-core in_maps (list of dicts) + assembly metadata."""
    import ml_dtypes

    BF_NP = ml_dtypes.bfloat16

    D = cfg["D"]
    S = cfg["S"]
    NQTOK = cfg["NQTOK"]
    QT = cfg["QT"]
    slots = cfg["slots"]
    masked = cfg["masked"]
    NH = D // HD
    NQ = NQTOK // QT
    MAXM = max(len(m) for m in masked)
    B = x.shape[0]
    n_blocks = S // 512
    if n_cores is None:
        n_cores = B * (2048 // NQTOK) if S == 2048 else B

    w64 = np.asarray(w_in, np.float64)
    wq = w64[0:D]
    wk = w64[D:2 * D]
    wv = w64[2 * D:3 * D]
    wq_c = wq - wq.mean(axis=0, keepdims=True)
    wk_c = wk - wk.mean(axis=0, keepdims=True)
    wqkT2 = np.concatenate([wq_c.T, wk_c.T], axis=1)
    # pre-tile to [2*NH, P, DC, P]: tile ec -> [p, dc, e] with contiguous rows
    wqkT = np.ascontiguousarray(
        wqkT2.reshape(D // P, P, 2 * (D // P), P).transpose(2, 1, 0, 3)
    ).astype(BF_NP)
    wvT = np.ascontiguousarray(wv.T).astype(BF_NP)
    woT = np.ascontiguousarray(np.asarray(w_out, np.float64).T).astype(BF_NP)

    inv = 1.0 / (10000.0 ** (np.arange(0, HD, 2, dtype=np.float64) / HD))
    tpos = np.arange(S, dtype=np.float64)
    fr = np.outer(tpos, inv)
    emb = np.concatenate([fr, fr], axis=-1)  # [S, HD]
    cosT = np.cos(emb).T  # [HD, S]
    sinTn = np.sin(emb).T

    # signed rotate-half permutation, as matmul lhsT:
    # out[p] = sum_{p'} rotmT[p', p] * in[p'] = rot_half(in)[p]
    h2 = HD // 2
    rotmT = np.zeros((P, P), np.float32)
    for p in range(h2):
        rotmT[p + h2, p] = -1.0
    for p in range(h2, HD):
        rotmT[p - h2, p] = 1.0
    rotmT = rotmT.astype(BF_NP)

    scale = 1.0 / math.sqrt(HD)
    gq_a = np.ascontiguousarray(
        (np.asarray(q_gamma, np.float64) * scale).reshape(NH, P).T
    ).astype(BF_NP)
    bq_a = np.ascontiguousarray(
        (np.asarray(q_beta, np.float64) * scale).reshape(NH, P).T
    ).astype(BF_NP)
    gk_a = np.ascontiguousarray(
        np.asarray(k_gamma, np.float64).reshape(NH, P).T
    ).astype(BF_NP)
    bk_a = np.ascontiguousarray(
        np.asarray(k_beta, np.float64).reshape(NH, P).T
    ).astype(BF_NP)

    in_maps = []
    meta = []
    cores_per_batch = max(1, n_cores // B)
    for c in range(n_cores):
        b = c // cores_per_batch
        r = c % cores_per_batch
        blocks = _q_blocks(r if cores_per_batch > 1 else 0, n_blocks)
        blocks = blocks[: NQTOK // 512]
        qtok = np.concatenate(
            [np.arange(bk_ * 512, (bk_ + 1) * 512) for bk_ in blocks]
        )
        xb = np.asarray(x[b], np.float32)  # [S, D]
        xT = np.ascontiguousarray(xb.T).astype(BF_NP)    # [D, S]
        xTq = np.ascontiguousarray(xT[:, qtok])
        cosq = np.ascontiguousarray(cosT[:, qtok]).astype(BF_NP)
        sinq = np.ascontiguousarray(sinTn[:, qtok]).astype(BF_NP)
        cosk = np.ascontiguousarray(cosT[:, :S]).astype(BF_NP)
        sink = np.ascontiguousarray(sinTn[:, :S]).astype(BF_NP)

        masks = np.zeros([NQ, P, MAXM, QT], np.float32)
        for t in range(NQ):
            assert QT == 512
            q_start = blocks[t] * 512
            qq = np.arange(QT)
            kk = np.arange(P)
            for mi, kc in enumerate(masked[t]):
                masks[t, :, mi, :] = (
                    (kc * P + kk[:, None]) <= (q_start + qq[None, :])
                ).astype(np.float32)
        masks = masks.astype(BF_NP)

        in_maps.append(dict(
            xTq=xTq, xT=xT, wqkT=wqkT, wvT=wvT, woT=woT,
            cosq=cosq, sinqn=sinq, cosk=cosk, sinkn=sink,
            gq=gq_a, bq=bq_a, gk=gk_a, bk=bk_a, masks=masks,
            rotm=rotmT,
            onesc=np.ones((P, 1), np.float32),
            onesr=np.ones((1, P), np.float32),
        ))
        meta.append(dict(b=b, qtok=qtok))
    return in_maps, meta


_PROGRAM_CACHE = {}


def _get_program(cfg_key, cfg, skip_gb=False):
    if cfg_key not in _PROGRAM_CACHE:
        _PROGRAM_CACHE[cfg_key] = build_program(cfg, skip_gb=skip_gb)
    return _PROGRAM_CACHE[cfg_key]


def run_full(x, w_in, w_out, q_gamma, q_beta, k_gamma, k_beta,
             trace=False):
    from concourse.bass_utils import run_bass_kernel_spmd

    cfg = FULL_CFG
    B = x.shape[0]
    n_cores = 2 * B
    in_maps, meta = make_host_data(
        x, w_in, w_out, q_gamma, q_beta, k_gamma, k_beta, cfg,
        n_cores=n_cores,
    )
    skip_gb = bool(
        np.all(np.asarray(q_gamma) == 1.0) and np.all(np.asarray(q_beta) == 0.0)
        and np.all(np.asarray(k_gamma) == 1.0)
        and np.all(np.asarray(k_beta) == 0.0)
    )
    nc = _get_program(("full", skip_gb), cfg, skip_gb)
    res = run_bass_kernel_spmd(
        nc, in_maps, core_ids=list(range(n_cores)), trace=trace,
    )
    S, D = cfg["S"], cfg["D"]
    out = np.empty((B, S, D), np.float32)
    for c in range(n_cores):
        o = res.results[c]["out"]  # [D, NQTOK]
        out[meta[c]["b"], meta[c]["qtok"], :] = o.T
    return out, res


def kernel(x, w_in, w_out, q_gamma, q_beta, k_gamma, k_beta, n_heads=16,
           **_ignored):
    x = np.asarray(x, np.float32)
    assert int(np.asarray(n_heads)) * HD == x.shape[-1]
    out, _ = run_full(
        np.asarray(x, np.float32),
        np.asarray(w_in, np.float32),
        np.asarray(w_out, np.float32),
        np.asarray(q_gamma, np.float32),
        np.asarray(q_beta, np.float32),
        np.asarray(k_gamma, np.float32),
        np.asarray(k_beta, np.float32),
    )
    return out


# revision 24
# speedup vs baseline: 1.5887x; 1.0739x over previous
"""Trainium2 Bass kernel for a custom attention block (qkv-proj + LN(q,k) +
RoPE + causal attention + out-proj), distributed over 8 NeuronCores.

Sharding: 2 cores per batch (B=4). Core role r=c%2 takes q-token blocks
{0,3} (r=0) or {1,2} (r=1) of 512 tokens; every core computes K/V for the
full 2048-token sequence of its batch (no collectives). The compiled
program is identical on all cores; all per-core differences are input
data (sliced x^T, cos/sin tables, causal masks).

v3: all matmul operands bf16 (same PE rate as fp32r at moving>=256, half
the DMA/SBUF); q/k/v round-trip DRAM in bf16, streamed per-head with
double buffering. Projection calls are split into a matmul part and a
finish part (LN+rope) and interleaved, so the in-order PE queue always
has the next phase's matmuls while the DVE works on the previous phase's
LN/rope. The attention slot loop is software-pipelined (QK of slot s+3
issues before PV of slot s). DMA issue is spread across engines (weights
on GpSimd, attention slabs on Vector, rest on Sync) because a single
engine's dma_start rate (~2.4/us) is a serial bottleneck. V-store DMAs
are batched 4 heads per descriptor-set.

LN: mean subtraction is folded into host-pre-centered w_in rows; variance
comes from Square + ones-matmul partition reduction; rsqrt(var+eps) is
computed as Exp(-0.5*Ln(var+eps)) so all ACT functions live in one table
set (natural_log_exp_and_others).
"""

import math

import numpy as np

import concourse.bass as bass
import concourse.mybir as mybir
import concourse.tile as tile
from concourse import bacc
from concourse.bass import ds, ts

F32 = mybir.dt.float32
F32R = mybir.dt.float32r
BF = mybir.dt.bfloat16
AF = mybir.ActivationFunctionType
OP = mybir.AluOpType

P = 128
HD = 128

FULL_CFG = dict(
    D=2048,           # model dim (contraction dim for projections)
    S=2048,           # kv tokens per core (full sequence of its batch)
    NQTOK=1024,       # q tokens per core
    PT=256,           # projection s-tile width (moving dim)
    QT=512,           # attention q-tile width (moving dim)
    slots=(8, 16),    # kv 128-chunks visited per q-tile
    masked=(tuple(range(0, 8)), tuple(range(8, 16))),  # slots that get a mask
    EXP_BIAS=8.0,
    EPS=1e-5,
)


def _r(ap):
    """fp32 -> fp32r view for matmul operands."""
    return ap.bitcast(F32R)


def _bc_mid(ap2d, n):
    """[P, T] AP -> [P, n, T] with stride-0 broadcast middle axis."""
    from concourse.bass_types import AP
    a = ap2d.ap
    assert len(a) == 2
    return AP(ap2d.tensor, ap2d.offset, [a[0], [0, n], a[1]])


def _bc_last(ap2d, n):
    """[P, H] AP -> [P, H, n] with stride-0 broadcast last axis."""
    from concourse.bass_types import AP
    a = ap2d.ap
    assert len(a) == 2
    return AP(ap2d.tensor, ap2d.offset, [a[0], a[1], [0, n]])


def build_program(cfg, skip_gb=False):
    D = cfg["D"]
    S = cfg["S"]
    NQTOK = cfg["NQTOK"]
    PT = cfg["PT"]
    QT = cfg["QT"]
    slots = cfg["slots"]
    masked = cfg["masked"]
    EXP_BIAS = cfg["EXP_BIAS"]
    EPS = cfg["EPS"]

    NH = D // HD              # heads == e-chunks per q (and per k)
    DC = D // P               # contraction chunks
    NQ = NQTOK // QT          # q tiles
    S2 = S // 2               # kv half (x residency granularity)
    KC2 = S2 // P             # kv chunks per half
    MAXM = max(len(m) for m in masked)
    VEW = 512                 # v-proj e-tile width
    VET = D // VEW

    nc = bacc.Bacc("TRN2", target_bir_lowering=False, debug=False)

    # ---- I/O ----
    xTq = nc.dram_tensor("xTq", [D, NQTOK], BF, kind="ExternalInput").ap()
    xT = nc.dram_tensor("xT", [D, S], BF, kind="ExternalInput").ap()
    wqkT = nc.dram_tensor("wqkT", [2 * NH, P, DC, P], BF,
                          kind="ExternalInput").ap()
    wvT = nc.dram_tensor("wvT", [D, D], BF, kind="ExternalInput").ap()
    woT = nc.dram_tensor("woT", [D, D], BF, kind="ExternalInput").ap()
    cosq_i = nc.dram_tensor("cosq", [HD, NQTOK], BF, kind="ExternalInput").ap()
    sinq_i = nc.dram_tensor("sinqn", [HD, NQTOK], BF, kind="ExternalInput").ap()
    cosk_i = nc.dram_tensor("cosk", [HD, S], BF, kind="ExternalInput").ap()
    sink_i = nc.dram_tensor("sinkn", [HD, S], BF, kind="ExternalInput").ap()
    rotm_i = nc.dram_tensor("rotm", [P, P], BF, kind="ExternalInput").ap()
    onesc_i = nc.dram_tensor("onesc", [P, 1], F32, kind="ExternalInput").ap()
    onesr_i = nc.dram_tensor("onesr", [1, P], F32, kind="ExternalInput").ap()
    gq_i = nc.dram_tensor("gq", [P, NH], BF, kind="ExternalInput").ap()
    bq_i = nc.dram_tensor("bq", [P, NH], BF, kind="ExternalInput").ap()
    gk_i = nc.dram_tensor("gk", [P, NH], BF, kind="ExternalInput").ap()
    bk_i = nc.dram_tensor("bk", [P, NH], BF, kind="ExternalInput").ap()
    masks_i = nc.dram_tensor("masks", [NQ, P, MAXM, QT], BF,
                             kind="ExternalInput").ap()
    out_t = nc.dram_tensor("out", [D, NQTOK], F32, kind="ExternalOutput").ap()

    with tile.TileContext(nc) as tc:
        import contextlib

        ctx = contextlib.ExitStack()
        with ctx:
            sb = ctx.enter_context(tc.tile_pool(name="sb", bufs=1))
            psum = ctx.enter_context(tc.tile_pool(name="ps", bufs=1, space="PSUM"))
            dram = ctx.enter_context(tc.tile_pool(name="dram", bufs=1, space="DRAM"))

            # ---- DRAM scratch (bf16) ----
            qts = dram.tile([P, NH, NQTOK], BF, tag="qts", name="qts")
            kts = dram.tile([P, NH, S], BF, tag="kts", name="kts")
            vs = dram.tile([NH, P, S // P, HD], BF, tag="vs", name="vs")

            # ---- constants / small inputs ----
            ones_col = sb.tile([P, 1], F32, tag="ones_col", name="ones_col")
            nc.sync.dma_start(_r(ones_col), _r(onesc_i))
            ones_row = sb.tile([1, P], F32, tag="ones_row", name="ones_row")
            nc.sync.dma_start(_r(ones_row), _r(onesr_i))
            eps1 = sb.tile([1, 1], F32, tag="eps1", name="eps1")
            nc.vector.memset(eps1, EPS)
            zero1 = sb.tile([1, 1], F32, tag="zero1", name="zero1")
            nc.vector.memset(zero1, 0.0)
            nege = sb.tile([P, 1], F32, tag="nege", name="nege")
            nc.vector.memset(nege, -EXP_BIAS)
            ones_cb = sb.tile([P, 1], BF, tag="ones_cb", name="ones_cb")
            nc.vector.memset(ones_cb, 1.0)
            rotm = sb.tile([P, P], BF, tag="rotm", name="rotm")
            nc.sync.dma_start(rotm, rotm_i)
            gq = sb.tile([P, NH], BF, tag="gq", name="gq")
            nc.sync.dma_start(gq, gq_i)
            bq = sb.tile([P, NH], BF, tag="bq", name="bq")
            nc.sync.dma_start(bq, bq_i)
            gk = sb.tile([P, NH], BF, tag="gk", name="gk")
            nc.sync.dma_start(gk, gk_i)
            bk = sb.tile([P, NH], BF, tag="bk", name="bk")
            nc.sync.dma_start(bk, bk_i)
            # rope tables resident in SBUF (bf16)
            cosk = sb.tile([HD, S], BF, tag="cosk", name="cosk")
            nc.scalar.dma_start(cosk, cosk_i)
            sink = sb.tile([HD, S], BF, tag="sink", name="sink")
            nc.scalar.dma_start(sink, sink_i)
            cosq = sb.tile([HD, NQTOK], BF, tag="cosq", name="cosq")
            nc.gpsimd.dma_start(cosq, cosq_i)
            sinq = sb.tile([HD, NQTOK], BF, tag="sinq", name="sinq")
            nc.gpsimd.dma_start(sinq, sinq_i)

            def proj_mm(x_src, tok0_src, n_st, wcol_off, rep_scale=1.0):
                """Matmul part of a projection over n_st*PT tokens: returns
                (xts, holds, pstats) with holds filled (pre-LN, bf16) and
                pstats accumulating sum-of-squares per st slice."""
                xts = []
                for pr in range(n_st // 2):
                    xt = sb.tile([P, DC, 2 * PT], BF, tag="xt", bufs=2,
                                 name="xt")
                    for d in range(DC):
                        nc.sync.dma_start(
                            xt[:, d],
                            x_src[ds(d * P, P),
                                  ds(tok0_src + pr * 2 * PT, 2 * PT)],
                        )
                    xts.append(xt)
                holds = [
                    sb.tile([P, NH, PT], BF, tag="hold", bufs=2 * n_st,
                            name="hold")
                    for _ in range(n_st)
                ]
                assert n_st % 2 == 0
                pstats = [
                    psum.tile([1, 2 * PT], F32, tag="stat", bufs=2,
                              name="pstats")
                    for _ in range(n_st // 2)
                ]
                for ec in range(NH):
                    w = sb.tile([P, DC, P], BF, tag="w", bufs=3, name="w")
                    nc.gpsimd.dma_start(w, wqkT[wcol_off + ec])
                    pss = {pr: psum.tile([P, 2 * PT], F32, tag="mm", bufs=4,
                                         name="psp")
                           for pr in range(n_st // 2)}
                    for d in range(DC):
                        for pr in range(n_st // 2):
                            nc.tensor.matmul(
                                pss[pr],
                                lhsT=w[:, d],
                                rhs=xts[pr][:, d],
                                start=(d == 0),
                                stop=(d == DC - 1),
                            )
                    sq_all = sb.tile([P, n_st * PT], F32, tag="sq", bufs=1,
                                     name="sq_all")
                    for st in range(n_st):
                        nc.scalar.copy(holds[st][:, ec],
                                       pss[st // 2][:, ds((st % 2) * PT, PT)])
                    for pr in range(n_st // 2):
                        nc.scalar.square(
                            _r(sq_all[:, ds(pr * 2 * PT, 2 * PT)]), pss[pr])
                        nc.tensor.matmul(
                            pstats[pr],
                            lhsT=_r(ones_col),
                            rhs=_r(sq_all[:, ds(pr * 2 * PT, 2 * PT)]),
                            start=(ec == 0),
                            stop=(ec == NH - 1),
                        )
                # rsig + its broadcast, emitted HERE so the PE ops sit right
                # after the stats matmuls (not behind a later phase)
                reps = []
                for st in range(n_st):
                    pst = pstats[st // 2][:, ds((st % 2) * PT, PT)]
                    lnv = sb.tile([1, PT], F32, tag="stats_sb", bufs=4,
                                  name="lnv")
                    nc.scalar.activation(lnv, pst, AF.Ln, scale=1.0 / D,
                                         bias=eps1)
                    rsig = sb.tile([1, PT], F32, tag="stats_sb", bufs=4,
                                   name="rsig")
                    nc.scalar.activation(_r(rsig), lnv, AF.Exp, bias=zero1,
                                         scale=-0.5)
                    if rep_scale != 1.0:
                        nc.scalar.activation(_r(rsig), rsig, AF.Copy,
                                             scale=rep_scale)
                    ps_rep = psum.tile([P, PT], F32, tag="mm", bufs=4,
                                       name="ps_rep")
                    nc.tensor.matmul(ps_rep, lhsT=_r(ones_row), rhs=_r(rsig))
                    rep = sb.tile([P, PT], BF, tag="rep", bufs=2 * n_st,
                                  name="rep")
                    nc.scalar.copy(rep, ps_rep)
                    reps.append(rep)
                return xts, holds, pstats, reps

            def proj_fin(holds, reps, n_st, cos_pair, cos_off, g_sb, b_sb,
                         dst, tok0_dst):
                """LN apply + rope + store for a projection's holds.
                Pure DVE/Scalar except the rotation matmuls."""
                cos_t, sin_t = cos_pair
                for st in range(n_st):
                    hold = holds[st]
                    csl = ds(cos_off + st * PT, PT)
                    # LN apply as 3 whole-slab DVE ops (stride-0 broadcast
                    # of rsig / gamma / beta across chunks)
                    nc.vector.tensor_tensor(hold, hold, _bc_mid(reps[st], NH),
                                            op=OP.mult)
                    if not skip_gb:
                        nc.vector.tensor_tensor(hold, hold,
                                                _bc_last(g_sb, PT),
                                                op=OP.mult)
                        nc.vector.tensor_tensor(hold, hold,
                                                _bc_last(b_sb, PT),
                                                op=OP.add)
                    # rotation matmuls stream back-to-back; Scalar drains the
                    # psums to bf16, then 3 whole-slab DVE ops finish rope
                    rot_all = sb.tile([P, NH, PT], BF, tag="rota", bufs=1,
                                      name="rot_all")
                    assert 2 * PT * 4 <= 2048  # one PSUM bank
                    for ec2 in range(NH // 2):
                        ps_rot = psum.tile([P, 2 * PT], F32, tag="mm", bufs=4,
                                           name="ps_rot")
                        nc.tensor.matmul(
                            ps_rot, lhsT=rotm,
                            rhs=hold[:, ds(2 * ec2, 2)].rearrange(
                                "p e t -> p (e t)"),
                        )
                        nc.scalar.copy(
                            rot_all[:, ds(2 * ec2, 2)].rearrange(
                                "p e t -> p (e t)"),
                            ps_rot,
                        )
                    nc.vector.tensor_tensor(rot_all, rot_all,
                                            _bc_mid(sin_t[:, csl], NH),
                                            op=OP.mult)
                    nc.vector.tensor_tensor(hold, hold,
                                            _bc_mid(cos_t[:, csl], NH),
                                            op=OP.mult)
                    nc.vector.tensor_tensor(hold, hold, rot_all, op=OP.add)
                    nc.sync.dma_start(
                        dst[:, :, ds(tok0_dst + st * PT, PT)], hold
                    )

            def v_proj(half, xts):
                """v-projection for one x half: x chunks (from the k-proj
                xt tiles) stationary, wv moving."""
                for scg in range(KC2 // 4):
                    scs = [scg * 4 + i for i in range(4)]
                    for et in range(VET):
                        psv = {sc: psum.tile([P, VEW], F32, tag="mm", bufs=4,
                                             name="psv")
                               for sc in scs}
                        for d in range(DC):
                            wv = sb.tile([P, VEW], BF, tag="wv", bufs=5,
                                         name="wv")
                            eng = nc.scalar if d % 2 else nc.gpsimd
                            eng.dma_start(
                                wv, wvT[ds(d * P, P), ds(et * VEW, VEW)]
                            )
                            for i, sc in enumerate(scs):
                                xtile = xts[sc // 4]
                                nc.tensor.matmul(
                                    psv[sc],
                                    lhsT=xtile[:, d, ds((sc % 4) * P, P)],
                                    rhs=wv,
                                    start=(d == 0),
                                    stop=(d == DC - 1),
                                )
                        for sc in scs:
                            vsb = sb.tile([P, VEW], BF, tag="vsb", bufs=2,
                                          name="vsb")
                            nc.scalar.copy(vsb, psv[sc])
                            gsc = half * KC2 + sc
                            # batched store: 4 heads in one DMA; layout
                            # [NH, P, KC, HD] keeps the attention-side read
                            # contiguous per partition
                            dst = vs[ds(et * (VEW // HD), VEW // HD),
                                     :, gsc, :]
                            nc.sync.dma_start(
                                dst.rearrange("h p hd -> p h hd"),
                                vsb.rearrange("p (h hd) -> p h hd", hd=HD),
                            )

            # ---- Projections, interleaved so PE always has matmuls while
            # the DVE finishes the previous call's LN/rope ----
            qscale = 1.0 / math.sqrt(HD) if skip_gb else 1.0
            a_xts, a_holds, a_pst, a_reps = proj_mm(xTq, 0, NQTOK // PT, 0,
                                                    rep_scale=qscale)
            b0_xts, b0_holds, b0_pst, b0_reps = proj_mm(xT, 0, S2 // PT, NH)
            v_proj(0, b0_xts)
            proj_fin(a_holds, a_reps, NQTOK // PT, (cosq, sinq), 0,
                     gq, bq, qts, 0)
            proj_fin(b0_holds, b0_reps, S2 // PT, (cosk, sink), 0,
                     gk, bk, kts, 0)
            b1_xts, b1_holds, b1_pst, b1_reps = proj_mm(xT, S2, S2 // PT, NH)
            v_proj(1, b1_xts)
            proj_fin(b1_holds, b1_reps, S2 // PT, (cosk, sink), S2,
                     gk, bk, kts, S2)

            # ---- Phase D+E: attention + out-projection per q tile ----
            for t in range(NQ):
                qsl_off = t * QT
                mt = sb.tile([P, MAXM, QT], BF, tag="masks", bufs=1,
                             name="mt")
                nc.sync.dma_start(mt, masks_i[t])
                mpos = {kc: i for i, kc in enumerate(masked[t])}
                n_slots = slots[t]
                n_half = (n_slots + KC2 - 1) // KC2  # kv halves needed
                ots = sb.tile([P, NH, QT], BF, tag="ots", bufs=1, name="ots")
                for h in range(NH):
                    qsl = sb.tile([P, QT], BF, tag="qslab", bufs=2,
                                  name="qsl")
                    nc.gpsimd.dma_start(qsl, qts[:, h, ds(qsl_off, QT)])
                    ksl = sb.tile([P, n_half * S2], BF, tag="kslab", bufs=2,
                                  name="ksl")
                    nc.gpsimd.dma_start(ksl, kts[:, h, ds(0, n_half * S2)])
                    vsl = sb.tile([P, n_half * KC2, HD], BF, tag="vslab",
                                  bufs=2, name="vsl")
                    nc.gpsimd.dma_start(
                        vsl, vs[h, :, ds(0, n_half * KC2), :]
                    )
                    psout = psum.tile([P, QT], F32, tag="acc", bufs=2,
                                      name="psout")
                    psden = psum.tile([1, QT], F32, tag="stat", bufs=2,
                                      name="psden")
                    # software-pipelined slot loop: QK runs PIPE slots ahead
                    # of exp/PV so the in-order PE stream never waits on the
                    # Scalar engine.
                    PIPE = 4
                    pssq = {}
                    ets = {}

                    def issue_qk(s):
                        pssq[s] = psum.tile([P, QT], F32, tag="mm", bufs=4,
                                            name="pssq")
                        nc.tensor.matmul(
                            pssq[s],
                            lhsT=ksl[:, ds(s * P, P)],
                            rhs=qsl,
                        )

                    def issue_exp(s):
                        et = sb.tile([P, QT], BF, tag="exp", bufs=PIPE + 1,
                                     name="et")
                        nc.scalar.activation(et, pssq[s], AF.Exp, bias=nege)
                        del pssq[s]
                        if s in mpos:
                            nc.vector.tensor_tensor(
                                et, et, mt[:, mpos[s]], op=OP.mult
                            )
                        ets[s] = et

                    for s in range(min(PIPE, n_slots)):
                        issue_qk(s)
                    issue_exp(0)
                    for s in range(n_slots):
                        if s + PIPE < n_slots:
                            issue_qk(s + PIPE)
                        if s + 1 < n_slots:
                            issue_exp(s + 1)
                        et = ets.pop(s)
                        nc.tensor.matmul(
                            psout,
                            lhsT=vsl[:, s],
                            rhs=et,
                            start=(s == 0),
                            stop=(s == n_slots - 1),
                        )
                        nc.tensor.matmul(
                            psden,
                            lhsT=ones_cb,
                            rhs=et,
                            start=(s == 0),
                            stop=(s == n_slots - 1),
                        )
                    rec0 = sb.tile([1, QT], F32, tag="stats_sb", bufs=4,
                                   name="rec0")
                    with nc.allow_low_precision(
                        reason="denominator reciprocal, 18 bits is plenty"
                    ):
                        nc.vector.reciprocal_approx_fast(rec0, psden)
                    rec = sb.tile([1, QT], F32, tag="stats_sb", bufs=4,
                                  name="rec")
                    nc.vector.tensor_copy(_r(rec), rec0)
                    psr = psum.tile([P, QT], F32, tag="acc", bufs=2,
                                    name="psr")
                    nc.tensor.matmul(psr, lhsT=_r(ones_row), rhs=_r(rec))
                    rsb = sb.tile([P, QT], BF, tag="rsb", bufs=1, name="rsb")
                    nc.vector.tensor_copy(rsb, psr)
                    nc.vector.tensor_tensor(ots[:, h], psout, rsb, op=OP.mult)

                # ---- Phase E: out-projection for this q tile (from SBUF),
                # 4 psf banks per wo load ----
                EG = 4
                for eg in range(NH // EG):
                    psf = [
                        psum.tile([P, QT], F32, tag="mm", bufs=4, name="psf")
                        for _ in range(EG)
                    ]
                    for h in range(NH):
                        wo = sb.tile([P, EG * P], BF, tag="wo", bufs=3,
                                     name="wo")
                        eng = nc.scalar if h % 2 else nc.gpsimd
                        eng.dma_start(
                            wo, woT[ds(h * P, P), ds(eg * EG * P, EG * P)]
                        )
                        for x in range(EG):
                            nc.tensor.matmul(
                                psf[x],
                                lhsT=wo[:, ds(x * P, P)],
                                rhs=ots[:, h],
                                start=(h == 0),
                                stop=(h == NH - 1),
                            )
                    for x in range(EG):
                        fsb = sb.tile([P, QT], F32, tag="fsb", bufs=2,
                                      name="fsb")
                        nc.vector.tensor_copy(fsb, psf[x])
                        nc.sync.dma_start(
                            out_t[ds((eg * EG + x) * P, P), ds(qsl_off, QT)],
                            fsb,
                        )

    nc.compile()
    return nc


# --------------------------------------------------------------------------
# Host-side prep and driver
# --------------------------------------------------------------------------

def _q_blocks(role, n_blocks):
    """q-block indices (each 512 tokens) for a core role."""
    if n_blocks == 4:
        return [0, 3] if role == 0 else [1, 2]
    return list(range(n_blocks))


def make_host_data(x, w_in, w_out, q_gamma, q_beta, k_gamma, k_beta, cfg,
                   n_cores=None):
    """Build per-core in_maps (list of dicts) + assembly metadata."""
    import ml_dtypes

    BF_NP = ml_dtypes.bfloat16

    D = cfg["D"]
    S = cfg["S"]
    NQTOK = cfg["NQTOK"]
    QT = cfg["QT"]
    slots = cfg["slots"]
    masked = cfg["masked"]
    NH = D // HD
    NQ = NQTOK // QT
    MAXM = max(len(m) for m in masked)
    B = x.shape[0]
    n_blocks = S // 512
    if n_cores is None:
        n_cores = B * (2048 // NQTOK) if S == 2048 else B

    w64 = np.asarray(w_in, np.float64)
    wq = w64[0:D]
    wk = w64[D:2 * D]
    wv = w64[2 * D:3 * D]
    wq_c = wq - wq.mean(axis=0, keepdims=True)
    wk_c = wk - wk.mean(axis=0, keepdims=True)
    wqkT2 = np.concatenate([wq_c.T, wk_c.T], axis=1)
    # pre-tile to [2*NH, P, DC, P]: tile ec -> [p, dc, e] with contiguous rows
    wqkT = np.ascontiguousarray(
        wqkT2.reshape(D // P, P, 2 * (D // P), P).transpose(2, 1, 0, 3)
    ).astype(BF_NP)
    wvT = np.ascontiguousarray(wv.T).astype(BF_NP)
    woT = np.ascontiguousarray(np.asarray(w_out, np.float64).T).astype(BF_NP)

    inv = 1.0 / (10000.0 ** (np.arange(0, HD, 2, dtype=np.float64) / HD))
    tpos = np.arange(S, dtype=np.float64)
    fr = np.outer(tpos, inv)
    emb = np.concatenate([fr, fr], axis=-1)  # [S, HD]
    cosT = np.cos(emb).T  # [HD, S]
    sinTn = np.sin(emb).T

    # signed rotate-half permutation, as matmul lhsT:
    # out[p] = sum_{p'} rotmT[p', p] * in[p'] = rot_half(in)[p]
    h2 = HD // 2
    rotmT = np.zeros((P, P), np.float32)
    for p in range(h2):
        rotmT[p + h2, p] = -1.0
    for p in range(h2, HD):
        rotmT[p - h2, p] = 1.0
    rotmT = rotmT.astype(BF_NP)

    scale = 1.0 / math.sqrt(HD)
    gq_a = np.ascontiguousarray(
        (np.asarray(q_gamma, np.float64) * scale).reshape(NH, P).T
    ).astype(BF_NP)
    bq_a = np.ascontiguousarray(
        (np.asarray(q_beta, np.float64) * scale).reshape(NH, P).T
    ).astype(BF_NP)
    gk_a = np.ascontiguousarray(
        np.asarray(k_gamma, np.float64).reshape(NH, P).T
    ).astype(BF_NP)
    bk_a = np.ascontiguousarray(
        np.asarray(k_beta, np.float64).reshape(NH, P).T
    ).astype(BF_NP)

    in_maps = []
    meta = []
    cores_per_batch = max(1, n_cores // B)
    for c in range(n_cores):
        b = c // cores_per_batch
        r = c % cores_per_batch
        blocks = _q_blocks(r if cores_per_batch > 1 else 0, n_blocks)
        blocks = blocks[: NQTOK // 512]
        qtok = np.concatenate(
            [np.arange(bk_ * 512, (bk_ + 1) * 512) for bk_ in blocks]
        )
        xb = np.asarray(x[b], np.float32)  # [S, D]
        xT = np.ascontiguousarray(xb.T).astype(BF_NP)    # [D, S]
        xTq = np.ascontiguousarray(xT[:, qtok])
        cosq = np.ascontiguousarray(cosT[:, qtok]).astype(BF_NP)
        sinq = np.ascontiguousarray(sinTn[:, qtok]).astype(BF_NP)
        cosk = np.ascontiguousarray(cosT[:, :S]).astype(BF_NP)
        sink = np.ascontiguousarray(sinTn[:, :S]).astype(BF_NP)

        masks = np.zeros([NQ, P, MAXM, QT], np.float32)
        for t in range(NQ):
            assert QT == 512
            q_start = blocks[t] * 512
            qq = np.arange(QT)
            kk = np.arange(P)
            for mi, kc in enumerate(masked[t]):
                masks[t, :, mi, :] = (
                    (kc * P + kk[:, None]) <= (q_start + qq[None, :])
                ).astype(np.float32)
        masks = masks.astype(BF_NP)

        in_maps.append(dict(
            xTq=xTq, xT=xT, wqkT=wqkT, wvT=wvT, woT=woT,
            cosq=cosq, sinqn=sinq, cosk=cosk, sinkn=sink,
            gq=gq_a, bq=bq_a, gk=gk_a, bk=bk_a, masks=masks,
            rotm=rotmT,
            onesc=np.ones((P, 1), np.float32),
            onesr=np.ones((1, P), np.float32),
        ))
        meta.append(dict(b=b, qtok=qtok))
    return in_maps, meta


_PROGRAM_CACHE = {}


def _get_program(cfg_key, cfg, skip_gb=False):
    if cfg_key not in _PROGRAM_CACHE:
        _PROGRAM_CACHE[cfg_key] = build_program(cfg, skip_gb=skip_gb)
    return _PROGRAM_CACHE[cfg_key]


def run_full(x, w_in, w_out, q_gamma, q_beta, k_gamma, k_beta,
             trace=False):
    from concourse.bass_utils import run_bass_kernel_spmd

    cfg = FULL_CFG
    B = x.shape[0]
    n_cores = 2 * B
    in_maps, meta = make_host_data(
        x, w_in, w_out, q_gamma, q_beta, k_gamma, k_beta, cfg,
        n_cores=n_cores,
    )
    skip_gb = bool(
        np.all(np.asarray(q_gamma) == 1.0) and np.all(np.asarray(q_beta) == 0.0)
        and np.all(np.asarray(k_gamma) == 1.0)
        and np.all(np.asarray(k_beta) == 0.0)
    )
    nc = _get_program(("full", skip_gb), cfg, skip_gb)
    res = run_bass_kernel_spmd(
        nc, in_maps, core_ids=list(range(n_cores)), trace=trace,
    )
    S, D = cfg["S"], cfg["D"]
    out = np.empty((B, S, D), np.float32)
    for c in range(n_cores):
        o = res.results[c]["out"]  # [D, NQTOK]
        out[meta[c]["b"], meta[c]["qtok"], :] = o.T
    return out, res


def kernel(x, w_in, w_out, q_gamma, q_beta, k_gamma, k_beta, n_heads=16,
           **_ignored):
    x = np.asarray(x, np.float32)
    assert int(np.asarray(n_heads)) * HD == x.shape[-1]
    out, _ = run_full(
        np.asarray(x, np.float32),
        np.asarray(w_in, np.float32),
        np.asarray(w_out, np.float32),
        np.asarray(q_gamma, np.float32),
        np.asarray(q_beta, np.float32),
        np.asarray(k_gamma, np.float32),
        np.asarray(k_beta, np.float32),
    )
    return out
